# revision 23
# baseline (speedup 1.0000x reference)
"""Trainium2 Bass kernel: boson-sampler probabilities via Glynn's permanent formula.

Math (per 18x18 complex matrix A):
  perm(A) = 2^(1-n) * sum_{d in {+-1}^n, d_0=+1} (prod_k d_k) * prod_i (sum_j d_j A[i,j])
The 2^17 sign vectors form a [128 x 1024] grid (7 "p" bits drive columns 11..17,
10 "f" bits drive columns 1..10; column 0 fixed +1). Row-sums factor as
rs_i = RP_i(p) + RF_i(f); rows are grouped [6,6,6] and each group's product
expands as T_g[p,f] = sum_{c<64} G_g[c,p] * H_g[c,f] -- fp32r matmuls on the
tensor engine (full rate at free>=256). The 64-row G/H tables (all sub-products
of 6 rows) are built hierarchically (pairs -> quads -> tables): packed operand
sets are assembled from SBUF "master" row tiles by 0/1 selection matmuls, and
each level is a set of elementwise complex-multiply ops column-split across
the DVE and GPSIMD engines. Glynn parity signs are folded into group 2's
tables via sign-scaled mask constants. G-side (p-axis) work packs real|imag
side by side in one [32, 256] master so every G matmul runs at free=256.
The final sum(T0*T1*T2) reduces via 8 scalar_tensor_tensor accumulations
(col-split DVE/Pool); |perm|^2, the (underflowed-to-zero) classical term and
the dark-count offset are applied on the host. One NeuronCore per batch
element.
"""

import sys

sys.path.insert(0, "/opt/trn_rl_repo")

import numpy as np

import concourse.bacc as bacc
import concourse.bass as bass
import concourse.tile as tile
from concourse import mybir
from concourse.bass_utils import run_bass_kernel_spmd

FP32 = mybir.dt.float32
FP32R = mybir.dt.float32r
OP = mybir.AluOpType

N = 18
PBITS, FBITS = 7, 10
P, F = 1 << PBITS, 1 << FBITS          # 128, 1024
EMU = 0.85 * (1 - 0.02) * (1 - 0.02) * (1 - 0.01)
DARK = 1e-6 * N
SCALE2 = float(2.0 ** (2 * (1 - N)))

# master row map (same for both sides; imag comp has zeros at ONES/SIGN)
M_RF = 0          # rows 0..17: row-sums RF_i / RP_i
M_SE = 18         # sign-scaled even row of pair 8 (RFe' / RPe')
M_SO = 19         # sign-scaled odd row (RFo' / RPo')
M_ONE = 20        # ones (real) / zeros (imag)
M_SGN = 21        # sign row sF / sP (real) / zeros (imag)
M_PP = 32         # rows 32..41: pair products PP_q (row 41 = PP' scaled)
M_PPS = 41        # row 41: PP' = sign-scaled pair-8 product (32-aligned for GPSIMD)
M_PAD = 42
CROWS = 48        # const-pack row count (REPA/REPB lhsTs span 48 rows)

# column split: DVE takes [0:x], GPSIMD takes [x:F] of each wide stage
FS_P1 = 704       # pair / L1 stages
FS_L2 = 608       # L2 table-build stages
FS_PF = 608       # p01 / final reduction stages

_CACHE = {}


def _pm_mask(nvals, bits):
    v = np.arange(nvals, dtype=np.uint32)
    m = (v[:, None] >> np.arange(bits, dtype=np.uint32)[None, :]) & 1
    return (1.0 - 2.0 * m).astype(np.float32).T.copy()   # [bits, nvals]


def _parity(nvals, bits):
    v = np.arange(nvals, dtype=np.uint32)
    pc = np.zeros(nvals, dtype=np.uint32)
    for k in range(bits):
        pc += (v >> k) & 1
    return np.where(pc % 2 == 0, 1.0, -1.0).astype(np.float32)


class SideSpec:
    """H: f-side (width 1024, pair-table identity at z=3);
       G: p-side (width 128, identity at z=0)."""

    def __init__(self, name, width, mult_z):
        self.name = name
        self.w = width
        self.mult_z = list(mult_z)
        self.idz = ({0, 1, 2, 3} - set(mult_z)).pop()

    # pair-table entry -> master row (pair q, entry z); identity z -> ones row
    def tab_row(self, q, z):
        if z == self.idz:
            return M_ONE
        if self.name == "H":
            return {0: M_PP + q, 1: 2 * q + 1, 2: 2 * q}[z]
        return {1: 2 * q, 2: 2 * q + 1, 3: M_PP + q}[z]

    # group-2 scaled pair-table entry (pair 8) -> master row
    def tab2_row(self, z):
        if z == self.idz:
            return M_SGN
        if self.name == "H":
            return {0: M_PPS, 1: M_SO, 2: M_SE}[z]
        return {1: M_SE, 2: M_SO, 3: M_PPS}[z]


HSPEC = SideSpec("H", F, (0, 1, 2))
GSPEC = SideSpec("G", P, (1, 2, 3))


def _sel(rows, m_pad=None):
    """Selection matrix [M_PAD, len(rows)] with one 1 per used column."""
    M = len(rows) if m_pad is None else m_pad
    s = np.zeros((M_PAD, M), np.float32)
    for m, k in enumerate(rows):
        if k is not None:
            s[k, m] = 1.0
    return s


# const pack column layout: computed once at import
def _build_pack():
    cols = {}
    blocks = []
    off = 0

    def add(name, arr):
        nonlocal off
        a = np.zeros((CROWS, arr.shape[1]), np.float32)
        a[0:arr.shape[0], :] = arr
        cols[name] = (off, arr.shape[1])
        blocks.append(a)
        off += arr.shape[1]

    # MFX: [ones;pm(10) | (ones;pm)*sF] (row 0 = ones source, row 11 = sF source)
    mF = np.concatenate([np.ones((1, F), np.float32), _pm_mask(F, FBITS)], axis=0)
    sF = _parity(F, FBITS)
    add("MFX", np.concatenate([mF, mF * sF[None, :]], axis=0))           # [22, F]
    # MPX: [pm(7) | pm*sP | ones | sP]
    mP = _pm_mask(P, PBITS)
    sP = _parity(P, PBITS)
    add("MPX", np.concatenate(
        [mP, mP * sP[None, :], np.ones((1, P), np.float32), sP[None, :]], axis=0))
    # unit columns for the widened RS matmul lhsT (cols 20,21), per side
    uh = np.zeros((22, 2), np.float32)
    uh[0, 0] = 1.0      # -> MFX row 0 (ones)
    uh[11, 1] = 1.0     # -> MFX row 11 (sF)
    ug = np.zeros((16, 2), np.float32)
    ug[14, 0] = 1.0     # -> MPX row 14 (ones)
    ug[15, 1] = 1.0     # -> MPX row 15 (sP)
    cols["_U"] = (uh, ug)

    def digits(c):
        return c % 4, (c // 4) % 4, c // 16          # za, zb, zc

    for spec in (HSPEC, GSPEC):
        nm = spec.name
        # L1: 48 distinct products tmp48 indexed by (g, c2), c2 = za + 4zb:
        # in0/in1 packed [48 | pad | 48] in one mm
        in0 = []
        in1 = []
        for g in range(3):
            for c2 in range(16):
                in0.append(spec.tab_row(3 * g, c2 % 4))
                in1.append(spec.tab_row(3 * g + 1, c2 // 4))
        add(f"SELL1_{nm}", _sel(in0 + [None] * 16 + in1, 112))
        # REP: replicate tmp48 rows into L2 src layout (lhsT for rep matmuls)
        # repA: [64g + c] <- tmp48[16g + (c % 16)] for g=0,1 ; repB: g=2
        repA = np.zeros((48, 128), np.float32)
        for g in range(2):
            for c in range(64):
                repA[16 * g + (c % 16), 64 * g + c] = 1.0
        add(f"REPA_{nm}", repA)
        repB = np.zeros((48, 64), np.float32)
        for c in range(64):
            repB[32 + (c % 16), c] = 1.0
        add(f"REPB_{nm}", repB)
        # L2 c-packs: set1 rows [64g+c] = tab_c[zc]; set2 = scaled tab2'[zc]
        rows = []
        for g in range(2):
            for c in range(64):
                rows.append(spec.tab_row(3 * g + 2, digits(c)[2]))
        add(f"SELL2A_{nm}", _sel(rows))
        rows = [spec.tab2_row(digits(c)[2]) for c in range(64)]
        add(f"SELL2B_{nm}", _sel(rows))
    return np.concatenate(blocks, axis=1), cols


CPACK, CPACK_COLS = _build_pack()
U_HG = CPACK_COLS.pop("_U")
CW = CPACK.shape[1]


def build_lts(Ar, Ai):
    """Host-side lhsT pack: per (side, comp) the widened row-sum lhsT
    (32 cols) and the pair-pack lhsT (42 cols), one [22, 296] tensor."""
    lts = np.zeros((22, 296), np.float32)
    for si, (lo, kb, K) in enumerate(((0, 11, 22), (11, 7, 16))):
        for ci, A in ((0, Ar), (1, Ai)):
            base = 74 * (2 * si + ci)
            AT = np.ascontiguousarray(A.T, dtype=np.float32)
            lts[0:kb, base:base + 18] = AT[lo:lo + kb, 0:18]
            lts[kb:2 * kb, base + 18:base + 20] = AT[lo:lo + kb, 16:18]
            if ci == 0:
                lts[0:K, base + 20:base + 22] = U_HG[si]
            pb = base + 32
            lts[0:kb, pb:pb + 9] = AT[lo:lo + kb, 0:17:2]
            lts[kb:2 * kb, pb + 9] = AT[lo:lo + kb, 16]
            lts[0:kb, pb + 32:pb + 41] = AT[lo:lo + kb, 1:18:2]
            lts[0:kb, pb + 41] = AT[lo:lo + kb, 17]
    return lts


def make_in_map(Ar, Ai):
    return {"CPACK": CPACK, "LTS": build_lts(Ar, Ai)}


def host_consts():
    return {"CPACK": CPACK}


# ---------------------------------------------------------------- kernel body
def build_kernel(loop_iters=None):
    nc = bacc.Bacc("TRN2", target_bir_lowering=False, debug=False)

    tens = {}
    tens["LTS"] = nc.dram_tensor("LTS", [22, 296], FP32, kind="ExternalInput").ap()
    tens["CPACK"] = nc.dram_tensor("CPACK", [CROWS, CW], FP32,
                                   kind="ExternalInput").ap()
    tens["OUT"] = nc.dram_tensor("OUT", [128, 4], FP32, kind="ExternalOutput").ap()

    with tile.TileContext(nc) as tc:
        if loop_iters is None:
            _body(nc, tc, tens)
        else:
            with tc.For_i(0, loop_iters, 1):
                _body(nc, tc, tens)
    nc.compile()
    return nc


def _body(nc, tc, tens):
    from contextlib import ExitStack

    ctx = ExitStack()
    pers = ctx.enter_context(tc.tile_pool(name="pers", bufs=1))
    pk = ctx.enter_context(tc.tile_pool(name="pk", bufs=2))
    cm = ctx.enter_context(tc.tile_pool(name="cm", bufs=2))
    psum_pool = ctx.enter_context(tc.tile_pool(name="psum", bufs=4, space="PSUM"))
    dma = nc.sync.dma_start
    dma2 = nc.gpsimd.dma_start          # SWDGE queues for small input loads

    def mmr(out_ap, lhsT_ap, rhs_ap, start=True, stop=True):
        """fp32r matmul: full rate (1 cyc/row) on trn2 when free >= 256."""
        nc.tensor.matmul(out_ap, lhsT_ap.bitcast(FP32R), rhs_ap.bitcast(FP32R),
                         start=start, stop=stop)

    def cmul6(rows, w, i0, i1, outr, outi):
        """DVE complex multiply: (i0r,i0i)*(i1r,i1i) -> (outr,outi)."""
        e = nc.vector
        i0r, i0i = i0
        i1r, i1i = i1
        t1 = cm.tile([rows, w], FP32, tag="cm_t1")
        t2 = cm.tile([rows, w], FP32, tag="cm_t2")
        e.tensor_mul(t1[:], i0r, i1r)
        e.tensor_mul(t2[:], i0i, i1i)
        e.tensor_sub(outr, t1[:], t2[:])
        e.tensor_mul(t1[:], i0r, i1i)
        e.tensor_mul(t2[:], i0i, i1r)
        e.tensor_add(outi, t1[:], t2[:])

    def cmul6p(rows, w, i0, i1, outr, outi):
        """GPSIMD complex multiply (plain tensor-tensor ops)."""
        e = nc.gpsimd
        i0r, i0i = i0
        i1r, i1i = i1
        t1 = cm.tile([rows, w], FP32, tag="gp_t1")
        t2 = cm.tile([rows, w], FP32, tag="gp_t2")
        e.tensor_mul(t1[:], i0r, i1r)
        e.tensor_mul(t2[:], i0i, i1i)
        e.tensor_sub(outr, t1[:], t2[:])
        e.tensor_mul(t1[:], i0r, i1i)
        e.tensor_mul(t2[:], i0i, i1r)
        e.tensor_add(outi, t1[:], t2[:])

    def sel_mm(sel_sb, msrc, m, w):
        """Pack = SEL.T @ master -> PSUM [m, w]."""
        ps = psum_pool.tile([m, w], FP32, tag="ps")
        for c0 in range(0, w, 512):
            c1 = min(c0 + 512, w)
            nc.tensor.matmul(ps[:, c0:c1], sel_sb[:], msrc[:, c0:c1],
                             start=True, stop=True)
        return ps

    # ---- stage 0: A loads, widened row-sum matmuls -> master rows 0..21
    cpk = pers.tile([CROWS, CW], FP32, tag="cpack")
    th1 = (CW // 3) & ~3
    th2 = (2 * CW // 3) & ~3
    dma(cpk[:, 0:th1], tens["CPACK"][:, 0:th1])
    nc.scalar.dma_start(cpk[:, th1:th2], tens["CPACK"][:, th1:th2])
    dma2(cpk[:, th2:CW], tens["CPACK"][:, th2:CW])

    def cslice(name, nrows=None):
        off, width = CPACK_COLS[name]
        nr = M_PAD if nrows is None else nrows
        return cpk[0:nr, off:off + width]

    lts = pers.tile([22, 296], FP32, tag="lts")
    dma(lts[:], tens["LTS"][:, :])
    lhsT_rs = {}
    lhsT_pp = {}
    KRS = {"H": 22, "G": 16}
    for si, side in enumerate("HG"):
        for ci, nm in enumerate("ri"):
            base = 74 * (2 * si + ci)
            K_rs = KRS[side]
            lhsT_rs[(side, nm)] = lts[0:K_rs, base:base + 32]
            lhsT_pp[(side, nm)] = lts[0:K_rs, base + 32:base + 74]

    mask_sb = {"H": cslice("MFX", 22), "G": cslice("MPX", 16)}
    sel_sb = {}
    for spec in (HSPEC, GSPEC):
        for s in ("SELL1", "SELL2A", "SELL2B"):
            key = f"{s}_{spec.name}"
            sel_sb[key] = cslice(key)
        for s in ("REPA", "REPB"):
            key = f"{s}_{spec.name}"
            sel_sb[key] = cslice(key, 48)

    # H masters: one [M_PAD, F] tile per component; G master: [M_PAD, 2P]
    # with real in cols 0:P, imag in P:2P. All 32 rows get written (22 by
    # the RS copy, 10 by the pair stage) -- no memset needed.
    masterH = {}
    for nm in "ri":
        t = pers.tile([M_PAD, F], FP32, tag=f"mstH{nm}", name=f"mstH{nm}")
        masterH[nm] = t
    masterG = pers.tile([M_PAD, 2 * P], FP32, tag="mstG")

    # ---- stage 1: pair products -> master rows 32..41 (packs come
    # straight from the rearranged ltp lhsTs -- no master dependency)
    P1T = F - FS_P1
    psH1 = {}
    for nm in "ri":
        ps = sel_mm(lhsT_pp[("H", nm)], mask_sb["H"], 42, F)
        sb = pk.tile([10, F], FP32, tag=f"halfH{nm}")
        nc.scalar.copy(sb[:], ps[32:42, :])
        p0t = pk.tile([10, P1T], FP32, tag=f"p0tH{nm}")
        nc.scalar.copy(p0t[:], ps[0:10, FS_P1:F])
        psH1[nm] = (ps, sb, p0t)
    cmul6(10, FS_P1,
          (psH1["r"][0][0:10, 0:FS_P1], psH1["i"][0][0:10, 0:FS_P1]),
          (psH1["r"][1][:, 0:FS_P1], psH1["i"][1][:, 0:FS_P1]),
          masterH["r"][M_PP:M_PP + 10, 0:FS_P1],
          masterH["i"][M_PP:M_PP + 10, 0:FS_P1])
    cmul6p(10, P1T,
           (psH1["r"][2][:], psH1["i"][2][:]),
           (psH1["r"][1][:, FS_P1:F], psH1["i"][1][:, FS_P1:F]),
           masterH["r"][M_PP:M_PP + 10, FS_P1:F],
           masterH["i"][M_PP:M_PP + 10, FS_P1:F])

    psG1 = psum_pool.tile([42, 2 * P], FP32, tag="ps")
    nc.tensor.matmul(psG1[:, 0:P], lhsT_pp[("G", "r")], mask_sb["G"][:],
                     start=True, stop=True)
    nc.tensor.matmul(psG1[:, P:2 * P], lhsT_pp[("G", "i")], mask_sb["G"][:],
                     start=True, stop=True)
    sbG1a = pk.tile([10, 2 * P], FP32, tag="selpGa")
    nc.scalar.copy(sbG1a[:], psG1[0:10, :])
    sbG1b = pk.tile([10, 2 * P], FP32, tag="selpGb")
    nc.scalar.copy(sbG1b[:], psG1[32:42, :])
    cmul6p(10, P,
           (sbG1a[:, 0:P], sbG1a[:, P:2 * P]),
           (sbG1b[:, 0:P], sbG1b[:, P:2 * P]),
           masterG[M_PP:M_PP + 10, 0:P], masterG[M_PP:M_PP + 10, P:2 * P])

    # row-sum masters (needed from L1 onward; emitted after the pair
    # stage so its PSUM evacuations win the ACT queue early)
    for nm in "ri":
        lt = lhsT_rs[("H", nm)]
        ps = psum_pool.tile([32, F], FP32, tag="ps")
        for c0 in range(0, F, 512):
            c1 = min(c0 + 512, F)
            nc.tensor.matmul(ps[:, c0:c1], lt, mask_sb["H"][:, c0:c1],
                             start=True, stop=True)
        nc.scalar.copy(masterH[nm][0:32, :], ps[:])
    psG = psum_pool.tile([32, 2 * P], FP32, tag="ps")
    nc.tensor.matmul(psG[:, 0:P], lhsT_rs[("G", "r")], mask_sb["G"][:],
                     start=True, stop=True)
    nc.tensor.matmul(psG[:, P:2 * P], lhsT_rs[("G", "i")], mask_sb["G"][:],
                     start=True, stop=True)
    nc.scalar.copy(masterG[0:32, :], psG[:])


    # ---- stage 2 (L1): tmp48[16g + c2] = tab_a[za] * tab_b[zb]
    t48H = {}
    l1t = {}
    for nm in "ri":
        ps = sel_mm(sel_sb["SELL1_H"], masterH[nm][:], 112, F)
        sb1 = pk.tile([48, F], FP32, tag=f"l1hH{nm}")
        nc.scalar.copy(sb1[:], ps[64:112, :])
        lt1 = pk.tile([48, P1T], FP32, tag=f"l1tH{nm}")
        nc.scalar.copy(lt1[:], ps[0:48, FS_P1:F])
        l1t[nm] = lt1
        t48 = pers.tile([48, F], FP32, tag=f"t48H{nm}", name=f"t48H{nm}")
        t48H[nm] = (ps, sb1, t48)
    cmul6(48, FS_P1,
          (t48H["r"][0][0:48, 0:FS_P1], t48H["i"][0][0:48, 0:FS_P1]),
          (t48H["r"][1][:, 0:FS_P1], t48H["i"][1][:, 0:FS_P1]),
          t48H["r"][2][:, 0:FS_P1], t48H["i"][2][:, 0:FS_P1])
    cmul6p(48, P1T,
           (l1t["r"][:], l1t["i"][:]),
           (t48H["r"][1][:, FS_P1:F], t48H["i"][1][:, FS_P1:F]),
           t48H["r"][2][:, FS_P1:F], t48H["i"][2][:, FS_P1:F])

    psL1G = sel_mm(sel_sb["SELL1_G"], masterG[:], 112, 2 * P)
    sbL1Ga = pk.tile([48, 2 * P], FP32, tag="l1Ga")
    nc.scalar.copy(sbL1Ga[:], psL1G[0:48, :])
    sbL1Gb = pk.tile([48, 2 * P], FP32, tag="l1Gb")
    nc.scalar.copy(sbL1Gb[:], psL1G[64:112, :])
    t48G = pers.tile([48, 2 * P], FP32, tag="t48G", name="t48G")
    cmul6p(48, P,
           (sbL1Ga[:, 0:P], sbL1Ga[:, P:2 * P]),
           (sbL1Gb[:, 0:P], sbL1Gb[:, P:2 * P]),
           t48G[:, 0:P], t48G[:, P:2 * P])

    # ---- stage 3 (L2): e_g = tmp * tab_c[zc], column-split DVE / GPSIMD
    # H g0+g1 fused as one [128, F] set -> eRH = [H0r; H1r], eIH = [H0i; H1i].
    # The T matmuls compensate with K=64 accumulating pairs.
    TL = F - FS_L2
    c01H = {}
    repH = {}
    reptH = {}
    for nm in "ri":
        c01 = sel_mm(sel_sb["SELL2A_H"], masterH[nm][:], 128, F)
        c01sb = pk.tile([128, F], FP32, tag=f"c01H{nm}")
        nc.scalar.copy(c01sb[:], c01[:])
        c01H[nm] = c01sb
        rep = sel_mm(sel_sb["REPA_H"], t48H[nm][2][:], 128, F)
        repH[nm] = rep
        rt = pk.tile([128, TL], FP32, tag=f"reptH{nm}")
        nc.scalar.copy(rt[:], repH[nm][:, FS_L2:F])
        reptH[nm] = rt
    eRH = pers.tile([128, F], FP32R, tag="eRH", name="eRH")
    eIH = pers.tile([128, F], FP32R, tag="eIH", name="eIH")
    cmul6(128, FS_L2,
          (repH["r"][0:128, 0:FS_L2], repH["i"][0:128, 0:FS_L2]),
          (c01H["r"][:, 0:FS_L2], c01H["i"][:, 0:FS_L2]),
          eRH[:, 0:FS_L2], eIH[:, 0:FS_L2])
    cmul6p(128, TL,
           (reptH["r"][:], reptH["i"][:]),
           (c01H["r"][:, FS_L2:F], c01H["i"][:, FS_L2:F]),
           eRH[:, FS_L2:F], eIH[:, FS_L2:F])

    cp2H = {}
    rbH = {}
    rbtH = {}
    for nm in "ri":
        c2 = sel_mm(sel_sb["SELL2B_H"], masterH[nm][:], 64, F)
        c2sb = pk.tile([64, F], FP32, tag=f"c2H{nm}")
        nc.scalar.copy(c2sb[:], c2[:])
        cp2H[nm] = c2sb
        rb = sel_mm(sel_sb["REPB_H"], t48H[nm][2][:], 64, F)
        rbH[nm] = rb
        rbt = pk.tile([64, TL], FP32, tag=f"rbtH{nm}")
        nc.scalar.copy(rbt[:], rb[:, FS_L2:F])
        rbtH[nm] = rbt
    eR2H = pers.tile([64, F], FP32R, tag="eR2H", name="eR2H")
    eI2H = pers.tile([64, F], FP32R, tag="eI2H", name="eI2H")
    cmul6(64, FS_L2,
          (rbH["r"][:, 0:FS_L2], rbH["i"][:, 0:FS_L2]),
          (cp2H["r"][:, 0:FS_L2], cp2H["i"][:, 0:FS_L2]),
          eR2H[:, 0:FS_L2], eI2H[:, 0:FS_L2])
    cmul6p(64, TL,
           (rbtH["r"][:], rbtH["i"][:]),
           (cp2H["r"][:, FS_L2:F], cp2H["i"][:, FS_L2:F]),
           eR2H[:, FS_L2:F], eI2H[:, FS_L2:F])

    # G side (GPSIMD, SBUF operands via single ACT evacuations)
    repG = sel_mm(sel_sb["REPA_G"], t48G[:], 128, 2 * P)
    repGsb = pk.tile([128, 2 * P], FP32, tag="repG")
    nc.scalar.copy(repGsb[:], repG[:])
    c01G = sel_mm(sel_sb["SELL2A_G"], masterG[:], 128, 2 * P)
    c01Gsb = pk.tile([128, 2 * P], FP32, tag="c01G")
    nc.scalar.copy(c01Gsb[:], c01G[:])
    eG01 = pers.tile([128, 2 * P], FP32R, tag="eG01", name="eG01")
    cmul6p(128, P,
           (repGsb[:, 0:P], repGsb[:, P:2 * P]),
           (c01Gsb[:, 0:P], c01Gsb[:, P:2 * P]),
           eG01[:, 0:P], eG01[:, P:2 * P])

    rbG = sel_mm(sel_sb["REPB_G"], t48G[:], 64, 2 * P)
    rbGsb = pk.tile([64, 2 * P], FP32, tag="rbG")
    nc.scalar.copy(rbGsb[:], rbG[:])
    c2G = sel_mm(sel_sb["SELL2B_G"], masterG[:], 64, 2 * P)
    c2Gsb = pk.tile([64, 2 * P], FP32, tag="c2G")
    nc.scalar.copy(c2Gsb[:], c2G[:])
    eG2 = pers.tile([64, 2 * P], FP32R, tag="eG2", name="eG2")
    cmul6p(64, P,
           (rbGsb[:, 0:P], rbGsb[:, P:2 * P]),
           (c2Gsb[:, 0:P], c2Gsb[:, P:2 * P]),
           eG2[:, 0:P], eG2[:, P:2 * P])

    # negated imag halves (lhsT for the real-part T matmuls)
    negG01 = pers.tile([128, P], FP32R, tag="negG01")
    nc.scalar.mul(negG01[:], eG01[:, P:2 * P], -1.0)
    negG2 = pers.tile([64, P], FP32R, tag="negG2")
    nc.scalar.mul(negG2[:], eG2[:, P:2 * P], -1.0)

    # ---- stage 4: T matmuls -- per (group, comp, chunk) a K=64 pair
    # accumulated in PSUM: Tr = Gr^T Hr + (-Gi)^T Hi ; Ti = Gr^T Hi + Gi^T Hr
    # Order: T0, T1 (combine inputs) first, then T2 (only needed by the
    # final reduction) so p01 overlaps the T2 matmuls.
    def t_mms(g):
        if g < 2:
            Gr = eG01[64 * g:64 * g + 64, 0:P]
            Gi = eG01[64 * g:64 * g + 64, P:2 * P]
            Gin = negG01[64 * g:64 * g + 64, :]
            Hr = eRH[64 * g:64 * g + 64, :]
            Hi = eIH[64 * g:64 * g + 64, :]
        else:
            Gr = eG2[:, 0:P]
            Gi = eG2[:, P:2 * P]
            Gin = negG2[:]
            Hr = eR2H[:]
            Hi = eI2H[:]
        tr = psum_pool.tile([P, F], FP32, tag="ps")
        ti = psum_pool.tile([P, F], FP32, tag="ps")
        for c0 in range(0, F, 512):
            c1 = c0 + 512
            mmr(tr[:, c0:c1], Gr, Hr[:, c0:c1], start=True, stop=False)
            mmr(tr[:, c0:c1], Gin, Hi[:, c0:c1], start=False, stop=True)
            mmr(ti[:, c0:c1], Gr, Hi[:, c0:c1], start=True, stop=False)
            mmr(ti[:, c0:c1], Gi, Hr[:, c0:c1], start=False, stop=True)
        return tr, ti

    t0r, t0i = t_mms(0)
    t1r_ps, t1i_ps = t_mms(1)
    t1r = pers.tile([P, F], FP32, tag="T1r")
    t1i = pers.tile([P, F], FP32, tag="T1i")
    for c0 in range(0, F, 512):
        c1 = c0 + 512
        nc.scalar.copy(t1r[:, c0:c1], t1r_ps[:, c0:c1])
        nc.scalar.copy(t1i[:, c0:c1], t1i_ps[:, c0:c1])

    # p01 = T0*T1: col-split DVE (T0 from PSUM) / GPSIMD (T0 tail via ACT)
    TP = F - FS_PF
    p01r = pers.tile([P, F], FP32, tag="p01r")
    p01i = pers.tile([P, F], FP32, tag="p01i")
    t0tr = pers.tile([P, TP], FP32, tag="t0tr")
    t0ti = pers.tile([P, TP], FP32, tag="t0ti")
    nc.scalar.copy(t0tr[:], t0r[:, FS_PF:F])
    nc.scalar.copy(t0ti[:], t0i[:, FS_PF:F])
    cmul6(P, FS_PF,
          (t0r[:, 0:FS_PF], t0i[:, 0:FS_PF]),
          (t1r[:, 0:FS_PF], t1i[:, 0:FS_PF]),
          p01r[:, 0:FS_PF], p01i[:, 0:FS_PF])
    cmul6p(P, TP,
           (t0tr[:], t0ti[:]),
           (t1r[:, FS_PF:F], t1i[:, FS_PF:F]),
           p01r[:, FS_PF:F], p01i[:, FS_PF:F])

    t2r, t2i = t_mms(2)

    # ---- final reduction: acc[p, k] = sum_f p01 * T2 products (DVE,
    # full width, T2 straight from PSUM).
    # Host combines: perm_r = c0 - c1, perm_i = c2 + c3
    scr2 = pers.tile([P, F], FP32, tag="ttr_scr")
    accD = pers.tile([P, 4], FP32, tag="accD")
    pairs = [(p01r, t2r), (p01i, t2i), (p01r, t2i), (p01i, t2r)]
    for k, (a, b) in enumerate(pairs):
        nc.vector.scalar_tensor_tensor(
            out=scr2[:], in0=b[:], scalar=1.0, in1=a[:],
            op0=OP.mult, op1=OP.mult, accum_out=accD[:, k:k + 1])

    dma(tens["OUT"][:, 0:4], accD[:])

    ctx.close()


# ---------------------------------------------------------------- entry point
def kernel(A_real: np.ndarray, A_imag: np.ndarray) -> np.ndarray:
    B = A_real.shape[0]
    assert B == 8 and A_real.shape == (B, N, N)
    if "nc" not in _CACHE:
        _CACHE["nc"] = build_kernel()
    nc = _CACHE["nc"]
    in_maps = [make_in_map(A_real[b], A_imag[b]) for b in range(B)]
    res = run_bass_kernel_spmd(nc, in_maps, list(range(B)))
    out = np.empty(B, dtype=np.float32)
    for b in range(B):
        acc = res.results[b]["OUT"].reshape(128, 4).astype(np.float64)
        s = acc.sum(axis=0)
        pr = s[0] - s[1]
        pi = s[2] + s[3]
        pa2 = np.float32(pr) ** 2 + np.float32(pi) ** 2
        out[b] = np.float32(EMU * SCALE2 * pa2 + DARK)
    return out


if __name__ == "__main__":
    A_real = np.load("/tmp/A_real.npy")
    A_imag = np.load("/tmp/A_imag.npy")
    print(kernel(A_real, A_imag))


# revision 31
# speedup vs baseline: 2.6468x; 2.6468x over previous
"""Trainium2 Bass kernel: boson-sampler probabilities via Glynn's permanent formula.

Math (per 18x18 complex matrix A):
  perm(A) = 2^(1-n) * sum_{d in {+-1}^n, d_0=+1} (prod_k d_k) * prod_i (sum_j d_j A[i,j])
The 2^17 sign vectors form a [128 x 1024] grid (7 "p" bits drive columns 11..17,
10 "f" bits drive columns 1..10; column 0 fixed +1). Row-sums factor as
rs_i = RP_i(p) + RF_i(f); rows are grouped [6,6,6] and each group's product
expands as T_g[p,f] = sum_{c<64} G_g[c,p] * H_g[c,f] -- fp32r matmuls on the
tensor engine (full rate at free>=256). The 64-row G/H tables (all sub-products
of 6 rows) are built hierarchically (pairs -> quads -> tables): packed operand
sets are assembled from SBUF "master" row tiles by 0/1 selection matmuls, and
each level is a set of elementwise complex-multiply ops column-split across
the DVE and GPSIMD engines. Glynn parity signs are folded into group 2's
tables via sign-scaled mask constants. G-side (p-axis) work packs real|imag
side by side in one [32, 256] master so every G matmul runs at free=256.
The final sum(T0*T1*T2) reduces via 8 scalar_tensor_tensor accumulations
(col-split DVE/Pool); |perm|^2, the (underflowed-to-zero) classical term and
the dark-count offset are applied on the host. One NeuronCore per batch
element.
"""

import sys

sys.path.insert(0, "/opt/trn_rl_repo")

import numpy as np

import concourse.bacc as bacc
import concourse.bass as bass
import concourse.tile as tile
from concourse import mybir
from concourse.bass_utils import run_bass_kernel_spmd

FP32 = mybir.dt.float32
FP32R = mybir.dt.float32r
OP = mybir.AluOpType

N = 18
PBITS, FBITS = 7, 10
P, F = 1 << PBITS, 1 << FBITS          # 128, 1024
EMU = 0.85 * (1 - 0.02) * (1 - 0.02) * (1 - 0.01)
DARK = 1e-6 * N
SCALE2 = float(2.0 ** (2 * (1 - N)))

# master row map (same for both sides; imag comp has zeros at ONES/SIGN)
M_RF = 0          # rows 0..17: row-sums RF_i / RP_i
M_SE = 18         # sign-scaled even row of pair 8 (RFe' / RPe')
M_SO = 19         # sign-scaled odd row (RFo' / RPo')
M_ONE = 20        # ones (real) / zeros (imag)
M_SGN = 21        # sign row sF / sP (real) / zeros (imag)
M_PP = 32         # rows 32..41: pair products PP_q (row 41 = PP' scaled)
M_PPS = 41        # row 41: PP' = sign-scaled pair-8 product (32-aligned for GPSIMD)
M_PAD = 42
CROWS = 48        # const-pack row count (REPA/REPB lhsTs span 48 rows)

# column split: DVE takes [0:x], GPSIMD takes [x:F] of each wide stage
FS_P1 = 704       # pair / L1 stages
FS_L2 = 576       # L2 table-build stages
FS_PF = 576       # p01 / final reduction stages

_CACHE = {}


def _pm_mask(nvals, bits):
    v = np.arange(nvals, dtype=np.uint32)
    m = (v[:, None] >> np.arange(bits, dtype=np.uint32)[None, :]) & 1
    return (1.0 - 2.0 * m).astype(np.float32).T.copy()   # [bits, nvals]


def _parity(nvals, bits):
    v = np.arange(nvals, dtype=np.uint32)
    pc = np.zeros(nvals, dtype=np.uint32)
    for k in range(bits):
        pc += (v >> k) & 1
    return np.where(pc % 2 == 0, 1.0, -1.0).astype(np.float32)


class SideSpec:
    """H: f-side (width 1024, pair-table identity at z=3);
       G: p-side (width 128, identity at z=0)."""

    def __init__(self, name, width, mult_z):
        self.name = name
        self.w = width
        self.mult_z = list(mult_z)
        self.idz = ({0, 1, 2, 3} - set(mult_z)).pop()

    # pair-table entry -> master row (pair q, entry z); identity z -> ones row
    def tab_row(self, q, z):
        if z == self.idz:
            return M_ONE
        if self.name == "H":
            return {0: M_PP + q, 1: 2 * q + 1, 2: 2 * q}[z]
        return {1: 2 * q, 2: 2 * q + 1, 3: M_PP + q}[z]

    # group-2 scaled pair-table entry (pair 8) -> master row
    def tab2_row(self, z):
        if z == self.idz:
            return M_SGN
        if self.name == "H":
            return {0: M_PPS, 1: M_SO, 2: M_SE}[z]
        return {1: M_SE, 2: M_SO, 3: M_PPS}[z]


HSPEC = SideSpec("H", F, (0, 1, 2))
GSPEC = SideSpec("G", P, (1, 2, 3))


def _sel(rows, m_pad=None):
    """Selection matrix [M_PAD, len(rows)] with one 1 per used column."""
    M = len(rows) if m_pad is None else m_pad
    s = np.zeros((M_PAD, M), np.float32)
    for m, k in enumerate(rows):
        if k is not None:
            s[k, m] = 1.0
    return s


# const pack column layout: computed once at import
def _build_pack():
    cols = {}
    blocks = []
    off = 0

    def add(name, arr):
        nonlocal off
        a = np.zeros((CROWS, arr.shape[1]), np.float32)
        a[0:arr.shape[0], :] = arr
        cols[name] = (off, arr.shape[1])
        blocks.append(a)
        off += arr.shape[1]

    # MFX: [ones;pm(10) | (ones;pm)*sF] (row 0 = ones source, row 11 = sF source)
    mF = np.concatenate([np.ones((1, F), np.float32), _pm_mask(F, FBITS)], axis=0)
    sF = _parity(F, FBITS)
    add("MFX", np.concatenate([mF, mF * sF[None, :]], axis=0))           # [22, F]
    # MPX: [pm(7) | pm*sP | ones | sP]
    mP = _pm_mask(P, PBITS)
    sP = _parity(P, PBITS)
    add("MPX", np.concatenate(
        [mP, mP * sP[None, :], np.ones((1, P), np.float32), sP[None, :]], axis=0))
    # unit columns for the widened RS matmul lhsT (cols 20,21), per side
    uh = np.zeros((22, 2), np.float32)
    uh[0, 0] = 1.0      # -> MFX row 0 (ones)
    uh[11, 1] = 1.0     # -> MFX row 11 (sF)
    ug = np.zeros((16, 2), np.float32)
    ug[14, 0] = 1.0     # -> MPX row 14 (ones)
    ug[15, 1] = 1.0     # -> MPX row 15 (sP)
    cols["_U"] = (uh, ug)

    def digits(c):
        return c % 4, (c // 4) % 4, c // 16          # za, zb, zc

    for spec in (HSPEC, GSPEC):
        nm = spec.name
        # L1: 48 distinct products tmp48 indexed by (g, c2), c2 = za + 4zb:
        # in0/in1 packed [48 | pad | 48] in one mm
        in0 = []
        in1 = []
        for g in range(3):
            for c2 in range(16):
                in0.append(spec.tab_row(3 * g, c2 % 4))
                in1.append(spec.tab_row(3 * g + 1, c2 // 4))
        add(f"SELL1_{nm}", _sel(in0 + [None] * 16 + in1, 112))
        # REP: replicate tmp48 rows into L2 src layout (lhsT for rep matmuls)
        # repA: [64g + c] <- tmp48[16g + (c % 16)] for g=0,1 ; repB: g=2
        repA = np.zeros((48, 128), np.float32)
        for g in range(2):
            for c in range(64):
                repA[16 * g + (c % 16), 64 * g + c] = 1.0
        add(f"REPA_{nm}", repA)
        repB = np.zeros((48, 64), np.float32)
        for c in range(64):
            repB[32 + (c % 16), c] = 1.0
        add(f"REPB_{nm}", repB)
        # L2 c-packs: set1 rows [64g+c] = tab_c[zc]; set2 = scaled tab2'[zc]
        rows = []
        for g in range(2):
            for c in range(64):
                rows.append(spec.tab_row(3 * g + 2, digits(c)[2]))
        add(f"SELL2A_{nm}", _sel(rows))
        rows = [spec.tab2_row(digits(c)[2]) for c in range(64)]
        add(f"SELL2B_{nm}", _sel(rows))
    return np.concatenate(blocks, axis=1), cols


CPACK, CPACK_COLS = _build_pack()
U_HG = CPACK_COLS.pop("_U")
CW = CPACK.shape[1]

def _build_packr():
    parts = []
    offs = {}
    off = 0
    for nm in ("REPA_H", "REPB_H", "REPA_G", "REPB_G"):
        o, w = CPACK_COLS[nm]
        parts.append(CPACK[0:CROWS, o:o + w])
        offs[nm] = (off, w)
        off += w
    return np.ascontiguousarray(np.concatenate(parts, axis=1)), offs


CPACKR, CPACKR_COLS = _build_packr()
CRW = CPACKR.shape[1]


def build_lts(Ar, Ai):
    """Host-side lhsT pack: per (side, comp) the widened row-sum lhsT
    (32 cols) and the pair-pack lhsT (42 cols), one [22, 296] tensor."""
    lts = np.zeros((22, 296), np.float32)
    for si, (lo, kb, K) in enumerate(((0, 11, 22), (11, 7, 16))):
        for ci, A in ((0, Ar), (1, Ai)):
            base = 74 * (2 * si + ci)
            AT = np.ascontiguousarray(A.T, dtype=np.float32)
            lts[0:kb, base:base + 18] = AT[lo:lo + kb, 0:18]
            lts[kb:2 * kb, base + 18:base + 20] = AT[lo:lo + kb, 16:18]
            if ci == 0:
                lts[0:K, base + 20:base + 22] = U_HG[si]
            pb = base + 32
            lts[0:kb, pb:pb + 9] = AT[lo:lo + kb, 0:17:2]
            lts[kb:2 * kb, pb + 9] = AT[lo:lo + kb, 16]
            lts[0:kb, pb + 32:pb + 41] = AT[lo:lo + kb, 1:18:2]
            lts[0:kb, pb + 41] = AT[lo:lo + kb, 17]
    return lts


def make_in_map(Ar, Ai):
    return {"CPACK": CPACK, "LTS": build_lts(Ar, Ai)}


def host_consts():
    return {"CPACK": CPACK}


# ---------------------------------------------------------------- kernel body
def build_kernel(loop_iters=None):
    nc = bacc.Bacc("TRN2", target_bir_lowering=False, debug=False)

    tens = {}
    tens["LTS"] = nc.dram_tensor("LTS", [22, 296], FP32, kind="ExternalInput").ap()
    tens["CPACK"] = nc.dram_tensor("CPACK", [CROWS, CW], FP32,
                                   kind="ExternalInput").ap()
    tens["OUT"] = nc.dram_tensor("OUT", [128, 4], FP32, kind="ExternalOutput").ap()

    with tile.TileContext(nc) as tc:
        if loop_iters is None:
            _body(nc, tc, tens)
        else:
            with tc.For_i(0, loop_iters, 1):
                _body(nc, tc, tens)
    nc.compile()
    return nc


def _body(nc, tc, tens):
    from contextlib import ExitStack

    ctx = ExitStack()
    pers = ctx.enter_context(tc.tile_pool(name="pers", bufs=1))
    pk = ctx.enter_context(tc.tile_pool(name="pk", bufs=2))
    cm = ctx.enter_context(tc.tile_pool(name="cm", bufs=2))
    psum_pool = ctx.enter_context(tc.tile_pool(name="psum", bufs=4, space="PSUM"))
    dma = nc.sync.dma_start
    dma2 = nc.gpsimd.dma_start          # SWDGE queues for small input loads

    def mmr(out_ap, lhsT_ap, rhs_ap, start=True, stop=True):
        """fp32r matmul: full rate (1 cyc/row) on trn2 when free >= 256."""
        nc.tensor.matmul(out_ap, lhsT_ap.bitcast(FP32R), rhs_ap.bitcast(FP32R),
                         start=start, stop=stop)

    def cmul6(rows, w, i0, i1, outr, outi):
        """DVE complex multiply: (i0r,i0i)*(i1r,i1i) -> (outr,outi)."""
        e = nc.vector
        i0r, i0i = i0
        i1r, i1i = i1
        t1 = cm.tile([rows, w], FP32, tag="cm_t1")
        t2 = cm.tile([rows, w], FP32, tag="cm_t2")
        e.tensor_mul(t1[:], i0r, i1r)
        e.tensor_mul(t2[:], i0i, i1i)
        e.tensor_sub(outr, t1[:], t2[:])
        e.tensor_mul(t1[:], i0r, i1i)
        e.tensor_mul(t2[:], i0i, i1r)
        e.tensor_add(outi, t1[:], t2[:])

    def cmul6p(rows, w, i0, i1, outr, outi):
        """GPSIMD complex multiply (plain tensor-tensor ops)."""
        e = nc.gpsimd
        i0r, i0i = i0
        i1r, i1i = i1
        t1 = cm.tile([rows, w], FP32, tag="gp_t1")
        t2 = cm.tile([rows, w], FP32, tag="gp_t2")
        e.tensor_mul(t1[:], i0r, i1r)
        e.tensor_mul(t2[:], i0i, i1i)
        e.tensor_sub(outr, t1[:], t2[:])
        e.tensor_mul(t1[:], i0r, i1i)
        e.tensor_mul(t2[:], i0i, i1r)
        e.tensor_add(outi, t1[:], t2[:])

    def sel_mm(sel_sb, msrc, m, w):
        """Pack = SEL.T @ master -> PSUM [m, w]."""
        ps = psum_pool.tile([m, w], FP32, tag="ps")
        for c0 in range(0, w, 512):
            c1 = min(c0 + 512, w)
            nc.tensor.matmul(ps[:, c0:c1], sel_sb[:], msrc[:, c0:c1],
                             start=True, stop=True)
        return ps

    # ---- stage 0: A loads, widened row-sum matmuls -> master rows 0..21
    lts = pers.tile([22, 296], FP32, tag="lts")
    dma(lts[:], tens["LTS"][:, :])
    cpk = pers.tile([CROWS, CW], FP32, tag="cpack")
    dma(cpk[:, 0:512], tens["CPACK"][:, 0:512])
    nc.scalar.dma_start(cpk[:, 512:1024], tens["CPACK"][:, 512:1024])
    dma2(cpk[:, 1024:CW], tens["CPACK"][:, 1024:CW])

    warm = psum_pool.tile([128, 64], FP32, tag="ps")
    for _ in range(8):
        nc.tensor.matmul(warm[:], cpk[0:48, 0:128], cpk[0:48, 0:64],
                         start=True, stop=True)

    def cslice(name, nrows=None):
        off, width = CPACK_COLS[name]
        nr = M_PAD if nrows is None else nrows
        return cpk[0:nr, off:off + width]

    lhsT_rs = {}
    lhsT_pp = {}
    KRS = {"H": 22, "G": 16}
    for si, side in enumerate("HG"):
        for ci, nm in enumerate("ri"):
            base = 74 * (2 * si + ci)
            K_rs = KRS[side]
            lhsT_rs[(side, nm)] = lts[0:K_rs, base:base + 32]
            lhsT_pp[(side, nm)] = lts[0:K_rs, base + 32:base + 74]

    mask_sb = {"H": cslice("MFX", 22), "G": cslice("MPX", 16)}
    sel_sb = {}
    for spec in (HSPEC, GSPEC):
        for s in ("SELL1", "SELL2A", "SELL2B"):
            key = f"{s}_{spec.name}"
            sel_sb[key] = cslice(key)
        for s in ("REPA", "REPB"):
            key = f"{s}_{spec.name}"
            sel_sb[key] = cslice(key, 48)

    # H masters: one [M_PAD, F] tile per component; G master: [M_PAD, 2P]
    # with real in cols 0:P, imag in P:2P. All 32 rows get written (22 by
    # the RS copy, 10 by the pair stage) -- no memset needed.
    masterH = {}
    for nm in "ri":
        t = pers.tile([M_PAD, F], FP32, tag=f"mstH{nm}", name=f"mstH{nm}")
        masterH[nm] = t
    masterG = pers.tile([M_PAD, 2 * P], FP32, tag="mstG")

    # ---- stage 1: pair products -> master rows 32..41 (packs come
    # straight from the rearranged ltp lhsTs -- no master dependency)
    P1T = F - FS_P1
    psH1 = {}
    for nm in "ri":
        ps = sel_mm(lhsT_pp[("H", nm)], mask_sb["H"], 42, F)
        sb = pk.tile([10, F], FP32, tag=f"halfH{nm}")
        if nm == "r":
            nc.scalar.copy(sb[:], ps[32:42, :])
        else:
            nc.vector.tensor_copy(sb[:], ps[32:42, :])
        p0t = pk.tile([10, P1T], FP32, tag=f"p0tH{nm}")
        nc.scalar.copy(p0t[:], ps[0:10, FS_P1:F])
        psH1[nm] = (ps, sb, p0t)
    cmul6(10, FS_P1,
          (psH1["r"][0][0:10, 0:FS_P1], psH1["i"][0][0:10, 0:FS_P1]),
          (psH1["r"][1][:, 0:FS_P1], psH1["i"][1][:, 0:FS_P1]),
          masterH["r"][M_PP:M_PP + 10, 0:FS_P1],
          masterH["i"][M_PP:M_PP + 10, 0:FS_P1])
    cmul6p(10, P1T,
           (psH1["r"][2][:], psH1["i"][2][:]),
           (psH1["r"][1][:, FS_P1:F], psH1["i"][1][:, FS_P1:F]),
           masterH["r"][M_PP:M_PP + 10, FS_P1:F],
           masterH["i"][M_PP:M_PP + 10, FS_P1:F])

    psG1 = psum_pool.tile([42, 2 * P], FP32, tag="ps")
    nc.tensor.matmul(psG1[:, 0:P], lhsT_pp[("G", "r")], mask_sb["G"][:],
                     start=True, stop=True)
    nc.tensor.matmul(psG1[:, P:2 * P], lhsT_pp[("G", "i")], mask_sb["G"][:],
                     start=True, stop=True)
    sbG1a = pk.tile([10, 2 * P], FP32, tag="selpGa")
    nc.scalar.copy(sbG1a[:], psG1[0:10, :])
    sbG1b = pk.tile([10, 2 * P], FP32, tag="selpGb")
    nc.scalar.copy(sbG1b[:], psG1[32:42, :])
    cmul6p(10, P,
           (sbG1a[:, 0:P], sbG1a[:, P:2 * P]),
           (sbG1b[:, 0:P], sbG1b[:, P:2 * P]),
           masterG[M_PP:M_PP + 10, 0:P], masterG[M_PP:M_PP + 10, P:2 * P])

    # row-sum masters (needed from L1 onward; emitted after the pair
    # stage so its PSUM evacuations win the ACT queue early)
    for nm in "ri":
        lt = lhsT_rs[("H", nm)]
        ps = psum_pool.tile([32, F], FP32, tag="ps")
        for c0 in range(0, F, 512):
            c1 = min(c0 + 512, F)
            nc.tensor.matmul(ps[:, c0:c1], lt, mask_sb["H"][:, c0:c1],
                             start=True, stop=True)
        nc.scalar.copy(masterH[nm][0:32, :], ps[:])
    psG = psum_pool.tile([32, 2 * P], FP32, tag="ps")
    nc.tensor.matmul(psG[:, 0:P], lhsT_rs[("G", "r")], mask_sb["G"][:],
                     start=True, stop=True)
    nc.tensor.matmul(psG[:, P:2 * P], lhsT_rs[("G", "i")], mask_sb["G"][:],
                     start=True, stop=True)
    nc.scalar.copy(masterG[0:32, :], psG[:])


    # ---- stage 2 (L1): tmp48[16g + c2] = tab_a[za] * tab_b[zb]
    t48H = {}
    l1t = {}
    for nm in "ri":
        ps = sel_mm(sel_sb["SELL1_H"], masterH[nm][:], 112, F)
        sb1 = pk.tile([48, F], FP32, tag=f"l1hH{nm}")
        if nm == "r":
            nc.scalar.copy(sb1[:], ps[64:112, :])
        else:
            nc.vector.tensor_copy(sb1[:], ps[64:112, :])
        lt1 = pk.tile([48, P1T], FP32, tag=f"l1tH{nm}")
        nc.scalar.copy(lt1[:], ps[0:48, FS_P1:F])
        l1t[nm] = lt1
        t48 = pers.tile([48, F], FP32, tag=f"t48H{nm}", name=f"t48H{nm}")
        t48H[nm] = (ps, sb1, t48)
    cmul6(48, FS_P1,
          (t48H["r"][0][0:48, 0:FS_P1], t48H["i"][0][0:48, 0:FS_P1]),
          (t48H["r"][1][:, 0:FS_P1], t48H["i"][1][:, 0:FS_P1]),
          t48H["r"][2][:, 0:FS_P1], t48H["i"][2][:, 0:FS_P1])
    cmul6p(48, P1T,
           (l1t["r"][:], l1t["i"][:]),
           (t48H["r"][1][:, FS_P1:F], t48H["i"][1][:, FS_P1:F]),
           t48H["r"][2][:, FS_P1:F], t48H["i"][2][:, FS_P1:F])

    psL1G = sel_mm(sel_sb["SELL1_G"], masterG[:], 112, 2 * P)
    sbL1Ga = pk.tile([48, 2 * P], FP32, tag="l1Ga")
    nc.scalar.copy(sbL1Ga[:], psL1G[0:48, :])
    sbL1Gb = pk.tile([48, 2 * P], FP32, tag="l1Gb")
    nc.scalar.copy(sbL1Gb[:], psL1G[64:112, :])
    t48G = pers.tile([48, 2 * P], FP32, tag="t48G", name="t48G")
    cmul6p(48, P,
           (sbL1Ga[:, 0:P], sbL1Ga[:, P:2 * P]),
           (sbL1Gb[:, 0:P], sbL1Gb[:, P:2 * P]),
           t48G[:, 0:P], t48G[:, P:2 * P])

    # ---- stage 3 (L2): e_g = tmp * tab_c[zc], column-split DVE / GPSIMD
    # H g0+g1 fused as one [128, F] set -> eRH = [H0r; H1r], eIH = [H0i; H1i].
    # The T matmuls compensate with K=64 accumulating pairs.
    TL = F - FS_L2
    c01H = {}
    repH = {}
    reptH = {}
    for nm in "ri":
        c01 = sel_mm(sel_sb["SELL2A_H"], masterH[nm][:], 128, F)
        c01sb = pk.tile([128, F], FP32, tag=f"c01H{nm}")
        if nm == "r":
            nc.scalar.copy(c01sb[:], c01[:])
        else:
            nc.vector.tensor_copy(c01sb[:], c01[:])
        c01H[nm] = c01sb
        rep = sel_mm(sel_sb["REPA_H"], t48H[nm][2][:], 128, F)
        repH[nm] = rep
        rt = pk.tile([128, TL], FP32, tag=f"reptH{nm}")
        nc.scalar.copy(rt[:], repH[nm][:, FS_L2:F])
        reptH[nm] = rt
    eRH = pers.tile([128, F], FP32R, tag="eRH", name="eRH")
    eIH = pers.tile([128, F], FP32R, tag="eIH", name="eIH")
    cmul6(128, FS_L2,
          (repH["r"][0:128, 0:FS_L2], repH["i"][0:128, 0:FS_L2]),
          (c01H["r"][:, 0:FS_L2], c01H["i"][:, 0:FS_L2]),
          eRH[:, 0:FS_L2], eIH[:, 0:FS_L2])
    cmul6p(128, TL,
           (reptH["r"][:], reptH["i"][:]),
           (c01H["r"][:, FS_L2:F], c01H["i"][:, FS_L2:F]),
           eRH[:, FS_L2:F], eIH[:, FS_L2:F])

    cp2H = {}
    rbH = {}
    rbtH = {}
    for nm in "ri":
        c2 = sel_mm(sel_sb["SELL2B_H"], masterH[nm][:], 64, F)
        c2sb = pk.tile([64, F], FP32, tag=f"c2H{nm}")
        nc.scalar.copy(c2sb[:], c2[:])
        cp2H[nm] = c2sb
        rb = sel_mm(sel_sb["REPB_H"], t48H[nm][2][:], 64, F)
        rbH[nm] = rb
        rbt = pk.tile([64, TL], FP32, tag=f"rbtH{nm}")
        nc.scalar.copy(rbt[:], rb[:, FS_L2:F])
        rbtH[nm] = rbt
    eR2H = pers.tile([64, F], FP32R, tag="eR2H", name="eR2H")
    eI2H = pers.tile([64, F], FP32R, tag="eI2H", name="eI2H")
    cmul6(64, FS_L2,
          (rbH["r"][:, 0:FS_L2], rbH["i"][:, 0:FS_L2]),
          (cp2H["r"][:, 0:FS_L2], cp2H["i"][:, 0:FS_L2]),
          eR2H[:, 0:FS_L2], eI2H[:, 0:FS_L2])
    cmul6p(64, TL,
           (rbtH["r"][:], rbtH["i"][:]),
           (cp2H["r"][:, FS_L2:F], cp2H["i"][:, FS_L2:F]),
           eR2H[:, FS_L2:F], eI2H[:, FS_L2:F])

    # G side (GPSIMD, SBUF operands via single ACT evacuations)
    repG = sel_mm(sel_sb["REPA_G"], t48G[:], 128, 2 * P)
    repGsb = pk.tile([128, 2 * P], FP32, tag="repG")
    nc.scalar.copy(repGsb[:], repG[:])
    c01G = sel_mm(sel_sb["SELL2A_G"], masterG[:], 128, 2 * P)
    c01Gsb = pk.tile([128, 2 * P], FP32, tag="c01G")
    nc.scalar.copy(c01Gsb[:], c01G[:])
    eG01 = pers.tile([128, 2 * P], FP32R, tag="eG01", name="eG01")
    cmul6p(128, P,
           (repGsb[:, 0:P], repGsb[:, P:2 * P]),
           (c01Gsb[:, 0:P], c01Gsb[:, P:2 * P]),
           eG01[:, 0:P], eG01[:, P:2 * P])

    rbG = sel_mm(sel_sb["REPB_G"], t48G[:], 64, 2 * P)
    rbGsb = pk.tile([64, 2 * P], FP32, tag="rbG")
    nc.scalar.copy(rbGsb[:], rbG[:])
    c2G = sel_mm(sel_sb["SELL2B_G"], masterG[:], 64, 2 * P)
    c2Gsb = pk.tile([64, 2 * P], FP32, tag="c2G")
    nc.scalar.copy(c2Gsb[:], c2G[:])
    eG2 = pers.tile([64, 2 * P], FP32R, tag="eG2", name="eG2")
    cmul6p(64, P,
           (rbGsb[:, 0:P], rbGsb[:, P:2 * P]),
           (c2Gsb[:, 0:P], c2Gsb[:, P:2 * P]),
           eG2[:, 0:P], eG2[:, P:2 * P])

    # negated imag halves (lhsT for the real-part T matmuls)
    negG01 = pers.tile([128, P], FP32R, tag="negG01")
    nc.scalar.mul(negG01[:], eG01[:, P:2 * P], -1.0)
    negG2 = pers.tile([64, P], FP32R, tag="negG2")
    nc.scalar.mul(negG2[:], eG2[:, P:2 * P], -1.0)

    # ---- stage 4: T matmuls -- per (group, comp, chunk) a K=64 pair
    # accumulated in PSUM: Tr = Gr^T Hr + (-Gi)^T Hi ; Ti = Gr^T Hi + Gi^T Hr
    # Order: T0, T1 (combine inputs) first, then T2 (only needed by the
    # final reduction) so p01 overlaps the T2 matmuls.
    def t_mms(g):
        if g < 2:
            Gr = eG01[64 * g:64 * g + 64, 0:P]
            Gi = eG01[64 * g:64 * g + 64, P:2 * P]
            Gin = negG01[64 * g:64 * g + 64, :]
            Hr = eRH[64 * g:64 * g + 64, :]
            Hi = eIH[64 * g:64 * g + 64, :]
        else:
            Gr = eG2[:, 0:P]
            Gi = eG2[:, P:2 * P]
            Gin = negG2[:]
            Hr = eR2H[:]
            Hi = eI2H[:]
        tr = psum_pool.tile([P, F], FP32, tag="ps")
        ti = psum_pool.tile([P, F], FP32, tag="ps")
        for c0 in range(0, F, 512):
            c1 = c0 + 512
            mmr(tr[:, c0:c1], Gr, Hr[:, c0:c1], start=True, stop=False)
            mmr(tr[:, c0:c1], Gin, Hi[:, c0:c1], start=False, stop=True)
            mmr(ti[:, c0:c1], Gr, Hi[:, c0:c1], start=True, stop=False)
            mmr(ti[:, c0:c1], Gi, Hr[:, c0:c1], start=False, stop=True)
        return tr, ti

    t0r, t0i = t_mms(0)
    t1r_ps, t1i_ps = t_mms(1)
    t1r = pers.tile([P, F], FP32, tag="T1r")
    t1i = pers.tile([P, F], FP32, tag="T1i")
    for c0 in range(0, F, 512):
        c1 = c0 + 512
        nc.scalar.copy(t1r[:, c0:c1], t1r_ps[:, c0:c1])
        nc.vector.tensor_copy(t1i[:, c0:c1], t1i_ps[:, c0:c1])

    # p01 = T0*T1: col-split DVE (T0 from PSUM) / GPSIMD (T0 tail via ACT)
    TP = F - FS_PF
    p01r = pers.tile([P, F], FP32, tag="p01r")
    p01i = pers.tile([P, F], FP32, tag="p01i")
    t0tr = pers.tile([P, TP], FP32, tag="t0tr")
    t0ti = pers.tile([P, TP], FP32, tag="t0ti")
    nc.scalar.copy(t0tr[:], t0r[:, FS_PF:F])
    nc.scalar.copy(t0ti[:], t0i[:, FS_PF:F])
    cmul6(P, FS_PF,
          (t0r[:, 0:FS_PF], t0i[:, 0:FS_PF]),
          (t1r[:, 0:FS_PF], t1i[:, 0:FS_PF]),
          p01r[:, 0:FS_PF], p01i[:, 0:FS_PF])
    cmul6p(P, TP,
           (t0tr[:], t0ti[:]),
           (t1r[:, FS_PF:F], t1i[:, FS_PF:F]),
           p01r[:, FS_PF:F], p01i[:, FS_PF:F])

    t2r, t2i = t_mms(2)

    # ---- final reduction: acc[p, k] = sum_f p01 * T2 products (DVE,
    # full width, T2 straight from PSUM).
    # Host combines: perm_r = c0 - c1, perm_i = c2 + c3
    scr2 = pers.tile([P, F], FP32, tag="ttr_scr")
    accD = pers.tile([P, 4], FP32, tag="accD")
    pairs = [(p01r, t2r), (p01i, t2i), (p01r, t2i), (p01i, t2r)]
    for k, (a, b) in enumerate(pairs):
        nc.vector.scalar_tensor_tensor(
            out=scr2[:], in0=b[:], scalar=1.0, in1=a[:],
            op0=OP.mult, op1=OP.mult, accum_out=accD[:, k:k + 1])

    dma(tens["OUT"][:, 0:4], accD[:])

    ctx.close()


# ---------------------------------------------------------------- entry point
def kernel(A_real: np.ndarray, A_imag: np.ndarray) -> np.ndarray:
    B = A_real.shape[0]
    assert B == 8 and A_real.shape == (B, N, N)
    if "nc" not in _CACHE:
        _CACHE["nc"] = build_kernel()
    nc = _CACHE["nc"]
    in_maps = [make_in_map(A_real[b], A_imag[b]) for b in range(B)]
    res = run_bass_kernel_spmd(nc, in_maps, list(range(B)))
    out = np.empty(B, dtype=np.float32)
    for b in range(B):
        acc = res.results[b]["OUT"].reshape(128, 4).astype(np.float64)
        s = acc.sum(axis=0)
        pr = s[0] - s[1]
        pi = s[2] + s[3]
        pa2 = np.float32(pr) ** 2 + np.float32(pi) ** 2
        out[b] = np.float32(EMU * SCALE2 * pa2 + DARK)
    return out


if __name__ == "__main__":
    A_real = np.load("/tmp/A_real.npy")
    A_imag = np.load("/tmp/A_imag.npy")
    print(kernel(A_real, A_imag))


# revision 34
# speedup vs baseline: 2.6694x; 1.0085x over previous
"""Trainium2 Bass kernel: boson-sampler probabilities via Glynn's permanent formula.

Math (per 18x18 complex matrix A):
  perm(A) = 2^(1-n) * sum_{d in {+-1}^n, d_0=+1} (prod_k d_k) * prod_i (sum_j d_j A[i,j])
The 2^17 sign vectors form a [128 x 1024] grid (7 "p" bits drive columns 11..17,
10 "f" bits drive columns 1..10; column 0 fixed +1). Row-sums factor as
rs_i = RP_i(p) + RF_i(f); rows are grouped [6,6,6] and each group's product
expands as T_g[p,f] = sum_{c<64} G_g[c,p] * H_g[c,f] -- fp32r matmuls on the
tensor engine (full rate at free>=256). The 64-row G/H tables (all sub-products
of 6 rows) are built hierarchically (pairs -> quads -> tables): packed operand
sets are assembled from SBUF "master" row tiles by 0/1 selection matmuls, and
each level is a set of elementwise complex-multiply ops column-split across
the DVE and GPSIMD engines. Glynn parity signs are folded into group 2's
tables via sign-scaled mask constants. G-side (p-axis) work packs real|imag
side by side in one [32, 256] master so every G matmul runs at free=256.
The final sum(T0*T1*T2) reduces via 8 scalar_tensor_tensor accumulations
(col-split DVE/Pool); |perm|^2, the (underflowed-to-zero) classical term and
the dark-count offset are applied on the host. One NeuronCore per batch
element.
"""

import sys

sys.path.insert(0, "/opt/trn_rl_repo")

import numpy as np

import concourse.bacc as bacc
import concourse.bass as bass
import concourse.tile as tile
from concourse import mybir
from concourse.bass_utils import run_bass_kernel_spmd

FP32 = mybir.dt.float32
FP32R = mybir.dt.float32r
OP = mybir.AluOpType

N = 18
PBITS, FBITS = 7, 10
P, F = 1 << PBITS, 1 << FBITS          # 128, 1024
EMU = 0.85 * (1 - 0.02) * (1 - 0.02) * (1 - 0.01)
DARK = 1e-6 * N
SCALE2 = float(2.0 ** (2 * (1 - N)))

# master row map (same for both sides; imag comp has zeros at ONES/SIGN)
M_RF = 0          # rows 0..17: row-sums RF_i / RP_i
M_SE = 18         # sign-scaled even row of pair 8 (RFe' / RPe')
M_SO = 19         # sign-scaled odd row (RFo' / RPo')
M_ONE = 20        # ones (real) / zeros (imag)
M_SGN = 21        # sign row sF / sP (real) / zeros (imag)
M_PP = 32         # rows 32..41: pair products PP_q (row 41 = PP' scaled)
M_PPS = 41        # row 41: PP' = sign-scaled pair-8 product (32-aligned for GPSIMD)
M_PAD = 42
CROWS = 48        # const-pack row count (REPA/REPB lhsTs span 48 rows)

# column split: DVE takes [0:x], GPSIMD takes [x:F] of each wide stage
FS_P1 = 704       # pair / L1 stages
FS_L2 = 576       # L2 table-build stages
FS_PF = 576       # p01 / final reduction stages

_CACHE = {}


def _pm_mask(nvals, bits):
    v = np.arange(nvals, dtype=np.uint32)
    m = (v[:, None] >> np.arange(bits, dtype=np.uint32)[None, :]) & 1
    return (1.0 - 2.0 * m).astype(np.float32).T.copy()   # [bits, nvals]


def _parity(nvals, bits):
    v = np.arange(nvals, dtype=np.uint32)
    pc = np.zeros(nvals, dtype=np.uint32)
    for k in range(bits):
        pc += (v >> k) & 1
    return np.where(pc % 2 == 0, 1.0, -1.0).astype(np.float32)


class SideSpec:
    """H: f-side (width 1024, pair-table identity at z=3);
       G: p-side (width 128, identity at z=0)."""

    def __init__(self, name, width, mult_z):
        self.name = name
        self.w = width
        self.mult_z = list(mult_z)
        self.idz = ({0, 1, 2, 3} - set(mult_z)).pop()

    # pair-table entry -> master row (pair q, entry z); identity z -> ones row
    def tab_row(self, q, z):
        if z == self.idz:
            return M_ONE
        if self.name == "H":
            return {0: M_PP + q, 1: 2 * q + 1, 2: 2 * q}[z]
        return {1: 2 * q, 2: 2 * q + 1, 3: M_PP + q}[z]

    # group-2 scaled pair-table entry (pair 8) -> master row
    def tab2_row(self, z):
        if z == self.idz:
            return M_SGN
        if self.name == "H":
            return {0: M_PPS, 1: M_SO, 2: M_SE}[z]
        return {1: M_SE, 2: M_SO, 3: M_PPS}[z]


HSPEC = SideSpec("H", F, (0, 1, 2))
GSPEC = SideSpec("G", P, (1, 2, 3))


def _sel(rows, m_pad=None):
    """Selection matrix [M_PAD, len(rows)] with one 1 per used column."""
    M = len(rows) if m_pad is None else m_pad
    s = np.zeros((M_PAD, M), np.float32)
    for m, k in enumerate(rows):
        if k is not None:
            s[k, m] = 1.0
    return s


# const pack column layout: computed once at import
def _build_pack():
    cols = {}
    blocks = []
    off = 0

    def add(name, arr):
        nonlocal off
        a = np.zeros((CROWS, arr.shape[1]), np.float32)
        a[0:arr.shape[0], :] = arr
        cols[name] = (off, arr.shape[1])
        blocks.append(a)
        off += arr.shape[1]

    # MFX: [ones;pm(10) | (ones;pm)*sF] (row 0 = ones source, row 11 = sF source)
    mF = np.concatenate([np.ones((1, F), np.float32), _pm_mask(F, FBITS)], axis=0)
    sF = _parity(F, FBITS)
    add("MFX", np.concatenate([mF, mF * sF[None, :]], axis=0))           # [22, F]
    # MPX: [pm(7) | pm*sP | ones | sP]
    mP = _pm_mask(P, PBITS)
    sP = _parity(P, PBITS)
    add("MPX", np.concatenate(
        [mP, mP * sP[None, :], np.ones((1, P), np.float32), sP[None, :]], axis=0))
    # unit columns for the widened RS matmul lhsT (cols 20,21), per side
    uh = np.zeros((22, 2), np.float32)
    uh[0, 0] = 1.0      # -> MFX row 0 (ones)
    uh[11, 1] = 1.0     # -> MFX row 11 (sF)
    ug = np.zeros((16, 2), np.float32)
    ug[14, 0] = 1.0     # -> MPX row 14 (ones)
    ug[15, 1] = 1.0     # -> MPX row 15 (sP)
    cols["_U"] = (uh, ug)

    def digits(c):
        return c % 4, (c // 4) % 4, c // 16          # za, zb, zc

    for spec in (HSPEC, GSPEC):
        nm = spec.name
        # L1: 48 distinct products tmp48 indexed by (g, c2), c2 = za + 4zb:
        # in0/in1 packed [48 | pad | 48] in one mm
        in0 = []
        in1 = []
        for g in range(3):
            for c2 in range(16):
                in0.append(spec.tab_row(3 * g, c2 % 4))
                in1.append(spec.tab_row(3 * g + 1, c2 // 4))
        add(f"SELL1_{nm}", _sel(in0 + [None] * 16 + in1, 112))
        # REP: replicate tmp48 rows into L2 src layout (lhsT for rep matmuls)
        # repA: [64g + c] <- tmp48[16g + (c % 16)] for g=0,1 ; repB: g=2
        repA = np.zeros((48, 128), np.float32)
        for g in range(2):
            for c in range(64):
                repA[16 * g + (c % 16), 64 * g + c] = 1.0
        add(f"REPA_{nm}", repA)
        repB = np.zeros((48, 64), np.float32)
        for c in range(64):
            repB[32 + (c % 16), c] = 1.0
        add(f"REPB_{nm}", repB)
        # L2 c-packs: set1 rows [64g+c] = tab_c[zc]; set2 = scaled tab2'[zc]
        rows = []
        for g in range(2):
            for c in range(64):
                rows.append(spec.tab_row(3 * g + 2, digits(c)[2]))
        add(f"SELL2A_{nm}", _sel(rows))
        rows = [spec.tab2_row(digits(c)[2]) for c in range(64)]
        add(f"SELL2B_{nm}", _sel(rows))
    return np.concatenate(blocks, axis=1), cols


CPACK, CPACK_COLS = _build_pack()
U_HG = CPACK_COLS.pop("_U")
CW = CPACK.shape[1]

def _build_packr():
    parts = []
    offs = {}
    off = 0
    for nm in ("REPA_H", "REPB_H", "REPA_G", "REPB_G"):
        o, w = CPACK_COLS[nm]
        parts.append(CPACK[0:CROWS, o:o + w])
        offs[nm] = (off, w)
        off += w
    return np.ascontiguousarray(np.concatenate(parts, axis=1)), offs


CPACKR, CPACKR_COLS = _build_packr()
CRW = CPACKR.shape[1]


def build_lts(Ar, Ai):
    """Host-side lhsT pack: per (side, comp) the widened row-sum lhsT
    (32 cols) and the pair-pack lhsT (42 cols), one [22, 296] tensor."""
    lts = np.zeros((22, 296), np.float32)
    for si, (lo, kb, K) in enumerate(((0, 11, 22), (11, 7, 16))):
        for ci, A in ((0, Ar), (1, Ai)):
            base = 74 * (2 * si + ci)
            AT = np.ascontiguousarray(A.T, dtype=np.float32)
            lts[0:kb, base:base + 18] = AT[lo:lo + kb, 0:18]
            lts[kb:2 * kb, base + 18:base + 20] = AT[lo:lo + kb, 16:18]
            if ci == 0:
                lts[0:K, base + 20:base + 22] = U_HG[si]
            pb = base + 32
            lts[0:kb, pb:pb + 9] = AT[lo:lo + kb, 0:17:2]
            lts[kb:2 * kb, pb + 9] = AT[lo:lo + kb, 16]
            lts[0:kb, pb + 32:pb + 41] = AT[lo:lo + kb, 1:18:2]
            lts[0:kb, pb + 41] = AT[lo:lo + kb, 17]
    return lts


def make_in_map(Ar, Ai):
    return {"CPACK": CPACK, "LTS": build_lts(Ar, Ai)}


def host_consts():
    return {"CPACK": CPACK}


# ---------------------------------------------------------------- kernel body
def build_kernel(loop_iters=None):
    nc = bacc.Bacc("TRN2", target_bir_lowering=False, debug=False)

    tens = {}
    tens["LTS"] = nc.dram_tensor("LTS", [22, 296], FP32, kind="ExternalInput").ap()
    tens["CPACK"] = nc.dram_tensor("CPACK", [CROWS, CW], FP32,
                                   kind="ExternalInput").ap()
    tens["OUT"] = nc.dram_tensor("OUT", [128, 4], FP32, kind="ExternalOutput").ap()

    with tile.TileContext(nc) as tc:
        if loop_iters is None:
            _body(nc, tc, tens)
        else:
            with tc.For_i(0, loop_iters, 1):
                _body(nc, tc, tens)
    nc.compile()
    return nc


def _body(nc, tc, tens):
    from contextlib import ExitStack

    ctx = ExitStack()
    pers = ctx.enter_context(tc.tile_pool(name="pers", bufs=1))
    pk = ctx.enter_context(tc.tile_pool(name="pk", bufs=2))
    cm = ctx.enter_context(tc.tile_pool(name="cm", bufs=2))
    psum_pool = ctx.enter_context(tc.tile_pool(name="psum", bufs=4, space="PSUM"))
    dma = nc.sync.dma_start
    dma2 = nc.gpsimd.dma_start          # SWDGE queues for small input loads

    def mmr(out_ap, lhsT_ap, rhs_ap, start=True, stop=True):
        """fp32r matmul: full rate (1 cyc/row) on trn2 when free >= 256."""
        nc.tensor.matmul(out_ap, lhsT_ap.bitcast(FP32R), rhs_ap.bitcast(FP32R),
                         start=start, stop=stop)

    def cmul6(rows, w, i0, i1, outr, outi):
        """DVE complex multiply: (i0r,i0i)*(i1r,i1i) -> (outr,outi)."""
        e = nc.vector
        i0r, i0i = i0
        i1r, i1i = i1
        t1 = cm.tile([rows, w], FP32, tag="cm_t1")
        t2 = cm.tile([rows, w], FP32, tag="cm_t2")
        e.tensor_mul(t1[:], i0r, i1r)
        e.tensor_mul(t2[:], i0i, i1i)
        e.tensor_sub(outr, t1[:], t2[:])
        e.tensor_mul(t1[:], i0r, i1i)
        e.tensor_mul(t2[:], i0i, i1r)
        e.tensor_add(outi, t1[:], t2[:])

    def cmul6p(rows, w, i0, i1, outr, outi):
        """GPSIMD complex multiply (plain tensor-tensor ops)."""
        e = nc.gpsimd
        i0r, i0i = i0
        i1r, i1i = i1
        t1 = cm.tile([rows, w], FP32, tag="gp_t1")
        t2 = cm.tile([rows, w], FP32, tag="gp_t2")
        e.tensor_mul(t1[:], i0r, i1r)
        e.tensor_mul(t2[:], i0i, i1i)
        e.tensor_sub(outr, t1[:], t2[:])
        e.tensor_mul(t1[:], i0r, i1i)
        e.tensor_mul(t2[:], i0i, i1r)
        e.tensor_add(outi, t1[:], t2[:])

    def sel_mm(sel_sb, msrc, m, w):
        """Pack = SEL.T @ master -> PSUM [m, w]."""
        ps = psum_pool.tile([m, w], FP32, tag="ps")
        for c0 in range(0, w, 512):
            c1 = min(c0 + 512, w)
            nc.tensor.matmul(ps[:, c0:c1], sel_sb[:], msrc[:, c0:c1],
                             start=True, stop=True)
        return ps

    # ---- stage 0: A loads, widened row-sum matmuls -> master rows 0..21
    lts = pers.tile([22, 296], FP32, tag="lts")
    dma(lts[:], tens["LTS"][:, :])
    cpk = pers.tile([CROWS, CW], FP32, tag="cpack")
    dma(cpk[:, 0:512], tens["CPACK"][:, 0:512])
    nc.scalar.dma_start(cpk[:, 512:1024], tens["CPACK"][:, 512:1024])
    dma2(cpk[:, 1024:CW], tens["CPACK"][:, 1024:CW])

    warm = psum_pool.tile([128, 64], FP32, tag="ps")
    for _ in range(8):
        nc.tensor.matmul(warm[:], cpk[0:48, 0:128], cpk[0:48, 0:64],
                         start=True, stop=True)

    def cslice(name, nrows=None):
        off, width = CPACK_COLS[name]
        nr = M_PAD if nrows is None else nrows
        return cpk[0:nr, off:off + width]

    lhsT_rs = {}
    lhsT_pp = {}
    KRS = {"H": 22, "G": 16}
    for si, side in enumerate("HG"):
        for ci, nm in enumerate("ri"):
            base = 74 * (2 * si + ci)
            K_rs = KRS[side]
            lhsT_rs[(side, nm)] = lts[0:K_rs, base:base + 32]
            lhsT_pp[(side, nm)] = lts[0:K_rs, base + 32:base + 74]

    mask_sb = {"H": cslice("MFX", 22), "G": cslice("MPX", 16)}
    sel_sb = {}
    for spec in (HSPEC, GSPEC):
        for s in ("SELL1", "SELL2A", "SELL2B"):
            key = f"{s}_{spec.name}"
            sel_sb[key] = cslice(key)
        for s in ("REPA", "REPB"):
            key = f"{s}_{spec.name}"
            sel_sb[key] = cslice(key, 48)

    # H masters: one [M_PAD, F] tile per component; G master: [M_PAD, 2P]
    # with real in cols 0:P, imag in P:2P. All 32 rows get written (22 by
    # the RS copy, 10 by the pair stage) -- no memset needed.
    masterH = {}
    for nm in "ri":
        t = pers.tile([M_PAD, F], FP32, tag=f"mstH{nm}", name=f"mstH{nm}")
        masterH[nm] = t
    masterG = pers.tile([M_PAD, 2 * P], FP32, tag="mstG")

    # ---- stage 1: pair products -> master rows 32..41 (packs come
    # straight from the rearranged ltp lhsTs -- no master dependency)
    P1T = F - FS_P1
    psH1 = {}
    for nm in "ri":
        ps = sel_mm(lhsT_pp[("H", nm)], mask_sb["H"], 42, F)
        sb = pk.tile([10, F], FP32, tag=f"halfH{nm}")
        if nm == "r":
            nc.scalar.copy(sb[:], ps[32:42, :])
        else:
            nc.vector.tensor_copy(sb[:], ps[32:42, :])
        p0t = pk.tile([10, P1T], FP32, tag=f"p0tH{nm}")
        nc.scalar.copy(p0t[:], ps[0:10, FS_P1:F])
        psH1[nm] = (ps, sb, p0t)
    cmul6(10, FS_P1,
          (psH1["r"][0][0:10, 0:FS_P1], psH1["i"][0][0:10, 0:FS_P1]),
          (psH1["r"][1][:, 0:FS_P1], psH1["i"][1][:, 0:FS_P1]),
          masterH["r"][M_PP:M_PP + 10, 0:FS_P1],
          masterH["i"][M_PP:M_PP + 10, 0:FS_P1])
    cmul6p(10, P1T,
           (psH1["r"][2][:], psH1["i"][2][:]),
           (psH1["r"][1][:, FS_P1:F], psH1["i"][1][:, FS_P1:F]),
           masterH["r"][M_PP:M_PP + 10, FS_P1:F],
           masterH["i"][M_PP:M_PP + 10, FS_P1:F])

    psG1 = psum_pool.tile([42, 2 * P], FP32, tag="ps")
    nc.tensor.matmul(psG1[:, 0:P], lhsT_pp[("G", "r")], mask_sb["G"][:],
                     start=True, stop=True)
    nc.tensor.matmul(psG1[:, P:2 * P], lhsT_pp[("G", "i")], mask_sb["G"][:],
                     start=True, stop=True)
    sbG1a = pk.tile([10, 2 * P], FP32, tag="selpGa")
    nc.scalar.copy(sbG1a[:], psG1[0:10, :])
    sbG1b = pk.tile([10, 2 * P], FP32, tag="selpGb")
    nc.scalar.copy(sbG1b[:], psG1[32:42, :])
    cmul6p(10, P,
           (sbG1a[:, 0:P], sbG1a[:, P:2 * P]),
           (sbG1b[:, 0:P], sbG1b[:, P:2 * P]),
           masterG[M_PP:M_PP + 10, 0:P], masterG[M_PP:M_PP + 10, P:2 * P])

    # row-sum masters (needed from L1 onward; emitted after the pair
    # stage so its PSUM evacuations win the ACT queue early)
    for nm in "ri":
        lt = lhsT_rs[("H", nm)]
        ps = psum_pool.tile([32, F], FP32, tag="ps")
        for c0 in range(0, F, 512):
            c1 = min(c0 + 512, F)
            nc.tensor.matmul(ps[:, c0:c1], lt, mask_sb["H"][:, c0:c1],
                             start=True, stop=True)
        nc.scalar.copy(masterH[nm][0:32, :], ps[:])
    psG = psum_pool.tile([32, 2 * P], FP32, tag="ps")
    nc.tensor.matmul(psG[:, 0:P], lhsT_rs[("G", "r")], mask_sb["G"][:],
                     start=True, stop=True)
    nc.tensor.matmul(psG[:, P:2 * P], lhsT_rs[("G", "i")], mask_sb["G"][:],
                     start=True, stop=True)
    nc.scalar.copy(masterG[0:32, :], psG[:])


    # ---- stage 2 (L1): tmp48[16g + c2] = tab_a[za] * tab_b[zb]
    t48H = {}
    l1t = {}
    for nm in "ri":
        ps = sel_mm(sel_sb["SELL1_H"], masterH[nm][:], 112, F)
        sb1 = pk.tile([48, F], FP32, tag=f"l1hH{nm}")
        if nm == "r":
            nc.scalar.copy(sb1[:], ps[64:112, :])
        else:
            nc.vector.tensor_copy(sb1[:], ps[64:112, :])
        lt1 = pk.tile([48, P1T], FP32, tag=f"l1tH{nm}")
        nc.scalar.copy(lt1[:], ps[0:48, FS_P1:F])
        l1t[nm] = lt1
        t48 = pers.tile([48, F], FP32, tag=f"t48H{nm}", name=f"t48H{nm}")
        t48H[nm] = (ps, sb1, t48)
    cmul6(48, FS_P1,
          (t48H["r"][0][0:48, 0:FS_P1], t48H["i"][0][0:48, 0:FS_P1]),
          (t48H["r"][1][:, 0:FS_P1], t48H["i"][1][:, 0:FS_P1]),
          t48H["r"][2][:, 0:FS_P1], t48H["i"][2][:, 0:FS_P1])
    cmul6p(48, P1T,
           (l1t["r"][:], l1t["i"][:]),
           (t48H["r"][1][:, FS_P1:F], t48H["i"][1][:, FS_P1:F]),
           t48H["r"][2][:, FS_P1:F], t48H["i"][2][:, FS_P1:F])

    psL1G = sel_mm(sel_sb["SELL1_G"], masterG[:], 112, 2 * P)
    sbL1Ga = pk.tile([48, 2 * P], FP32, tag="l1Ga")
    nc.scalar.copy(sbL1Ga[:], psL1G[0:48, :])
    sbL1Gb = pk.tile([48, 2 * P], FP32, tag="l1Gb")
    nc.scalar.copy(sbL1Gb[:], psL1G[64:112, :])
    t48G = pers.tile([48, 2 * P], FP32, tag="t48G", name="t48G")
    cmul6p(48, P,
           (sbL1Ga[:, 0:P], sbL1Ga[:, P:2 * P]),
           (sbL1Gb[:, 0:P], sbL1Gb[:, P:2 * P]),
           t48G[:, 0:P], t48G[:, P:2 * P])

    # ---- stage 3 (L2): e_g = tmp * tab_c[zc], column-split DVE / GPSIMD
    # H g0+g1 fused as one [128, F] set -> eRH = [H0r; H1r], eIH = [H0i; H1i].
    # The T matmuls compensate with K=64 accumulating pairs.
    TL = F - FS_L2
    c01H = {}
    repH = {}
    reptH = {}
    for nm in "ri":
        c01 = sel_mm(sel_sb["SELL2A_H"], masterH[nm][:], 128, F)
        c01sb = pk.tile([128, F], FP32, tag=f"c01H{nm}")
        if nm == "r":
            nc.scalar.copy(c01sb[:], c01[:])
        else:
            nc.vector.tensor_copy(c01sb[:], c01[:])
        c01H[nm] = c01sb
        rep = sel_mm(sel_sb["REPA_H"], t48H[nm][2][:], 128, F)
        repH[nm] = rep
        rt = pk.tile([128, TL], FP32, tag=f"reptH{nm}")
        nc.scalar.copy(rt[:], repH[nm][:, FS_L2:F])
        reptH[nm] = rt
    eRH = pers.tile([128, F], FP32R, tag="eRH", name="eRH")
    eIH = pers.tile([128, F], FP32R, tag="eIH", name="eIH")
    cmul6(128, FS_L2,
          (repH["r"][0:128, 0:FS_L2], repH["i"][0:128, 0:FS_L2]),
          (c01H["r"][:, 0:FS_L2], c01H["i"][:, 0:FS_L2]),
          eRH[:, 0:FS_L2], eIH[:, 0:FS_L2])
    cmul6p(128, TL,
           (reptH["r"][:], reptH["i"][:]),
           (c01H["r"][:, FS_L2:F], c01H["i"][:, FS_L2:F]),
           eRH[:, FS_L2:F], eIH[:, FS_L2:F])

    cp2H = {}
    rbH = {}
    rbtH = {}
    for nm in "ri":
        c2 = sel_mm(sel_sb["SELL2B_H"], masterH[nm][:], 64, F)
        c2sb = pk.tile([64, F], FP32, tag=f"c2H{nm}")
        nc.scalar.copy(c2sb[:], c2[:])
        cp2H[nm] = c2sb
        rb = sel_mm(sel_sb["REPB_H"], t48H[nm][2][:], 64, F)
        rbH[nm] = rb
        rbt = pk.tile([64, TL], FP32, tag=f"rbtH{nm}")
        nc.scalar.copy(rbt[:], rb[:, FS_L2:F])
        rbtH[nm] = rbt
    eR2H = pers.tile([64, F], FP32R, tag="eR2H", name="eR2H")
    eI2H = pers.tile([64, F], FP32R, tag="eI2H", name="eI2H")
    cmul6(64, FS_L2,
          (rbH["r"][:, 0:FS_L2], rbH["i"][:, 0:FS_L2]),
          (cp2H["r"][:, 0:FS_L2], cp2H["i"][:, 0:FS_L2]),
          eR2H[:, 0:FS_L2], eI2H[:, 0:FS_L2])
    cmul6p(64, TL,
           (rbtH["r"][:], rbtH["i"][:]),
           (cp2H["r"][:, FS_L2:F], cp2H["i"][:, FS_L2:F]),
           eR2H[:, FS_L2:F], eI2H[:, FS_L2:F])

    # G side (GPSIMD, SBUF operands via single ACT evacuations)
    repG = sel_mm(sel_sb["REPA_G"], t48G[:], 128, 2 * P)
    repGsb = pk.tile([128, 2 * P], FP32, tag="repG")
    nc.scalar.copy(repGsb[:], repG[:])
    c01G = sel_mm(sel_sb["SELL2A_G"], masterG[:], 128, 2 * P)
    c01Gsb = pk.tile([128, 2 * P], FP32, tag="c01G")
    nc.scalar.copy(c01Gsb[:], c01G[:])
    eG01 = pers.tile([128, 2 * P], FP32R, tag="eG01", name="eG01")
    cmul6p(128, P,
           (repGsb[:, 0:P], repGsb[:, P:2 * P]),
           (c01Gsb[:, 0:P], c01Gsb[:, P:2 * P]),
           eG01[:, 0:P], eG01[:, P:2 * P])

    rbG = sel_mm(sel_sb["REPB_G"], t48G[:], 64, 2 * P)
    rbGsb = pk.tile([64, 2 * P], FP32, tag="rbG")
    nc.scalar.copy(rbGsb[:], rbG[:])
    c2G = sel_mm(sel_sb["SELL2B_G"], masterG[:], 64, 2 * P)
    c2Gsb = pk.tile([64, 2 * P], FP32, tag="c2G")
    nc.scalar.copy(c2Gsb[:], c2G[:])
    eG2 = pers.tile([64, 2 * P], FP32R, tag="eG2", name="eG2")
    cmul6p(64, P,
           (rbGsb[:, 0:P], rbGsb[:, P:2 * P]),
           (c2Gsb[:, 0:P], c2Gsb[:, P:2 * P]),
           eG2[:, 0:P], eG2[:, P:2 * P])

    # negated imag halves (lhsT for the real-part T matmuls)
    negG01 = pers.tile([128, P], FP32R, tag="negG01")
    nc.scalar.mul(negG01[:], eG01[:, P:2 * P], -1.0)
    negG2 = pers.tile([64, P], FP32R, tag="negG2")
    nc.scalar.mul(negG2[:], eG2[:, P:2 * P], -1.0)

    # ---- stage 4: T matmuls -- per (group, comp, chunk) a K=64 pair
    # accumulated in PSUM: Tr = Gr^T Hr + (-Gi)^T Hi ; Ti = Gr^T Hi + Gi^T Hr
    # Order: T0, T1 (combine inputs) first, then T2 (only needed by the
    # final reduction) so p01 overlaps the T2 matmuls.
    def t_mms(g):
        if g < 2:
            Gr = eG01[64 * g:64 * g + 64, 0:P]
            Gi = eG01[64 * g:64 * g + 64, P:2 * P]
            Gin = negG01[64 * g:64 * g + 64, :]
            Hr = eRH[64 * g:64 * g + 64, :]
            Hi = eIH[64 * g:64 * g + 64, :]
        else:
            Gr = eG2[:, 0:P]
            Gi = eG2[:, P:2 * P]
            Gin = negG2[:]
            Hr = eR2H[:]
            Hi = eI2H[:]
        tr = psum_pool.tile([P, F], FP32, tag="ps")
        ti = psum_pool.tile([P, F], FP32, tag="ps")
        for c0 in range(0, F, 512):
            c1 = c0 + 512
            mmr(tr[:, c0:c1], Gr, Hr[:, c0:c1], start=True, stop=False)
            mmr(tr[:, c0:c1], Gin, Hi[:, c0:c1], start=False, stop=True)
            mmr(ti[:, c0:c1], Gr, Hi[:, c0:c1], start=True, stop=False)
            mmr(ti[:, c0:c1], Gi, Hr[:, c0:c1], start=False, stop=True)
        return tr, ti

    t0r, t0i = t_mms(0)
    t1r_ps, t1i_ps = t_mms(1)
    t1r = pers.tile([P, F], FP32, tag="T1r")
    t1i = pers.tile([P, F], FP32, tag="T1i")
    for c0 in range(0, F, 512):
        c1 = c0 + 512
        nc.scalar.copy(t1r[:, c0:c1], t1r_ps[:, c0:c1])
        nc.vector.tensor_copy(t1i[:, c0:c1], t1i_ps[:, c0:c1])

    # p01 = T0*T1: col-split DVE (T0 from PSUM) / GPSIMD (T0 tail via ACT)
    TP = F - FS_PF
    p01r = pers.tile([P, F], FP32, tag="p01r")
    p01i = pers.tile([P, F], FP32, tag="p01i")
    t0tr = pers.tile([P, TP], FP32, tag="t0tr")
    t0ti = pers.tile([P, TP], FP32, tag="t0ti")
    nc.scalar.copy(t0tr[:], t0r[:, FS_PF:F])
    nc.scalar.copy(t0ti[:], t0i[:, FS_PF:F])
    cmul6(P, FS_PF,
          (t0r[:, 0:FS_PF], t0i[:, 0:FS_PF]),
          (t1r[:, 0:FS_PF], t1i[:, 0:FS_PF]),
          p01r[:, 0:FS_PF], p01i[:, 0:FS_PF])
    cmul6p(P, TP,
           (t0tr[:], t0ti[:]),
           (t1r[:, FS_PF:F], t1i[:, FS_PF:F]),
           p01r[:, FS_PF:F], p01i[:, FS_PF:F])

    t2r, t2i = t_mms(2)

    # ---- final reduction: acc[p, k] = sum_f p01 * T2 products (DVE,
    # full width, T2 straight from PSUM).
    # Host combines: perm_r = c0 - c1, perm_i = c2 + c3
    scr2 = pers.tile([P, F], FP32, tag="ttr_scr")
    accD = pers.tile([P, 4], FP32, tag="accD")
    pairs = [(p01r, t2r), (p01i, t2i), (p01r, t2i), (p01i, t2r)]
    for k, (a, b) in enumerate(pairs):
        nc.vector.scalar_tensor_tensor(
            out=scr2[:], in0=b[:], scalar=1.0, in1=a[:],
            op0=OP.mult, op1=OP.mult, accum_out=accD[:, k:k + 1])

    dma(tens["OUT"][:, 0:4], accD[:])

    ctx.close()


# ---------------------------------------------------------------- entry point
def kernel(A_real: np.ndarray, A_imag: np.ndarray) -> np.ndarray:
    B = A_real.shape[0]
    assert B == 8 and A_real.shape == (B, N, N)
    if "nc" not in _CACHE:
        _CACHE["nc"] = build_kernel()
    nc = _CACHE["nc"]
    in_maps = [make_in_map(A_real[b], A_imag[b]) for b in range(B)]
    res = run_bass_kernel_spmd(nc, in_maps, list(range(B)))
    out = np.empty(B, dtype=np.float32)
    for b in range(B):
        acc = res.results[b]["OUT"].reshape(128, 4).astype(np.float64)
        s = acc.sum(axis=0)
        pr = s[0] - s[1]
        pi = s[2] + s[3]
        pa2 = np.float32(pr) ** 2 + np.float32(pi) ** 2
        out[b] = np.float32(EMU * SCALE2 * pa2 + DARK)
    return out


if __name__ == "__main__":
    A_real = np.load("/tmp/A_real.npy")
    A_imag = np.load("/tmp/A_imag.npy")
    print(kernel(A_real, A_imag))


# revision 35
# speedup vs baseline: 2.6947x; 1.0095x over previous
"""Trainium2 Bass kernel: boson-sampler probabilities via Glynn's permanent formula.

Math (per 18x18 complex matrix A):
  perm(A) = 2^(1-n) * sum_{d in {+-1}^n, d_0=+1} (prod_k d_k) * prod_i (sum_j d_j A[i,j])
The 2^17 sign vectors form a [128 x 1024] grid (7 "p" bits drive columns 11..17,
10 "f" bits drive columns 1..10; column 0 fixed +1). Row-sums factor as
rs_i = RP_i(p) + RF_i(f); rows are grouped [6,6,6] and each group's product
expands as T_g[p,f] = sum_{c<64} G_g[c,p] * H_g[c,f] -- fp32r matmuls on the
tensor engine (full rate at free>=256). The 64-row G/H tables (all sub-products
of 6 rows) are built hierarchically (pairs -> quads -> tables): packed operand
sets are assembled from SBUF "master" row tiles by 0/1 selection matmuls, and
each level is a set of elementwise complex-multiply ops column-split across
the DVE and GPSIMD engines. Glynn parity signs are folded into group 2's
tables via sign-scaled mask constants. G-side (p-axis) work packs real|imag
side by side in one [32, 256] master so every G matmul runs at free=256.
The final sum(T0*T1*T2) reduces via 8 scalar_tensor_tensor accumulations
(col-split DVE/Pool); |perm|^2, the (underflowed-to-zero) classical term and
the dark-count offset are applied on the host. One NeuronCore per batch
element.
"""

import sys

sys.path.insert(0, "/opt/trn_rl_repo")

import numpy as np

import concourse.bacc as bacc
import concourse.bass as bass
import concourse.tile as tile
from concourse import mybir
from concourse.bass_utils import run_bass_kernel_spmd

FP32 = mybir.dt.float32
FP32R = mybir.dt.float32r
OP = mybir.AluOpType

N = 18
PBITS, FBITS = 7, 10
P, F = 1 << PBITS, 1 << FBITS          # 128, 1024
EMU = 0.85 * (1 - 0.02) * (1 - 0.02) * (1 - 0.01)
DARK = 1e-6 * N
SCALE2 = float(2.0 ** (2 * (1 - N)))

# master row map (same for both sides; imag comp has zeros at ONES/SIGN)
M_RF = 0          # rows 0..17: row-sums RF_i / RP_i
M_SE = 18         # sign-scaled even row of pair 8 (RFe' / RPe')
M_SO = 19         # sign-scaled odd row (RFo' / RPo')
M_ONE = 20        # ones (real) / zeros (imag)
M_SGN = 21        # sign row sF / sP (real) / zeros (imag)
M_PP = 32         # rows 32..41: pair products PP_q (row 41 = PP' scaled)
M_PPS = 41        # row 41: PP' = sign-scaled pair-8 product (32-aligned for GPSIMD)
M_PAD = 42
CROWS = 48        # const-pack row count (REPA/REPB lhsTs span 48 rows)

# column split: DVE takes [0:x], GPSIMD takes [x:F] of each wide stage
FS_P1 = 704       # pair / L1 stages
FS_L2 = 576       # L2 table-build stages
FS_PF = 576       # p01 / final reduction stages

_CACHE = {}


def _pm_mask(nvals, bits):
    v = np.arange(nvals, dtype=np.uint32)
    m = (v[:, None] >> np.arange(bits, dtype=np.uint32)[None, :]) & 1
    return (1.0 - 2.0 * m).astype(np.float32).T.copy()   # [bits, nvals]


def _parity(nvals, bits):
    v = np.arange(nvals, dtype=np.uint32)
    pc = np.zeros(nvals, dtype=np.uint32)
    for k in range(bits):
        pc += (v >> k) & 1
    return np.where(pc % 2 == 0, 1.0, -1.0).astype(np.float32)


class SideSpec:
    """H: f-side (width 1024, pair-table identity at z=3);
       G: p-side (width 128, identity at z=0)."""

    def __init__(self, name, width, mult_z):
        self.name = name
        self.w = width
        self.mult_z = list(mult_z)
        self.idz = ({0, 1, 2, 3} - set(mult_z)).pop()

    # pair-table entry -> master row (pair q, entry z); identity z -> ones row
    def tab_row(self, q, z):
        if z == self.idz:
            return M_ONE
        if self.name == "H":
            return {0: M_PP + q, 1: 2 * q + 1, 2: 2 * q}[z]
        return {1: 2 * q, 2: 2 * q + 1, 3: M_PP + q}[z]

    # group-2 scaled pair-table entry (pair 8) -> master row
    def tab2_row(self, z):
        if z == self.idz:
            return M_SGN
        if self.name == "H":
            return {0: M_PPS, 1: M_SO, 2: M_SE}[z]
        return {1: M_SE, 2: M_SO, 3: M_PPS}[z]


HSPEC = SideSpec("H", F, (0, 1, 2))
GSPEC = SideSpec("G", P, (1, 2, 3))


def _sel(rows, m_pad=None):
    """Selection matrix [M_PAD, len(rows)] with one 1 per used column."""
    M = len(rows) if m_pad is None else m_pad
    s = np.zeros((M_PAD, M), np.float32)
    for m, k in enumerate(rows):
        if k is not None:
            s[k, m] = 1.0
    return s


# const pack column layout: computed once at import
def _build_pack():
    cols = {}
    blocks = []
    off = 0

    def add(name, arr):
        nonlocal off
        a = np.zeros((CROWS, arr.shape[1]), np.float32)
        a[0:arr.shape[0], :] = arr
        cols[name] = (off, arr.shape[1])
        blocks.append(a)
        off += arr.shape[1]

    # MFX: [ones;pm(10) | (ones;pm)*sF] (row 0 = ones source, row 11 = sF source)
    mF = np.concatenate([np.ones((1, F), np.float32), _pm_mask(F, FBITS)], axis=0)
    sF = _parity(F, FBITS)
    add("MFX", np.concatenate([mF, mF * sF[None, :]], axis=0))           # [22, F]
    # MPX: [pm(7) | pm*sP | ones | sP]
    mP = _pm_mask(P, PBITS)
    sP = _parity(P, PBITS)
    add("MPX", np.concatenate(
        [mP, mP * sP[None, :], np.ones((1, P), np.float32), sP[None, :]], axis=0))
    # unit columns for the widened RS matmul lhsT (cols 20,21), per side
    uh = np.zeros((22, 2), np.float32)
    uh[0, 0] = 1.0      # -> MFX row 0 (ones)
    uh[11, 1] = 1.0     # -> MFX row 11 (sF)
    ug = np.zeros((16, 2), np.float32)
    ug[14, 0] = 1.0     # -> MPX row 14 (ones)
    ug[15, 1] = 1.0     # -> MPX row 15 (sP)
    cols["_U"] = (uh, ug)

    def digits(c):
        return c % 4, (c // 4) % 4, c // 16          # za, zb, zc

    for spec in (HSPEC, GSPEC):
        nm = spec.name
        # L1: 48 distinct products tmp48 indexed by (g, c2), c2 = za + 4zb:
        # in0/in1 packed [48 | pad | 48] in one mm
        in0 = []
        in1 = []
        for g in range(3):
            for c2 in range(16):
                in0.append(spec.tab_row(3 * g, c2 % 4))
                in1.append(spec.tab_row(3 * g + 1, c2 // 4))
        add(f"SELL1_{nm}", _sel(in0 + [None] * 16 + in1, 112))
        # REP: replicate tmp48 rows into L2 src layout (lhsT for rep matmuls)
        # repA: [64g + c] <- tmp48[16g + (c % 16)] for g=0,1 ; repB: g=2
        repA = np.zeros((48, 128), np.float32)
        for g in range(2):
            for c in range(64):
                repA[16 * g + (c % 16), 64 * g + c] = 1.0
        add(f"REPA_{nm}", repA)
        repB = np.zeros((48, 64), np.float32)
        for c in range(64):
            repB[32 + (c % 16), c] = 1.0
        add(f"REPB_{nm}", repB)
        # L2 c-packs: set1 rows [64g+c] = tab_c[zc]; set2 = scaled tab2'[zc]
        rows = []
        for g in range(2):
            for c in range(64):
                rows.append(spec.tab_row(3 * g + 2, digits(c)[2]))
        add(f"SELL2A_{nm}", _sel(rows))
        rows = [spec.tab2_row(digits(c)[2]) for c in range(64)]
        add(f"SELL2B_{nm}", _sel(rows))
    return np.concatenate(blocks, axis=1), cols


CPACK, CPACK_COLS = _build_pack()
U_HG = CPACK_COLS.pop("_U")
CW = CPACK.shape[1]

def _build_packr():
    parts = []
    offs = {}
    off = 0
    for nm in ("REPA_H", "REPB_H", "REPA_G", "REPB_G"):
        o, w = CPACK_COLS[nm]
        parts.append(CPACK[0:CROWS, o:o + w])
        offs[nm] = (off, w)
        off += w
    return np.ascontiguousarray(np.concatenate(parts, axis=1)), offs


CPACKR, CPACKR_COLS = _build_packr()
CRW = CPACKR.shape[1]


def build_lts(Ar, Ai):
    """Host-side lhsT pack: per (side, comp) the widened row-sum lhsT
    (32 cols) and the pair-pack lhsT (42 cols), one [22, 296] tensor."""
    lts = np.zeros((22, 296), np.float32)
    for si, (lo, kb, K) in enumerate(((0, 11, 22), (11, 7, 16))):
        for ci, A in ((0, Ar), (1, Ai)):
            base = 74 * (2 * si + ci)
            AT = np.ascontiguousarray(A.T, dtype=np.float32)
            lts[0:kb, base:base + 18] = AT[lo:lo + kb, 0:18]
            lts[kb:2 * kb, base + 18:base + 20] = AT[lo:lo + kb, 16:18]
            if ci == 0:
                lts[0:K, base + 20:base + 22] = U_HG[si]
            pb = base + 32
            lts[0:kb, pb:pb + 9] = AT[lo:lo + kb, 0:17:2]
            lts[kb:2 * kb, pb + 9] = AT[lo:lo + kb, 16]
            lts[0:kb, pb + 32:pb + 41] = AT[lo:lo + kb, 1:18:2]
            lts[0:kb, pb + 41] = AT[lo:lo + kb, 17]
    return lts


def make_in_map(Ar, Ai):
    return {"CPACK": CPACK, "LTS": build_lts(Ar, Ai)}


def host_consts():
    return {"CPACK": CPACK}


# ---------------------------------------------------------------- kernel body
def build_kernel(loop_iters=None):
    nc = bacc.Bacc("TRN2", target_bir_lowering=False, debug=False)

    tens = {}
    tens["LTS"] = nc.dram_tensor("LTS", [22, 296], FP32, kind="ExternalInput").ap()
    tens["CPACK"] = nc.dram_tensor("CPACK", [CROWS, CW], FP32,
                                   kind="ExternalInput").ap()
    tens["OUT"] = nc.dram_tensor("OUT", [128, 4], FP32, kind="ExternalOutput").ap()

    with tile.TileContext(nc) as tc:
        if loop_iters is None:
            _body(nc, tc, tens)
        else:
            with tc.For_i(0, loop_iters, 1):
                _body(nc, tc, tens)
    nc.compile()
    return nc


def _body(nc, tc, tens):
    from contextlib import ExitStack

    ctx = ExitStack()
    pers = ctx.enter_context(tc.tile_pool(name="pers", bufs=1))
    pk = ctx.enter_context(tc.tile_pool(name="pk", bufs=2))
    cm = ctx.enter_context(tc.tile_pool(name="cm", bufs=2))
    psum_pool = ctx.enter_context(tc.tile_pool(name="psum", bufs=4, space="PSUM"))
    dma = nc.sync.dma_start
    dma2 = nc.gpsimd.dma_start          # SWDGE queues for small input loads

    def mmr(out_ap, lhsT_ap, rhs_ap, start=True, stop=True):
        """fp32r matmul: full rate (1 cyc/row) on trn2 when free >= 256."""
        nc.tensor.matmul(out_ap, lhsT_ap.bitcast(FP32R), rhs_ap.bitcast(FP32R),
                         start=start, stop=stop)

    def cmul6(rows, w, i0, i1, outr, outi):
        """DVE complex multiply: (i0r,i0i)*(i1r,i1i) -> (outr,outi)."""
        e = nc.vector
        i0r, i0i = i0
        i1r, i1i = i1
        t1 = cm.tile([rows, w], FP32, tag="cm_t1")
        t2 = cm.tile([rows, w], FP32, tag="cm_t2")
        e.tensor_mul(t1[:], i0r, i1r)
        e.tensor_mul(t2[:], i0i, i1i)
        e.tensor_sub(outr, t1[:], t2[:])
        e.tensor_mul(t1[:], i0r, i1i)
        e.tensor_mul(t2[:], i0i, i1r)
        e.tensor_add(outi, t1[:], t2[:])

    def cmul6p(rows, w, i0, i1, outr, outi):
        """GPSIMD complex multiply (plain tensor-tensor ops)."""
        e = nc.gpsimd
        i0r, i0i = i0
        i1r, i1i = i1
        t1 = cm.tile([rows, w], FP32, tag="gp_t1")
        t2 = cm.tile([rows, w], FP32, tag="gp_t2")
        e.tensor_mul(t1[:], i0r, i1r)
        e.tensor_mul(t2[:], i0i, i1i)
        e.tensor_sub(outr, t1[:], t2[:])
        e.tensor_mul(t1[:], i0r, i1i)
        e.tensor_mul(t2[:], i0i, i1r)
        e.tensor_add(outi, t1[:], t2[:])

    def sel_mm(sel_sb, msrc, m, w):
        """Pack = SEL.T @ master -> PSUM [m, w]."""
        ps = psum_pool.tile([m, w], FP32, tag="ps")
        for c0 in range(0, w, 512):
            c1 = min(c0 + 512, w)
            nc.tensor.matmul(ps[:, c0:c1], sel_sb[:], msrc[:, c0:c1],
                             start=True, stop=True)
        return ps

    # ---- stage 0: A loads, widened row-sum matmuls -> master rows 0..21
    cpk = pers.tile([CROWS, CW], FP32, tag="cpack")
    dma(cpk[:, 0:128], tens["CPACK"][:, 0:128])
    lts = pers.tile([22, 296], FP32, tag="lts")
    dma(lts[:], tens["LTS"][:, :])
    dma(cpk[:, 128:512], tens["CPACK"][:, 128:512])
    nc.scalar.dma_start(cpk[:, 512:1024], tens["CPACK"][:, 512:1024])
    dma2(cpk[:, 1024:CW], tens["CPACK"][:, 1024:CW])

    warm = psum_pool.tile([128, 64], FP32, tag="ps")
    for _ in range(8):
        nc.tensor.matmul(warm[:], cpk[0:48, 0:128], cpk[0:48, 0:64],
                         start=True, stop=True)

    def cslice(name, nrows=None):
        off, width = CPACK_COLS[name]
        nr = M_PAD if nrows is None else nrows
        return cpk[0:nr, off:off + width]

    lhsT_rs = {}
    lhsT_pp = {}
    KRS = {"H": 22, "G": 16}
    for si, side in enumerate("HG"):
        for ci, nm in enumerate("ri"):
            base = 74 * (2 * si + ci)
            K_rs = KRS[side]
            lhsT_rs[(side, nm)] = lts[0:K_rs, base:base + 32]
            lhsT_pp[(side, nm)] = lts[0:K_rs, base + 32:base + 74]

    mask_sb = {"H": cslice("MFX", 22), "G": cslice("MPX", 16)}
    sel_sb = {}
    for spec in (HSPEC, GSPEC):
        for s in ("SELL1", "SELL2A", "SELL2B"):
            key = f"{s}_{spec.name}"
            sel_sb[key] = cslice(key)
        for s in ("REPA", "REPB"):
            key = f"{s}_{spec.name}"
            sel_sb[key] = cslice(key, 48)

    # H masters: one [M_PAD, F] tile per component; G master: [M_PAD, 2P]
    # with real in cols 0:P, imag in P:2P. All 32 rows get written (22 by
    # the RS copy, 10 by the pair stage) -- no memset needed.
    masterH = {}
    for nm in "ri":
        t = pers.tile([M_PAD, F], FP32, tag=f"mstH{nm}", name=f"mstH{nm}")
        masterH[nm] = t
    masterG = pers.tile([M_PAD, 2 * P], FP32, tag="mstG")

    # ---- stage 1: pair products -> master rows 32..41 (packs come
    # straight from the rearranged ltp lhsTs -- no master dependency)
    P1T = F - FS_P1
    psH1 = {}
    for nm in "ri":
        ps = sel_mm(lhsT_pp[("H", nm)], mask_sb["H"], 42, F)
        sb = pk.tile([10, F], FP32, tag=f"halfH{nm}")
        if nm == "r":
            nc.scalar.copy(sb[:], ps[32:42, :])
        else:
            nc.vector.tensor_copy(sb[:], ps[32:42, :])
        p0t = pk.tile([10, P1T], FP32, tag=f"p0tH{nm}")
        nc.scalar.copy(p0t[:], ps[0:10, FS_P1:F])
        psH1[nm] = (ps, sb, p0t)
    cmul6(10, FS_P1,
          (psH1["r"][0][0:10, 0:FS_P1], psH1["i"][0][0:10, 0:FS_P1]),
          (psH1["r"][1][:, 0:FS_P1], psH1["i"][1][:, 0:FS_P1]),
          masterH["r"][M_PP:M_PP + 10, 0:FS_P1],
          masterH["i"][M_PP:M_PP + 10, 0:FS_P1])
    cmul6p(10, P1T,
           (psH1["r"][2][:], psH1["i"][2][:]),
           (psH1["r"][1][:, FS_P1:F], psH1["i"][1][:, FS_P1:F]),
           masterH["r"][M_PP:M_PP + 10, FS_P1:F],
           masterH["i"][M_PP:M_PP + 10, FS_P1:F])

    psG1 = psum_pool.tile([42, 2 * P], FP32, tag="ps")
    nc.tensor.matmul(psG1[:, 0:P], lhsT_pp[("G", "r")], mask_sb["G"][:],
                     start=True, stop=True)
    nc.tensor.matmul(psG1[:, P:2 * P], lhsT_pp[("G", "i")], mask_sb["G"][:],
                     start=True, stop=True)
    sbG1a = pk.tile([10, 2 * P], FP32, tag="selpGa")
    nc.scalar.copy(sbG1a[:], psG1[0:10, :])
    sbG1b = pk.tile([10, 2 * P], FP32, tag="selpGb")
    nc.scalar.copy(sbG1b[:], psG1[32:42, :])
    cmul6p(10, P,
           (sbG1a[:, 0:P], sbG1a[:, P:2 * P]),
           (sbG1b[:, 0:P], sbG1b[:, P:2 * P]),
           masterG[M_PP:M_PP + 10, 0:P], masterG[M_PP:M_PP + 10, P:2 * P])

    # row-sum masters (needed from L1 onward; emitted after the pair
    # stage so its PSUM evacuations win the ACT queue early)
    for nm in "ri":
        lt = lhsT_rs[("H", nm)]
        ps = psum_pool.tile([32, F], FP32, tag="ps")
        for c0 in range(0, F, 512):
            c1 = min(c0 + 512, F)
            nc.tensor.matmul(ps[:, c0:c1], lt, mask_sb["H"][:, c0:c1],
                             start=True, stop=True)
        nc.scalar.copy(masterH[nm][0:32, :], ps[:])
    psG = psum_pool.tile([32, 2 * P], FP32, tag="ps")
    nc.tensor.matmul(psG[:, 0:P], lhsT_rs[("G", "r")], mask_sb["G"][:],
                     start=True, stop=True)
    nc.tensor.matmul(psG[:, P:2 * P], lhsT_rs[("G", "i")], mask_sb["G"][:],
                     start=True, stop=True)
    nc.scalar.copy(masterG[0:32, :], psG[:])


    # ---- stage 2 (L1): tmp48[16g + c2] = tab_a[za] * tab_b[zb]
    t48H = {}
    l1t = {}
    for nm in "ri":
        ps = sel_mm(sel_sb["SELL1_H"], masterH[nm][:], 112, F)
        sb1 = pk.tile([48, F], FP32, tag=f"l1hH{nm}")
        if nm == "r":
            nc.scalar.copy(sb1[:], ps[64:112, :])
        else:
            nc.vector.tensor_copy(sb1[:], ps[64:112, :])
        lt1 = pk.tile([48, P1T], FP32, tag=f"l1tH{nm}")
        nc.scalar.copy(lt1[:], ps[0:48, FS_P1:F])
        l1t[nm] = lt1
        t48 = pers.tile([48, F], FP32, tag=f"t48H{nm}", name=f"t48H{nm}")
        t48H[nm] = (ps, sb1, t48)
    cmul6(48, FS_P1,
          (t48H["r"][0][0:48, 0:FS_P1], t48H["i"][0][0:48, 0:FS_P1]),
          (t48H["r"][1][:, 0:FS_P1], t48H["i"][1][:, 0:FS_P1]),
          t48H["r"][2][:, 0:FS_P1], t48H["i"][2][:, 0:FS_P1])
    cmul6p(48, P1T,
           (l1t["r"][:], l1t["i"][:]),
           (t48H["r"][1][:, FS_P1:F], t48H["i"][1][:, FS_P1:F]),
           t48H["r"][2][:, FS_P1:F], t48H["i"][2][:, FS_P1:F])

    psL1G = sel_mm(sel_sb["SELL1_G"], masterG[:], 112, 2 * P)
    sbL1Ga = pk.tile([48, 2 * P], FP32, tag="l1Ga")
    nc.scalar.copy(sbL1Ga[:], psL1G[0:48, :])
    sbL1Gb = pk.tile([48, 2 * P], FP32, tag="l1Gb")
    nc.scalar.copy(sbL1Gb[:], psL1G[64:112, :])
    t48G = pers.tile([48, 2 * P], FP32, tag="t48G", name="t48G")
    cmul6p(48, P,
           (sbL1Ga[:, 0:P], sbL1Ga[:, P:2 * P]),
           (sbL1Gb[:, 0:P], sbL1Gb[:, P:2 * P]),
           t48G[:, 0:P], t48G[:, P:2 * P])

    # ---- stage 3 (L2): e_g = tmp * tab_c[zc], column-split DVE / GPSIMD
    # H g0+g1 fused as one [128, F] set -> eRH = [H0r; H1r], eIH = [H0i; H1i].
    # The T matmuls compensate with K=64 accumulating pairs.
    TL = F - FS_L2
    c01H = {}
    repH = {}
    reptH = {}
    for nm in "ri":
        c01 = sel_mm(sel_sb["SELL2A_H"], masterH[nm][:], 128, F)
        c01sb = pk.tile([128, F], FP32, tag=f"c01H{nm}")
        if nm == "r":
            nc.scalar.copy(c01sb[:], c01[:])
        else:
            nc.vector.tensor_copy(c01sb[:], c01[:])
        c01H[nm] = c01sb
        rep = sel_mm(sel_sb["REPA_H"], t48H[nm][2][:], 128, F)
        repH[nm] = rep
        rt = pk.tile([128, TL], FP32, tag=f"reptH{nm}")
        nc.scalar.copy(rt[:], repH[nm][:, FS_L2:F])
        reptH[nm] = rt
    eRH = pers.tile([128, F], FP32R, tag="eRH", name="eRH")
    eIH = pers.tile([128, F], FP32R, tag="eIH", name="eIH")
    cmul6(128, FS_L2,
          (repH["r"][0:128, 0:FS_L2], repH["i"][0:128, 0:FS_L2]),
          (c01H["r"][:, 0:FS_L2], c01H["i"][:, 0:FS_L2]),
          eRH[:, 0:FS_L2], eIH[:, 0:FS_L2])
    cmul6p(128, TL,
           (reptH["r"][:], reptH["i"][:]),
           (c01H["r"][:, FS_L2:F], c01H["i"][:, FS_L2:F]),
           eRH[:, FS_L2:F], eIH[:, FS_L2:F])

    cp2H = {}
    rbH = {}
    rbtH = {}
    for nm in "ri":
        c2 = sel_mm(sel_sb["SELL2B_H"], masterH[nm][:], 64, F)
        c2sb = pk.tile([64, F], FP32, tag=f"c2H{nm}")
        nc.scalar.copy(c2sb[:], c2[:])
        cp2H[nm] = c2sb
        rb = sel_mm(sel_sb["REPB_H"], t48H[nm][2][:], 64, F)
        rbH[nm] = rb
        rbt = pk.tile([64, TL], FP32, tag=f"rbtH{nm}")
        nc.scalar.copy(rbt[:], rb[:, FS_L2:F])
        rbtH[nm] = rbt
    eR2H = pers.tile([64, F], FP32R, tag="eR2H", name="eR2H")
    eI2H = pers.tile([64, F], FP32R, tag="eI2H", name="eI2H")
    cmul6(64, FS_L2,
          (rbH["r"][:, 0:FS_L2], rbH["i"][:, 0:FS_L2]),
          (cp2H["r"][:, 0:FS_L2], cp2H["i"][:, 0:FS_L2]),
          eR2H[:, 0:FS_L2], eI2H[:, 0:FS_L2])
    cmul6p(64, TL,
           (rbtH["r"][:], rbtH["i"][:]),
           (cp2H["r"][:, FS_L2:F], cp2H["i"][:, FS_L2:F]),
           eR2H[:, FS_L2:F], eI2H[:, FS_L2:F])

    # G side (GPSIMD, SBUF operands via single ACT evacuations)
    repG = sel_mm(sel_sb["REPA_G"], t48G[:], 128, 2 * P)
    repGsb = pk.tile([128, 2 * P], FP32, tag="repG")
    nc.scalar.copy(repGsb[:], repG[:])
    c01G = sel_mm(sel_sb["SELL2A_G"], masterG[:], 128, 2 * P)
    c01Gsb = pk.tile([128, 2 * P], FP32, tag="c01G")
    nc.scalar.copy(c01Gsb[:], c01G[:])
    eG01 = pers.tile([128, 2 * P], FP32R, tag="eG01", name="eG01")
    cmul6p(128, P,
           (repGsb[:, 0:P], repGsb[:, P:2 * P]),
           (c01Gsb[:, 0:P], c01Gsb[:, P:2 * P]),
           eG01[:, 0:P], eG01[:, P:2 * P])

    rbG = sel_mm(sel_sb["REPB_G"], t48G[:], 64, 2 * P)
    rbGsb = pk.tile([64, 2 * P], FP32, tag="rbG")
    nc.scalar.copy(rbGsb[:], rbG[:])
    c2G = sel_mm(sel_sb["SELL2B_G"], masterG[:], 64, 2 * P)
    c2Gsb = pk.tile([64, 2 * P], FP32, tag="c2G")
    nc.scalar.copy(c2Gsb[:], c2G[:])
    eG2 = pers.tile([64, 2 * P], FP32R, tag="eG2", name="eG2")
    cmul6p(64, P,
           (rbGsb[:, 0:P], rbGsb[:, P:2 * P]),
           (c2Gsb[:, 0:P], c2Gsb[:, P:2 * P]),
           eG2[:, 0:P], eG2[:, P:2 * P])

    # negated imag halves (lhsT for the real-part T matmuls)
    negG01 = pers.tile([128, P], FP32R, tag="negG01")
    nc.scalar.mul(negG01[:], eG01[:, P:2 * P], -1.0)
    negG2 = pers.tile([64, P], FP32R, tag="negG2")
    nc.scalar.mul(negG2[:], eG2[:, P:2 * P], -1.0)

    # ---- stage 4: T matmuls -- per (group, comp, chunk) a K=64 pair
    # accumulated in PSUM: Tr = Gr^T Hr + (-Gi)^T Hi ; Ti = Gr^T Hi + Gi^T Hr
    # Order: T0, T1 (combine inputs) first, then T2 (only needed by the
    # final reduction) so p01 overlaps the T2 matmuls.
    def t_mms(g):
        if g < 2:
            Gr = eG01[64 * g:64 * g + 64, 0:P]
            Gi = eG01[64 * g:64 * g + 64, P:2 * P]
            Gin = negG01[64 * g:64 * g + 64, :]
            Hr = eRH[64 * g:64 * g + 64, :]
            Hi = eIH[64 * g:64 * g + 64, :]
        else:
            Gr = eG2[:, 0:P]
            Gi = eG2[:, P:2 * P]
            Gin = negG2[:]
            Hr = eR2H[:]
            Hi = eI2H[:]
        tr = psum_pool.tile([P, F], FP32, tag="ps")
        ti = psum_pool.tile([P, F], FP32, tag="ps")
        for c0 in range(0, F, 512):
            c1 = c0 + 512
            mmr(tr[:, c0:c1], Gr, Hr[:, c0:c1], start=True, stop=False)
            mmr(tr[:, c0:c1], Gin, Hi[:, c0:c1], start=False, stop=True)
            mmr(ti[:, c0:c1], Gr, Hi[:, c0:c1], start=True, stop=False)
            mmr(ti[:, c0:c1], Gi, Hr[:, c0:c1], start=False, stop=True)
        return tr, ti

    t0r, t0i = t_mms(0)
    t1r_ps, t1i_ps = t_mms(1)
    t1r = pers.tile([P, F], FP32, tag="T1r")
    t1i = pers.tile([P, F], FP32, tag="T1i")
    for c0 in range(0, F, 512):
        c1 = c0 + 512
        nc.scalar.copy(t1r[:, c0:c1], t1r_ps[:, c0:c1])
        nc.vector.tensor_copy(t1i[:, c0:c1], t1i_ps[:, c0:c1])

    # p01 = T0*T1: col-split DVE (T0 from PSUM) / GPSIMD (T0 tail via ACT)
    TP = F - FS_PF
    p01r = pers.tile([P, F], FP32, tag="p01r")
    p01i = pers.tile([P, F], FP32, tag="p01i")
    t0tr = pers.tile([P, TP], FP32, tag="t0tr")
    t0ti = pers.tile([P, TP], FP32, tag="t0ti")
    nc.scalar.copy(t0tr[:], t0r[:, FS_PF:F])
    nc.scalar.copy(t0ti[:], t0i[:, FS_PF:F])
    cmul6(P, FS_PF,
          (t0r[:, 0:FS_PF], t0i[:, 0:FS_PF]),
          (t1r[:, 0:FS_PF], t1i[:, 0:FS_PF]),
          p01r[:, 0:FS_PF], p01i[:, 0:FS_PF])
    cmul6p(P, TP,
           (t0tr[:], t0ti[:]),
           (t1r[:, FS_PF:F], t1i[:, FS_PF:F]),
           p01r[:, FS_PF:F], p01i[:, FS_PF:F])

    t2r, t2i = t_mms(2)

    # ---- final reduction: acc[p, k] = sum_f p01 * T2 products (DVE,
    # full width, T2 straight from PSUM).
    # Host combines: perm_r = c0 - c1, perm_i = c2 + c3
    scr2 = pers.tile([P, F], FP32, tag="ttr_scr")
    accD = pers.tile([P, 4], FP32, tag="accD")
    pairs = [(p01r, t2r), (p01i, t2i), (p01r, t2i), (p01i, t2r)]
    for k, (a, b) in enumerate(pairs):
        nc.vector.scalar_tensor_tensor(
            out=scr2[:], in0=b[:], scalar=1.0, in1=a[:],
            op0=OP.mult, op1=OP.mult, accum_out=accD[:, k:k + 1])

    dma(tens["OUT"][:, 0:4], accD[:])

    ctx.close()


# ---------------------------------------------------------------- entry point
def kernel(A_real: np.ndarray, A_imag: np.ndarray) -> np.ndarray:
    B = A_real.shape[0]
    assert B == 8 and A_real.shape == (B, N, N)
    if "nc" not in _CACHE:
        _CACHE["nc"] = build_kernel()
    nc = _CACHE["nc"]
    in_maps = [make_in_map(A_real[b], A_imag[b]) for b in range(B)]
    res = run_bass_kernel_spmd(nc, in_maps, list(range(B)))
    out = np.empty(B, dtype=np.float32)
    for b in range(B):
        acc = res.results[b]["OUT"].reshape(128, 4).astype(np.float64)
        s = acc.sum(axis=0)
        pr = s[0] - s[1]
        pi = s[2] + s[3]
        pa2 = np.float32(pr) ** 2 + np.float32(pi) ** 2
        out[b] = np.float32(EMU * SCALE2 * pa2 + DARK)
    return out


if __name__ == "__main__":
    A_real = np.load("/tmp/A_real.npy")
    A_imag = np.load("/tmp/A_imag.npy")
    print(kernel(A_real, A_imag))


# revision 36
# speedup vs baseline: 2.8711x; 1.0655x over previous
"""Trainium2 Bass kernel: boson-sampler probabilities via Glynn's permanent formula.

Math (per 18x18 complex matrix A):
  perm(A) = 2^(1-n) * sum_{d in {+-1}^n, d_0=+1} (prod_k d_k) * prod_i (sum_j d_j A[i,j])
The 2^17 sign vectors form a [128 x 1024] grid (7 "p" bits drive columns 11..17,
10 "f" bits drive columns 1..10; column 0 fixed +1). Row-sums factor as
rs_i = RP_i(p) + RF_i(f); rows are grouped [6,6,6] and each group's product
expands as T_g[p,f] = sum_{c<64} G_g[c,p] * H_g[c,f] -- fp32r matmuls on the
tensor engine (full rate at free>=256). The 64-row G/H tables (all sub-products
of 6 rows) are built hierarchically (pairs -> quads -> tables): packed operand
sets are assembled from SBUF "master" row tiles by 0/1 selection matmuls, and
each level is a set of elementwise complex-multiply ops column-split across
the DVE and GPSIMD engines. Glynn parity signs are folded into group 2's
tables via sign-scaled mask constants. G-side (p-axis) work packs real|imag
side by side in one [32, 256] master so every G matmul runs at free=256.
The final sum(T0*T1*T2) reduces via 8 scalar_tensor_tensor accumulations
(col-split DVE/Pool); |perm|^2, the (underflowed-to-zero) classical term and
the dark-count offset are applied on the host. One NeuronCore per batch
element.
"""

import sys

sys.path.insert(0, "/opt/trn_rl_repo")

import numpy as np

import concourse.bacc as bacc
import concourse.bass as bass
import concourse.tile as tile
from concourse import mybir
from concourse.bass_utils import run_bass_kernel_spmd

FP32 = mybir.dt.float32
FP32R = mybir.dt.float32r
OP = mybir.AluOpType

N = 18
PBITS, FBITS = 7, 10
P, F = 1 << PBITS, 1 << FBITS          # 128, 1024
EMU = 0.85 * (1 - 0.02) * (1 - 0.02) * (1 - 0.01)
DARK = 1e-6 * N
SCALE2 = float(2.0 ** (2 * (1 - N)))

# master row map (same for both sides; imag comp has zeros at ONES/SIGN)
M_RF = 0          # rows 0..17: row-sums RF_i / RP_i
M_SE = 18         # sign-scaled even row of pair 8 (RFe' / RPe')
M_SO = 19         # sign-scaled odd row (RFo' / RPo')
M_ONE = 20        # ones (real) / zeros (imag)
M_SGN = 21        # sign row sF / sP (real) / zeros (imag)
M_PP = 32         # rows 32..41: pair products PP_q (row 41 = PP' scaled)
M_PPS = 41        # row 41: PP' = sign-scaled pair-8 product (32-aligned for GPSIMD)
M_PAD = 42
CROWS = 48        # const-pack row count (REPA/REPB lhsTs span 48 rows)

# column split: DVE takes [0:x], GPSIMD takes [x:F] of each wide stage
FS_P1 = 704       # pair / L1 stages
FS_L2 = 576       # L2 table-build stages
FS_PF = 576       # p01 / final reduction stages

_CACHE = {}


def _pm_mask(nvals, bits):
    v = np.arange(nvals, dtype=np.uint32)
    m = (v[:, None] >> np.arange(bits, dtype=np.uint32)[None, :]) & 1
    return (1.0 - 2.0 * m).astype(np.float32).T.copy()   # [bits, nvals]


def _parity(nvals, bits):
    v = np.arange(nvals, dtype=np.uint32)
    pc = np.zeros(nvals, dtype=np.uint32)
    for k in range(bits):
        pc += (v >> k) & 1
    return np.where(pc % 2 == 0, 1.0, -1.0).astype(np.float32)


class SideSpec:
    """H: f-side (width 1024, pair-table identity at z=3);
       G: p-side (width 128, identity at z=0)."""

    def __init__(self, name, width, mult_z):
        self.name = name
        self.w = width
        self.mult_z = list(mult_z)
        self.idz = ({0, 1, 2, 3} - set(mult_z)).pop()

    # pair-table entry -> master row (pair q, entry z); identity z -> ones row
    def tab_row(self, q, z):
        if z == self.idz:
            return M_ONE
        if self.name == "H":
            return {0: M_PP + q, 1: 2 * q + 1, 2: 2 * q}[z]
        return {1: 2 * q, 2: 2 * q + 1, 3: M_PP + q}[z]

    # group-2 scaled pair-table entry (pair 8) -> master row
    def tab2_row(self, z):
        if z == self.idz:
            return M_SGN
        if self.name == "H":
            return {0: M_PPS, 1: M_SO, 2: M_SE}[z]
        return {1: M_SE, 2: M_SO, 3: M_PPS}[z]


HSPEC = SideSpec("H", F, (0, 1, 2))
GSPEC = SideSpec("G", P, (1, 2, 3))


def _sel(rows, m_pad=None):
    """Selection matrix [M_PAD, len(rows)] with one 1 per used column."""
    M = len(rows) if m_pad is None else m_pad
    s = np.zeros((M_PAD, M), np.float32)
    for m, k in enumerate(rows):
        if k is not None:
            s[k, m] = 1.0
    return s


# const pack column layout: computed once at import
def _build_pack():
    cols = {}
    blocks = []
    off = 0

    def add(name, arr):
        nonlocal off
        a = np.zeros((CROWS, arr.shape[1]), np.float32)
        a[0:arr.shape[0], :] = arr
        cols[name] = (off, arr.shape[1])
        blocks.append(a)
        off += arr.shape[1]

    # MFX: [ones;pm(10) | (ones;pm)*sF] (row 0 = ones source, row 11 = sF source)
    mF = np.concatenate([np.ones((1, F), np.float32), _pm_mask(F, FBITS)], axis=0)
    sF = _parity(F, FBITS)
    add("MFX", np.concatenate([mF, mF * sF[None, :]], axis=0))           # [22, F]
    # MPX: [pm(7) | pm*sP | ones | sP]
    mP = _pm_mask(P, PBITS)
    sP = _parity(P, PBITS)
    add("MPX", np.concatenate(
        [mP, mP * sP[None, :], np.ones((1, P), np.float32), sP[None, :]], axis=0))
    # unit columns for the widened RS matmul lhsT (cols 20,21), per side
    uh = np.zeros((22, 2), np.float32)
    uh[0, 0] = 1.0      # -> MFX row 0 (ones)
    uh[11, 1] = 1.0     # -> MFX row 11 (sF)
    ug = np.zeros((16, 2), np.float32)
    ug[14, 0] = 1.0     # -> MPX row 14 (ones)
    ug[15, 1] = 1.0     # -> MPX row 15 (sP)
    cols["_U"] = (uh, ug)

    def digits(c):
        return c % 4, (c // 4) % 4, c // 16          # za, zb, zc

    for spec in (HSPEC, GSPEC):
        nm = spec.name
        # L1: 48 distinct products tmp48 indexed by (g, c2), c2 = za + 4zb:
        # in0/in1 packed [48 | pad | 48] in one mm
        in0 = []
        in1 = []
        for g in range(3):
            for c2 in range(16):
                in0.append(spec.tab_row(3 * g, c2 % 4))
                in1.append(spec.tab_row(3 * g + 1, c2 // 4))
        add(f"SELL1_{nm}", _sel(in0 + [None] * 16 + in1, 112))
        # REP: replicate tmp48 rows into L2 src layout (lhsT for rep matmuls)
        # repA: [64g + c] <- tmp48[16g + (c % 16)] for g=0,1 ; repB: g=2
        repA = np.zeros((48, 128), np.float32)
        for g in range(2):
            for c in range(64):
                repA[16 * g + (c % 16), 64 * g + c] = 1.0
        add(f"REPA_{nm}", repA)
        repB = np.zeros((48, 64), np.float32)
        for c in range(64):
            repB[32 + (c % 16), c] = 1.0
        add(f"REPB_{nm}", repB)
        # L2 c-packs: set1 rows [64g+c] = tab_c[zc]; set2 = scaled tab2'[zc]
        rows = []
        for g in range(2):
            for c in range(64):
                rows.append(spec.tab_row(3 * g + 2, digits(c)[2]))
        add(f"SELL2A_{nm}", _sel(rows))
        rows = [spec.tab2_row(digits(c)[2]) for c in range(64)]
        add(f"SELL2B_{nm}", _sel(rows))
    return np.concatenate(blocks, axis=1), cols


CPACK, CPACK_COLS = _build_pack()
U_HG = CPACK_COLS.pop("_U")
CW = CPACK.shape[1]

def _build_packr():
    parts = []
    offs = {}
    off = 0
    for nm in ("REPA_H", "REPB_H", "REPA_G", "REPB_G"):
        o, w = CPACK_COLS[nm]
        parts.append(CPACK[0:CROWS, o:o + w])
        offs[nm] = (off, w)
        off += w
    return np.ascontiguousarray(np.concatenate(parts, axis=1)), offs


CPACKR, CPACKR_COLS = _build_packr()
CRW = CPACKR.shape[1]


def build_lts(Ar, Ai):
    """Host-side lhsT pack: per (side, comp) the widened row-sum lhsT
    (32 cols) and the pair-pack lhsT (42 cols), one [22, 296] tensor."""
    lts = np.zeros((22, 296), np.float32)
    for si, (lo, kb, K) in enumerate(((0, 11, 22), (11, 7, 16))):
        for ci, A in ((0, Ar), (1, Ai)):
            base = 74 * (2 * si + ci)
            AT = np.ascontiguousarray(A.T, dtype=np.float32)
            lts[0:kb, base:base + 18] = AT[lo:lo + kb, 0:18]
            lts[kb:2 * kb, base + 18:base + 20] = AT[lo:lo + kb, 16:18]
            if ci == 0:
                lts[0:K, base + 20:base + 22] = U_HG[si]
            pb = base + 32
            lts[0:kb, pb:pb + 9] = AT[lo:lo + kb, 0:17:2]
            lts[kb:2 * kb, pb + 9] = AT[lo:lo + kb, 16]
            lts[0:kb, pb + 32:pb + 41] = AT[lo:lo + kb, 1:18:2]
            lts[0:kb, pb + 41] = AT[lo:lo + kb, 17]
    return lts


def make_in_map(Ar, Ai):
    return {"CPACK": CPACK, "CPACKR": CPACKR, "LTS": build_lts(Ar, Ai)}


def host_consts():
    return {"CPACK": CPACK}


# ---------------------------------------------------------------- kernel body
def build_kernel(loop_iters=None):
    nc = bacc.Bacc("TRN2", target_bir_lowering=False, debug=False)

    tens = {}
    tens["LTS"] = nc.dram_tensor("LTS", [22, 296], FP32, kind="ExternalInput").ap()
    tens["CPACK"] = nc.dram_tensor("CPACK", [CROWS, CW], FP32,
                                   kind="ExternalInput").ap()
    tens["CPACKR"] = nc.dram_tensor("CPACKR", [CROWS, CRW], FP32R,
                                    kind="ExternalInput").ap()
    tens["OUT"] = nc.dram_tensor("OUT", [128, 4], FP32, kind="ExternalOutput").ap()

    with tile.TileContext(nc) as tc:
        if loop_iters is None:
            _body(nc, tc, tens)
        else:
            with tc.For_i(0, loop_iters, 1):
                _body(nc, tc, tens)
    nc.compile()
    return nc


def _body(nc, tc, tens):
    from contextlib import ExitStack

    ctx = ExitStack()
    pers = ctx.enter_context(tc.tile_pool(name="pers", bufs=1))
    pk = ctx.enter_context(tc.tile_pool(name="pk", bufs=2))
    cm = ctx.enter_context(tc.tile_pool(name="cm", bufs=2))
    psum_pool = ctx.enter_context(tc.tile_pool(name="psum", bufs=4, space="PSUM"))
    dma = nc.sync.dma_start
    dma2 = nc.gpsimd.dma_start          # SWDGE queues for small input loads

    def mmr(out_ap, lhsT_ap, rhs_ap, start=True, stop=True):
        """fp32r matmul: full rate (1 cyc/row) on trn2 when free >= 256."""
        nc.tensor.matmul(out_ap, lhsT_ap.bitcast(FP32R), rhs_ap.bitcast(FP32R),
                         start=start, stop=stop)

    def cmul6(rows, w, i0, i1, outr, outi):
        """DVE complex multiply: (i0r,i0i)*(i1r,i1i) -> (outr,outi)."""
        e = nc.vector
        i0r, i0i = i0
        i1r, i1i = i1
        t1 = cm.tile([rows, w], FP32, tag="cm_t1")
        t2 = cm.tile([rows, w], FP32, tag="cm_t2")
        e.tensor_mul(t1[:], i0r, i1r)
        e.tensor_mul(t2[:], i0i, i1i)
        e.tensor_sub(outr, t1[:], t2[:])
        e.tensor_mul(t1[:], i0r, i1i)
        e.tensor_mul(t2[:], i0i, i1r)
        e.tensor_add(outi, t1[:], t2[:])

    def cmul6p(rows, w, i0, i1, outr, outi):
        """GPSIMD complex multiply (plain tensor-tensor ops)."""
        e = nc.gpsimd
        i0r, i0i = i0
        i1r, i1i = i1
        t1 = cm.tile([rows, w], FP32, tag="gp_t1")
        t2 = cm.tile([rows, w], FP32, tag="gp_t2")
        e.tensor_mul(t1[:], i0r, i1r)
        e.tensor_mul(t2[:], i0i, i1i)
        e.tensor_sub(outr, t1[:], t2[:])
        e.tensor_mul(t1[:], i0r, i1i)
        e.tensor_mul(t2[:], i0i, i1r)
        e.tensor_add(outi, t1[:], t2[:])

    def sel_mm(sel_sb, msrc, m, w):
        """Pack = SEL.T @ master -> PSUM [m, w]."""
        ps = psum_pool.tile([m, w], FP32, tag="ps")
        for c0 in range(0, w, 512):
            c1 = min(c0 + 512, w)
            nc.tensor.matmul(ps[:, c0:c1], sel_sb[:], msrc[:, c0:c1],
                             start=True, stop=True)
        return ps

    # ---- stage 0: A loads, widened row-sum matmuls -> master rows 0..21
    cpk = pers.tile([CROWS, CW], FP32, tag="cpack")
    dma(cpk[:, 0:128], tens["CPACK"][:, 0:128])
    cpkr = pers.tile([CROWS, CRW], FP32R, tag="cpackr")
    nc.scalar.dma_start(cpkr[:], tens["CPACKR"][:, :])

    def crslice(name):
        off, width = CPACKR_COLS[name]
        return cpkr[0:48, off:off + width]
    lts = pers.tile([22, 296], FP32, tag="lts")
    dma(lts[:], tens["LTS"][:, :])
    dma(cpk[:, 128:512], tens["CPACK"][:, 128:512])
    nc.scalar.dma_start(cpk[:, 512:1024], tens["CPACK"][:, 512:1024])
    dma2(cpk[:, 1024:CW], tens["CPACK"][:, 1024:CW])

    warm = psum_pool.tile([128, 64], FP32, tag="ps")
    for _ in range(8):
        nc.tensor.matmul(warm[:], cpk[0:48, 0:128], cpk[0:48, 0:64],
                         start=True, stop=True)

    def cslice(name, nrows=None):
        off, width = CPACK_COLS[name]
        nr = M_PAD if nrows is None else nrows
        return cpk[0:nr, off:off + width]

    lhsT_rs = {}
    lhsT_pp = {}
    KRS = {"H": 22, "G": 16}
    for si, side in enumerate("HG"):
        for ci, nm in enumerate("ri"):
            base = 74 * (2 * si + ci)
            K_rs = KRS[side]
            lhsT_rs[(side, nm)] = lts[0:K_rs, base:base + 32]
            lhsT_pp[(side, nm)] = lts[0:K_rs, base + 32:base + 74]

    mask_sb = {"H": cslice("MFX", 22), "G": cslice("MPX", 16)}
    sel_sb = {}
    for spec in (HSPEC, GSPEC):
        for s in ("SELL1", "SELL2A", "SELL2B"):
            key = f"{s}_{spec.name}"
            sel_sb[key] = cslice(key)
        for s in ("REPA", "REPB"):
            key = f"{s}_{spec.name}"
            sel_sb[key] = cslice(key, 48)

    # H masters: one [M_PAD, F] tile per component; G master: [M_PAD, 2P]
    # with real in cols 0:P, imag in P:2P. All 32 rows get written (22 by
    # the RS copy, 10 by the pair stage) -- no memset needed.
    masterH = {}
    for nm in "ri":
        t = pers.tile([M_PAD, F], FP32, tag=f"mstH{nm}", name=f"mstH{nm}")
        masterH[nm] = t
    masterG = pers.tile([M_PAD, 2 * P], FP32, tag="mstG")

    # ---- stage 1: pair products -> master rows 32..41 (packs come
    # straight from the rearranged ltp lhsTs -- no master dependency)
    P1T = F - FS_P1
    psH1 = {}
    for nm in "ri":
        ps = sel_mm(lhsT_pp[("H", nm)], mask_sb["H"], 42, F)
        sb = pk.tile([10, F], FP32, tag=f"halfH{nm}")
        if nm == "r":
            nc.scalar.copy(sb[:], ps[32:42, :])
        else:
            nc.vector.tensor_copy(sb[:], ps[32:42, :])
        p0t = pk.tile([10, P1T], FP32, tag=f"p0tH{nm}")
        nc.scalar.copy(p0t[:], ps[0:10, FS_P1:F])
        psH1[nm] = (ps, sb, p0t)
    cmul6(10, FS_P1,
          (psH1["r"][0][0:10, 0:FS_P1], psH1["i"][0][0:10, 0:FS_P1]),
          (psH1["r"][1][:, 0:FS_P1], psH1["i"][1][:, 0:FS_P1]),
          masterH["r"][M_PP:M_PP + 10, 0:FS_P1],
          masterH["i"][M_PP:M_PP + 10, 0:FS_P1])
    cmul6p(10, P1T,
           (psH1["r"][2][:], psH1["i"][2][:]),
           (psH1["r"][1][:, FS_P1:F], psH1["i"][1][:, FS_P1:F]),
           masterH["r"][M_PP:M_PP + 10, FS_P1:F],
           masterH["i"][M_PP:M_PP + 10, FS_P1:F])

    psG1 = psum_pool.tile([42, 2 * P], FP32, tag="ps")
    nc.tensor.matmul(psG1[:, 0:P], lhsT_pp[("G", "r")], mask_sb["G"][:],
                     start=True, stop=True)
    nc.tensor.matmul(psG1[:, P:2 * P], lhsT_pp[("G", "i")], mask_sb["G"][:],
                     start=True, stop=True)
    sbG1a = pk.tile([10, 2 * P], FP32, tag="selpGa")
    nc.scalar.copy(sbG1a[:], psG1[0:10, :])
    sbG1b = pk.tile([10, 2 * P], FP32, tag="selpGb")
    nc.scalar.copy(sbG1b[:], psG1[32:42, :])
    cmul6p(10, P,
           (sbG1a[:, 0:P], sbG1a[:, P:2 * P]),
           (sbG1b[:, 0:P], sbG1b[:, P:2 * P]),
           masterG[M_PP:M_PP + 10, 0:P], masterG[M_PP:M_PP + 10, P:2 * P])

    # row-sum masters (needed from L1 onward; emitted after the pair
    # stage so its PSUM evacuations win the ACT queue early)
    for nm in "ri":
        lt = lhsT_rs[("H", nm)]
        ps = psum_pool.tile([32, F], FP32, tag="ps")
        for c0 in range(0, F, 512):
            c1 = min(c0 + 512, F)
            nc.tensor.matmul(ps[:, c0:c1], lt, mask_sb["H"][:, c0:c1],
                             start=True, stop=True)
        nc.scalar.copy(masterH[nm][0:32, :], ps[:])
    psG = psum_pool.tile([32, 2 * P], FP32, tag="ps")
    nc.tensor.matmul(psG[:, 0:P], lhsT_rs[("G", "r")], mask_sb["G"][:],
                     start=True, stop=True)
    nc.tensor.matmul(psG[:, P:2 * P], lhsT_rs[("G", "i")], mask_sb["G"][:],
                     start=True, stop=True)
    nc.scalar.copy(masterG[0:32, :], psG[:])


    # ---- stage 2 (L1): tmp48[16g + c2] = tab_a[za] * tab_b[zb]
    t48H = {}
    l1t = {}
    for nm in "ri":
        ps = sel_mm(sel_sb["SELL1_H"], masterH[nm][:], 112, F)
        sb1 = pk.tile([48, F], FP32, tag=f"l1hH{nm}")
        if nm == "r":
            nc.scalar.copy(sb1[:], ps[64:112, :])
        else:
            nc.vector.tensor_copy(sb1[:], ps[64:112, :])
        lt1 = pk.tile([48, P1T], FP32, tag=f"l1tH{nm}")
        nc.scalar.copy(lt1[:], ps[0:48, FS_P1:F])
        l1t[nm] = lt1
        t48 = pers.tile([48, F], FP32R, tag=f"t48H{nm}", name=f"t48H{nm}")
        t48H[nm] = (ps, sb1, t48)
    cmul6(48, FS_P1,
          (t48H["r"][0][0:48, 0:FS_P1], t48H["i"][0][0:48, 0:FS_P1]),
          (t48H["r"][1][:, 0:FS_P1], t48H["i"][1][:, 0:FS_P1]),
          t48H["r"][2][:, 0:FS_P1], t48H["i"][2][:, 0:FS_P1])
    cmul6p(48, P1T,
           (l1t["r"][:], l1t["i"][:]),
           (t48H["r"][1][:, FS_P1:F], t48H["i"][1][:, FS_P1:F]),
           t48H["r"][2][:, FS_P1:F], t48H["i"][2][:, FS_P1:F])

    psL1G = sel_mm(sel_sb["SELL1_G"], masterG[:], 112, 2 * P)
    sbL1Ga = pk.tile([48, 2 * P], FP32, tag="l1Ga")
    nc.scalar.copy(sbL1Ga[:], psL1G[0:48, :])
    sbL1Gb = pk.tile([48, 2 * P], FP32, tag="l1Gb")
    nc.scalar.copy(sbL1Gb[:], psL1G[64:112, :])
    t48G = pers.tile([48, 2 * P], FP32R, tag="t48G", name="t48G")
    cmul6p(48, P,
           (sbL1Ga[:, 0:P], sbL1Ga[:, P:2 * P]),
           (sbL1Gb[:, 0:P], sbL1Gb[:, P:2 * P]),
           t48G[:, 0:P], t48G[:, P:2 * P])

    # ---- stage 3 (L2): e_g = tmp * tab_c[zc], column-split DVE / GPSIMD
    # H g0+g1 fused as one [128, F] set -> eRH = [H0r; H1r], eIH = [H0i; H1i].
    # The T matmuls compensate with K=64 accumulating pairs.
    TL = F - FS_L2
    c01H = {}
    repH = {}
    reptH = {}
    for nm in "ri":
        c01 = sel_mm(sel_sb["SELL2A_H"], masterH[nm][:], 128, F)
        c01sb = pk.tile([128, F], FP32, tag=f"c01H{nm}")
        if nm == "r":
            nc.scalar.copy(c01sb[:], c01[:])
        else:
            nc.vector.tensor_copy(c01sb[:], c01[:])
        c01H[nm] = c01sb
        rep = psum_pool.tile([128, F], FP32, tag="ps")
        for c0 in range(0, F, 512):
            mmr(rep[:, c0:c0 + 512], crslice("REPA_H"),
                t48H[nm][2][:, c0:c0 + 512])
        repH[nm] = rep
        rt = pk.tile([128, TL], FP32, tag=f"reptH{nm}")
        nc.scalar.copy(rt[:], repH[nm][:, FS_L2:F])
        reptH[nm] = rt
    eRH = pers.tile([128, F], FP32R, tag="eRH", name="eRH")
    eIH = pers.tile([128, F], FP32R, tag="eIH", name="eIH")
    cmul6(128, FS_L2,
          (repH["r"][0:128, 0:FS_L2], repH["i"][0:128, 0:FS_L2]),
          (c01H["r"][:, 0:FS_L2], c01H["i"][:, 0:FS_L2]),
          eRH[:, 0:FS_L2], eIH[:, 0:FS_L2])
    cmul6p(128, TL,
           (reptH["r"][:], reptH["i"][:]),
           (c01H["r"][:, FS_L2:F], c01H["i"][:, FS_L2:F]),
           eRH[:, FS_L2:F], eIH[:, FS_L2:F])

    cp2H = {}
    rbH = {}
    rbtH = {}
    for nm in "ri":
        c2 = sel_mm(sel_sb["SELL2B_H"], masterH[nm][:], 64, F)
        c2sb = pk.tile([64, F], FP32, tag=f"c2H{nm}")
        nc.scalar.copy(c2sb[:], c2[:])
        cp2H[nm] = c2sb
        rb = psum_pool.tile([64, F], FP32, tag="ps")
        for c0 in range(0, F, 512):
            mmr(rb[:, c0:c0 + 512], crslice("REPB_H"),
                t48H[nm][2][:, c0:c0 + 512])
        rbH[nm] = rb
        rbt = pk.tile([64, TL], FP32, tag=f"rbtH{nm}")
        nc.scalar.copy(rbt[:], rb[:, FS_L2:F])
        rbtH[nm] = rbt
    eR2H = pers.tile([64, F], FP32R, tag="eR2H", name="eR2H")
    eI2H = pers.tile([64, F], FP32R, tag="eI2H", name="eI2H")
    cmul6(64, FS_L2,
          (rbH["r"][:, 0:FS_L2], rbH["i"][:, 0:FS_L2]),
          (cp2H["r"][:, 0:FS_L2], cp2H["i"][:, 0:FS_L2]),
          eR2H[:, 0:FS_L2], eI2H[:, 0:FS_L2])
    cmul6p(64, TL,
           (rbtH["r"][:], rbtH["i"][:]),
           (cp2H["r"][:, FS_L2:F], cp2H["i"][:, FS_L2:F]),
           eR2H[:, FS_L2:F], eI2H[:, FS_L2:F])

    # G side (GPSIMD, SBUF operands via single ACT evacuations)
    repG = psum_pool.tile([128, 2 * P], FP32, tag="ps")
    mmr(repG[:], crslice("REPA_G"), t48G[:])
    repGsb = pk.tile([128, 2 * P], FP32, tag="repG")
    nc.scalar.copy(repGsb[:], repG[:])
    c01G = sel_mm(sel_sb["SELL2A_G"], masterG[:], 128, 2 * P)
    c01Gsb = pk.tile([128, 2 * P], FP32, tag="c01G")
    nc.scalar.copy(c01Gsb[:], c01G[:])
    eG01 = pers.tile([128, 2 * P], FP32R, tag="eG01", name="eG01")
    cmul6p(128, P,
           (repGsb[:, 0:P], repGsb[:, P:2 * P]),
           (c01Gsb[:, 0:P], c01Gsb[:, P:2 * P]),
           eG01[:, 0:P], eG01[:, P:2 * P])

    rbG = psum_pool.tile([64, 2 * P], FP32, tag="ps")
    mmr(rbG[:], crslice("REPB_G"), t48G[:])
    rbGsb = pk.tile([64, 2 * P], FP32, tag="rbG")
    nc.scalar.copy(rbGsb[:], rbG[:])
    c2G = sel_mm(sel_sb["SELL2B_G"], masterG[:], 64, 2 * P)
    c2Gsb = pk.tile([64, 2 * P], FP32, tag="c2G")
    nc.scalar.copy(c2Gsb[:], c2G[:])
    eG2 = pers.tile([64, 2 * P], FP32R, tag="eG2", name="eG2")
    cmul6p(64, P,
           (rbGsb[:, 0:P], rbGsb[:, P:2 * P]),
           (c2Gsb[:, 0:P], c2Gsb[:, P:2 * P]),
           eG2[:, 0:P], eG2[:, P:2 * P])

    # negated imag halves (lhsT for the real-part T matmuls)
    negG01 = pers.tile([128, P], FP32R, tag="negG01")
    nc.scalar.mul(negG01[:], eG01[:, P:2 * P], -1.0)
    negG2 = pers.tile([64, P], FP32R, tag="negG2")
    nc.scalar.mul(negG2[:], eG2[:, P:2 * P], -1.0)

    # ---- stage 4: T matmuls -- per (group, comp, chunk) a K=64 pair
    # accumulated in PSUM: Tr = Gr^T Hr + (-Gi)^T Hi ; Ti = Gr^T Hi + Gi^T Hr
    # Order: T0, T1 (combine inputs) first, then T2 (only needed by the
    # final reduction) so p01 overlaps the T2 matmuls.
    def t_mms(g):
        if g < 2:
            Gr = eG01[64 * g:64 * g + 64, 0:P]
            Gi = eG01[64 * g:64 * g + 64, P:2 * P]
            Gin = negG01[64 * g:64 * g + 64, :]
            Hr = eRH[64 * g:64 * g + 64, :]
            Hi = eIH[64 * g:64 * g + 64, :]
        else:
            Gr = eG2[:, 0:P]
            Gi = eG2[:, P:2 * P]
            Gin = negG2[:]
            Hr = eR2H[:]
            Hi = eI2H[:]
        tr = psum_pool.tile([P, F], FP32, tag="ps")
        ti = psum_pool.tile([P, F], FP32, tag="ps")
        for c0 in range(0, F, 512):
            c1 = c0 + 512
            mmr(tr[:, c0:c1], Gr, Hr[:, c0:c1], start=True, stop=False)
            mmr(tr[:, c0:c1], Gin, Hi[:, c0:c1], start=False, stop=True)
            mmr(ti[:, c0:c1], Gr, Hi[:, c0:c1], start=True, stop=False)
            mmr(ti[:, c0:c1], Gi, Hr[:, c0:c1], start=False, stop=True)
        return tr, ti

    t0r, t0i = t_mms(0)
    t1r_ps, t1i_ps = t_mms(1)
    t1r = pers.tile([P, F], FP32, tag="T1r")
    t1i = pers.tile([P, F], FP32, tag="T1i")
    for c0 in range(0, F, 512):
        c1 = c0 + 512
        nc.scalar.copy(t1r[:, c0:c1], t1r_ps[:, c0:c1])
        nc.vector.tensor_copy(t1i[:, c0:c1], t1i_ps[:, c0:c1])

    # p01 = T0*T1: col-split DVE (T0 from PSUM) / GPSIMD (T0 tail via ACT)
    TP = F - FS_PF
    p01r = pers.tile([P, F], FP32, tag="p01r")
    p01i = pers.tile([P, F], FP32, tag="p01i")
    t0tr = pers.tile([P, TP], FP32, tag="t0tr")
    t0ti = pers.tile([P, TP], FP32, tag="t0ti")
    nc.scalar.copy(t0tr[:], t0r[:, FS_PF:F])
    nc.scalar.copy(t0ti[:], t0i[:, FS_PF:F])
    cmul6(P, FS_PF,
          (t0r[:, 0:FS_PF], t0i[:, 0:FS_PF]),
          (t1r[:, 0:FS_PF], t1i[:, 0:FS_PF]),
          p01r[:, 0:FS_PF], p01i[:, 0:FS_PF])
    cmul6p(P, TP,
           (t0tr[:], t0ti[:]),
           (t1r[:, FS_PF:F], t1i[:, FS_PF:F]),
           p01r[:, FS_PF:F], p01i[:, FS_PF:F])

    t2r, t2i = t_mms(2)

    # ---- final reduction: acc[p, k] = sum_f p01 * T2 products (DVE,
    # full width, T2 straight from PSUM).
    # Host combines: perm_r = c0 - c1, perm_i = c2 + c3
    scr2 = pers.tile([P, F], FP32, tag="ttr_scr")
    accD = pers.tile([P, 4], FP32, tag="accD")
    pairs = [(p01r, t2r), (p01i, t2i), (p01r, t2i), (p01i, t2r)]
    for k, (a, b) in enumerate(pairs):
        nc.vector.scalar_tensor_tensor(
            out=scr2[:], in0=b[:], scalar=1.0, in1=a[:],
            op0=OP.mult, op1=OP.mult, accum_out=accD[:, k:k + 1])

    dma(tens["OUT"][:, 0:4], accD[:])

    ctx.close()


# ---------------------------------------------------------------- entry point
def kernel(A_real: np.ndarray, A_imag: np.ndarray) -> np.ndarray:
    B = A_real.shape[0]
    assert B == 8 and A_real.shape == (B, N, N)
    if "nc" not in _CACHE:
        _CACHE["nc"] = build_kernel()
    nc = _CACHE["nc"]
    in_maps = [make_in_map(A_real[b], A_imag[b]) for b in range(B)]
    res = run_bass_kernel_spmd(nc, in_maps, list(range(B)))
    out = np.empty(B, dtype=np.float32)
    for b in range(B):
        acc = res.results[b]["OUT"].reshape(128, 4).astype(np.float64)
        s = acc.sum(axis=0)
        pr = s[0] - s[1]
        pi = s[2] + s[3]
        pa2 = np.float32(pr) ** 2 + np.float32(pi) ** 2
        out[b] = np.float32(EMU * SCALE2 * pa2 + DARK)
    return out


if __name__ == "__main__":
    A_real = np.load("/tmp/A_real.npy")
    A_imag = np.load("/tmp/A_imag.npy")
    print(kernel(A_real, A_imag))


# revision 39
# speedup vs baseline: 2.9393x; 1.0237x over previous
"""Trainium2 Bass kernel: boson-sampler probabilities via Glynn's permanent formula.

Math (per 18x18 complex matrix A):
  perm(A) = 2^(1-n) * sum_{d in {+-1}^n, d_0=+1} (prod_k d_k) * prod_i (sum_j d_j A[i,j])
The 2^17 sign vectors form a [128 x 1024] grid (7 "p" bits drive columns 11..17,
10 "f" bits drive columns 1..10; column 0 fixed +1). Row-sums factor as
rs_i = RP_i(p) + RF_i(f); rows are grouped [6,6,6] and each group's product
expands as T_g[p,f] = sum_{c<64} G_g[c,p] * H_g[c,f] -- fp32r matmuls on the
tensor engine (full rate at free>=256). The 64-row G/H tables (all sub-products
of 6 rows) are built hierarchically (pairs -> quads -> tables): packed operand
sets are assembled from SBUF "master" row tiles by 0/1 selection matmuls, and
each level is a set of elementwise complex-multiply ops column-split across
the DVE and GPSIMD engines. Glynn parity signs are folded into group 2's
tables via sign-scaled mask constants. G-side (p-axis) work packs real|imag
side by side in one [32, 256] master so every G matmul runs at free=256.
The final sum(T0*T1*T2) reduces via 8 scalar_tensor_tensor accumulations
(col-split DVE/Pool); |perm|^2, the (underflowed-to-zero) classical term and
the dark-count offset are applied on the host. One NeuronCore per batch
element.
"""

import sys

sys.path.insert(0, "/opt/trn_rl_repo")

import numpy as np

import concourse.bacc as bacc
import concourse.bass as bass
import concourse.tile as tile
from concourse import mybir
from concourse.bass_utils import run_bass_kernel_spmd

FP32 = mybir.dt.float32
FP32R = mybir.dt.float32r
OP = mybir.AluOpType

N = 18
PBITS, FBITS = 7, 10
P, F = 1 << PBITS, 1 << FBITS          # 128, 1024
EMU = 0.85 * (1 - 0.02) * (1 - 0.02) * (1 - 0.01)
DARK = 1e-6 * N
SCALE2 = float(2.0 ** (2 * (1 - N)))

# master row map (same for both sides; imag comp has zeros at ONES/SIGN)
M_RF = 0          # rows 0..17: row-sums RF_i / RP_i
M_SE = 18         # sign-scaled even row of pair 8 (RFe' / RPe')
M_SO = 19         # sign-scaled odd row (RFo' / RPo')
M_ONE = 20        # ones (real) / zeros (imag)
M_SGN = 21        # sign row sF / sP (real) / zeros (imag)
M_PP = 32         # rows 32..41: pair products PP_q (row 41 = PP' scaled)
M_PPS = 41        # row 41: PP' = sign-scaled pair-8 product (32-aligned for GPSIMD)
M_PAD = 42
CROWS = 48        # const-pack row count (REPA/REPB lhsTs span 48 rows)

# column split: DVE takes [0:x], GPSIMD takes [x:F] of each wide stage
FS_P1 = 704       # pair / L1 stages
FS_L2 = 576       # L2 table-build stages
FS_PF = 576       # p01 / final reduction stages

_CACHE = {}


def _pm_mask(nvals, bits):
    v = np.arange(nvals, dtype=np.uint32)
    m = (v[:, None] >> np.arange(bits, dtype=np.uint32)[None, :]) & 1
    return (1.0 - 2.0 * m).astype(np.float32).T.copy()   # [bits, nvals]


def _parity(nvals, bits):
    v = np.arange(nvals, dtype=np.uint32)
    pc = np.zeros(nvals, dtype=np.uint32)
    for k in range(bits):
        pc += (v >> k) & 1
    return np.where(pc % 2 == 0, 1.0, -1.0).astype(np.float32)


class SideSpec:
    """H: f-side (width 1024, pair-table identity at z=3);
       G: p-side (width 128, identity at z=0)."""

    def __init__(self, name, width, mult_z):
        self.name = name
        self.w = width
        self.mult_z = list(mult_z)
        self.idz = ({0, 1, 2, 3} - set(mult_z)).pop()

    # pair-table entry -> master row (pair q, entry z); identity z -> ones row
    def tab_row(self, q, z):
        if z == self.idz:
            return M_ONE
        if self.name == "H":
            return {0: M_PP + q, 1: 2 * q + 1, 2: 2 * q}[z]
        return {1: 2 * q, 2: 2 * q + 1, 3: M_PP + q}[z]

    # group-2 scaled pair-table entry (pair 8) -> master row
    def tab2_row(self, z):
        if z == self.idz:
            return M_SGN
        if self.name == "H":
            return {0: M_PPS, 1: M_SO, 2: M_SE}[z]
        return {1: M_SE, 2: M_SO, 3: M_PPS}[z]


HSPEC = SideSpec("H", F, (0, 1, 2))
GSPEC = SideSpec("G", P, (1, 2, 3))


def _sel(rows, m_pad=None):
    """Selection matrix [M_PAD, len(rows)] with one 1 per used column."""
    M = len(rows) if m_pad is None else m_pad
    s = np.zeros((M_PAD, M), np.float32)
    for m, k in enumerate(rows):
        if k is not None:
            s[k, m] = 1.0
    return s


# const pack column layout: computed once at import
def _build_pack():
    cols = {}
    blocks = []
    off = 0

    def add(name, arr):
        nonlocal off
        a = np.zeros((CROWS, arr.shape[1]), np.float32)
        a[0:arr.shape[0], :] = arr
        cols[name] = (off, arr.shape[1])
        blocks.append(a)
        off += arr.shape[1]

    # MFX: [ones;pm(10) | (ones;pm)*sF] (row 0 = ones source, row 11 = sF source)
    mF = np.concatenate([np.ones((1, F), np.float32), _pm_mask(F, FBITS)], axis=0)
    sF = _parity(F, FBITS)
    add("MFX", np.concatenate([mF, mF * sF[None, :]], axis=0))           # [22, F]
    # MPX: [pm(7) | pm*sP | ones | sP]
    mP = _pm_mask(P, PBITS)
    sP = _parity(P, PBITS)
    add("MPX", np.concatenate(
        [mP, mP * sP[None, :], np.ones((1, P), np.float32), sP[None, :]], axis=0))
    # unit columns for the widened RS matmul lhsT (cols 20,21), per side
    uh = np.zeros((22, 2), np.float32)
    uh[0, 0] = 1.0      # -> MFX row 0 (ones)
    uh[11, 1] = 1.0     # -> MFX row 11 (sF)
    ug = np.zeros((16, 2), np.float32)
    ug[14, 0] = 1.0     # -> MPX row 14 (ones)
    ug[15, 1] = 1.0     # -> MPX row 15 (sP)
    cols["_U"] = (uh, ug)

    def digits(c):
        return c % 4, (c // 4) % 4, c // 16          # za, zb, zc

    for spec in (HSPEC, GSPEC):
        nm = spec.name
        # L1: 48 distinct products tmp48 indexed by (g, c2), c2 = za + 4zb:
        # in0/in1 packed [48 | pad | 48] in one mm
        in0 = []
        in1 = []
        for g in range(3):
            for c2 in range(16):
                in0.append(spec.tab_row(3 * g, c2 % 4))
                in1.append(spec.tab_row(3 * g + 1, c2 // 4))
        add(f"SELL1_{nm}", _sel(in0 + [None] * 16 + in1, 112))
        # REP: replicate tmp48 rows into L2 src layout (lhsT for rep matmuls)
        # repA: [64g + c] <- tmp48[16g + (c % 16)] for g=0,1 ; repB: g=2
        repA = np.zeros((48, 128), np.float32)
        for g in range(2):
            for c in range(64):
                repA[16 * g + (c % 16), 64 * g + c] = 1.0
        add(f"REPA_{nm}", repA)
        repB = np.zeros((48, 64), np.float32)
        for c in range(64):
            repB[32 + (c % 16), c] = 1.0
        add(f"REPB_{nm}", repB)
        # L2 c-packs: set1 rows [64g+c] = tab_c[zc]; set2 = scaled tab2'[zc]
        rows = []
        for g in range(2):
            for c in range(64):
                rows.append(spec.tab_row(3 * g + 2, digits(c)[2]))
        add(f"SELL2A_{nm}", _sel(rows))
        rows = [spec.tab2_row(digits(c)[2]) for c in range(64)]
        add(f"SELL2B_{nm}", _sel(rows))
    return np.concatenate(blocks, axis=1), cols


CPACK, CPACK_COLS = _build_pack()
U_HG = CPACK_COLS.pop("_U")
CW = CPACK.shape[1]

def _build_packr():
    parts = []
    offs = {}
    off = 0
    for nm in ("REPA_H", "REPB_H", "REPA_G", "REPB_G"):
        o, w = CPACK_COLS[nm]
        parts.append(CPACK[0:CROWS, o:o + w])
        offs[nm] = (off, w)
        off += w
    return np.ascontiguousarray(np.concatenate(parts, axis=1)), offs


CPACKR, CPACKR_COLS = _build_packr()
CRW = CPACKR.shape[1]


def build_lts(Ar, Ai):
    """Host-side lhsT pack: per (side, comp) the widened row-sum lhsT
    (32 cols) and the pair-pack lhsT (42 cols), one [22, 296] tensor."""
    lts = np.zeros((22, 296), np.float32)
    for si, (lo, kb, K) in enumerate(((0, 11, 22), (11, 7, 16))):
        for ci, A in ((0, Ar), (1, Ai)):
            base = 74 * (2 * si + ci)
            AT = np.ascontiguousarray(A.T, dtype=np.float32)
            lts[0:kb, base:base + 18] = AT[lo:lo + kb, 0:18]
            lts[kb:2 * kb, base + 18:base + 20] = AT[lo:lo + kb, 16:18]
            if ci == 0:
                lts[0:K, base + 20:base + 22] = U_HG[si]
            pb = base + 32
            lts[0:kb, pb:pb + 9] = AT[lo:lo + kb, 0:17:2]
            lts[kb:2 * kb, pb + 9] = AT[lo:lo + kb, 16]
            lts[0:kb, pb + 32:pb + 41] = AT[lo:lo + kb, 1:18:2]
            lts[0:kb, pb + 41] = AT[lo:lo + kb, 17]
    return lts


def make_in_map(Ar, Ai):
    return {"CPACK": CPACK, "CPACKR": CPACKR, "LTS": build_lts(Ar, Ai)}


def host_consts():
    return {"CPACK": CPACK}


# ---------------------------------------------------------------- kernel body
def build_kernel(loop_iters=None):
    nc = bacc.Bacc("TRN2", target_bir_lowering=False, debug=False)

    tens = {}
    tens["LTS"] = nc.dram_tensor("LTS", [22, 296], FP32, kind="ExternalInput").ap()
    tens["CPACK"] = nc.dram_tensor("CPACK", [CROWS, CW], FP32,
                                   kind="ExternalInput").ap()
    tens["CPACKR"] = nc.dram_tensor("CPACKR", [CROWS, CRW], FP32R,
                                    kind="ExternalInput").ap()
    tens["OUT"] = nc.dram_tensor("OUT", [128, 4], FP32, kind="ExternalOutput").ap()

    with tile.TileContext(nc) as tc:
        if loop_iters is None:
            _body(nc, tc, tens)
        else:
            with tc.For_i(0, loop_iters, 1):
                _body(nc, tc, tens)
    nc.compile()
    return nc


def _body(nc, tc, tens):
    from contextlib import ExitStack

    ctx = ExitStack()
    pers = ctx.enter_context(tc.tile_pool(name="pers", bufs=1))
    pk = ctx.enter_context(tc.tile_pool(name="pk", bufs=2))
    cm = ctx.enter_context(tc.tile_pool(name="cm", bufs=2))
    psum_pool = ctx.enter_context(tc.tile_pool(name="psum", bufs=4, space="PSUM"))
    dma = nc.sync.dma_start
    dma2 = nc.gpsimd.dma_start          # SWDGE queues for small input loads

    def mmr(out_ap, lhsT_ap, rhs_ap, start=True, stop=True):
        """fp32r matmul: full rate (1 cyc/row) on trn2 when free >= 256."""
        nc.tensor.matmul(out_ap, lhsT_ap.bitcast(FP32R), rhs_ap.bitcast(FP32R),
                         start=start, stop=stop)

    def cmul6(rows, w, i0, i1, outr, outi):
        """DVE complex multiply: (i0r,i0i)*(i1r,i1i) -> (outr,outi)."""
        e = nc.vector
        i0r, i0i = i0
        i1r, i1i = i1
        t1 = cm.tile([rows, w], FP32, tag="cm_t1")
        t2 = cm.tile([rows, w], FP32, tag="cm_t2")
        e.tensor_mul(t1[:], i0r, i1r)
        e.tensor_mul(t2[:], i0i, i1i)
        e.tensor_sub(outr, t1[:], t2[:])
        e.tensor_mul(t1[:], i0r, i1i)
        e.tensor_mul(t2[:], i0i, i1r)
        e.tensor_add(outi, t1[:], t2[:])

    def cmul6p(rows, w, i0, i1, outr, outi):
        """GPSIMD complex multiply (plain tensor-tensor ops)."""
        e = nc.gpsimd
        i0r, i0i = i0
        i1r, i1i = i1
        t1 = cm.tile([rows, w], FP32, tag="gp_t1")
        t2 = cm.tile([rows, w], FP32, tag="gp_t2")
        e.tensor_mul(t1[:], i0r, i1r)
        e.tensor_mul(t2[:], i0i, i1i)
        e.tensor_sub(outr, t1[:], t2[:])
        e.tensor_mul(t1[:], i0r, i1i)
        e.tensor_mul(t2[:], i0i, i1r)
        e.tensor_add(outi, t1[:], t2[:])

    def sel_mm(sel_sb, msrc, m, w):
        """Pack = SEL.T @ master -> PSUM [m, w]."""
        ps = psum_pool.tile([m, w], FP32, tag="ps")
        for c0 in range(0, w, 512):
            c1 = min(c0 + 512, w)
            nc.tensor.matmul(ps[:, c0:c1], sel_sb[:], msrc[:, c0:c1],
                             start=True, stop=True)
        return ps

    # ---- stage 0: A loads, widened row-sum matmuls -> master rows 0..21
    cpk = pers.tile([CROWS, CW], FP32, tag="cpack")
    dma(cpk[:, 0:128], tens["CPACK"][:, 0:128])
    cpkr = pers.tile([CROWS, CRW], FP32R, tag="cpackr")
    nc.scalar.dma_start(cpkr[:], tens["CPACKR"][:, :])

    def crslice(name):
        off, width = CPACKR_COLS[name]
        return cpkr[0:48, off:off + width]
    lts = pers.tile([22, 296], FP32, tag="lts")
    dma(lts[:], tens["LTS"][:, :])
    dma(cpk[:, 128:512], tens["CPACK"][:, 128:512])
    nc.scalar.dma_start(cpk[:, 512:1024], tens["CPACK"][:, 512:1024])
    dma2(cpk[:, 1024:CW], tens["CPACK"][:, 1024:CW])

    warm = psum_pool.tile([128, 64], FP32, tag="ps")
    for _ in range(8):
        nc.tensor.matmul(warm[:], cpk[0:48, 0:128], cpk[0:48, 0:64],
                         start=True, stop=True)

    def cslice(name, nrows=None):
        off, width = CPACK_COLS[name]
        nr = M_PAD if nrows is None else nrows
        return cpk[0:nr, off:off + width]

    lhsT_rs = {}
    lhsT_pp = {}
    KRS = {"H": 22, "G": 16}
    for si, side in enumerate("HG"):
        for ci, nm in enumerate("ri"):
            base = 74 * (2 * si + ci)
            K_rs = KRS[side]
            lhsT_rs[(side, nm)] = lts[0:K_rs, base:base + 32]
            lhsT_pp[(side, nm)] = lts[0:K_rs, base + 32:base + 74]

    mask_sb = {"H": cslice("MFX", 22), "G": cslice("MPX", 16)}
    sel_sb = {}
    for spec in (HSPEC, GSPEC):
        for s in ("SELL1", "SELL2A", "SELL2B"):
            key = f"{s}_{spec.name}"
            sel_sb[key] = cslice(key)
        for s in ("REPA", "REPB"):
            key = f"{s}_{spec.name}"
            sel_sb[key] = cslice(key, 48)

    # H masters: one [M_PAD, F] tile per component; G master: [M_PAD, 2P]
    # with real in cols 0:P, imag in P:2P. All 32 rows get written (22 by
    # the RS copy, 10 by the pair stage) -- no memset needed.
    masterH = {}
    for nm in "ri":
        t = pers.tile([M_PAD, F], FP32, tag=f"mstH{nm}", name=f"mstH{nm}")
        masterH[nm] = t
    masterG = pers.tile([M_PAD, 2 * P], FP32, tag="mstG")

    # ---- stage 1: pair products -> master rows 32..41 (packs come
    # straight from the rearranged ltp lhsTs -- no master dependency)
    P1T = F - FS_P1
    psH1 = {}
    for nm in "ri":
        ps = sel_mm(lhsT_pp[("H", nm)], mask_sb["H"], 42, F)
        sb = pk.tile([10, F], FP32, tag=f"halfH{nm}")
        if nm == "r":
            nc.scalar.copy(sb[:], ps[32:42, :])
        else:
            nc.vector.tensor_copy(sb[:], ps[32:42, :])
        p0t = pk.tile([10, P1T], FP32, tag=f"p0tH{nm}")
        nc.scalar.copy(p0t[:], ps[0:10, FS_P1:F])
        psH1[nm] = (ps, sb, p0t)
    cmul6(10, FS_P1,
          (psH1["r"][0][0:10, 0:FS_P1], psH1["i"][0][0:10, 0:FS_P1]),
          (psH1["r"][1][:, 0:FS_P1], psH1["i"][1][:, 0:FS_P1]),
          masterH["r"][M_PP:M_PP + 10, 0:FS_P1],
          masterH["i"][M_PP:M_PP + 10, 0:FS_P1])
    cmul6p(10, P1T,
           (psH1["r"][2][:], psH1["i"][2][:]),
           (psH1["r"][1][:, FS_P1:F], psH1["i"][1][:, FS_P1:F]),
           masterH["r"][M_PP:M_PP + 10, FS_P1:F],
           masterH["i"][M_PP:M_PP + 10, FS_P1:F])

    psG1 = psum_pool.tile([42, 2 * P], FP32, tag="ps")
    nc.tensor.matmul(psG1[:, 0:P], lhsT_pp[("G", "r")], mask_sb["G"][:],
                     start=True, stop=True)
    nc.tensor.matmul(psG1[:, P:2 * P], lhsT_pp[("G", "i")], mask_sb["G"][:],
                     start=True, stop=True)
    sbG1a = pk.tile([10, 2 * P], FP32, tag="selpGa")
    nc.scalar.copy(sbG1a[:], psG1[0:10, :])
    sbG1b = pk.tile([10, 2 * P], FP32, tag="selpGb")
    nc.scalar.copy(sbG1b[:], psG1[32:42, :])
    cmul6p(10, P,
           (sbG1a[:, 0:P], sbG1a[:, P:2 * P]),
           (sbG1b[:, 0:P], sbG1b[:, P:2 * P]),
           masterG[M_PP:M_PP + 10, 0:P], masterG[M_PP:M_PP + 10, P:2 * P])

    # row-sum masters (needed from L1 onward; emitted after the pair
    # stage so its PSUM evacuations win the ACT queue early)
    for nm in "ri":
        lt = lhsT_rs[("H", nm)]
        ps = psum_pool.tile([32, F], FP32, tag="ps")
        for c0 in range(0, F, 512):
            c1 = min(c0 + 512, F)
            nc.tensor.matmul(ps[:, c0:c1], lt, mask_sb["H"][:, c0:c1],
                             start=True, stop=True)
        nc.scalar.copy(masterH[nm][0:32, :], ps[:])
    psG = psum_pool.tile([32, 2 * P], FP32, tag="ps")
    nc.tensor.matmul(psG[:, 0:P], lhsT_rs[("G", "r")], mask_sb["G"][:],
                     start=True, stop=True)
    nc.tensor.matmul(psG[:, P:2 * P], lhsT_rs[("G", "i")], mask_sb["G"][:],
                     start=True, stop=True)
    nc.scalar.copy(masterG[0:32, :], psG[:])


    # ---- stage 2 (L1): tmp48[16g + c2] = tab_a[za] * tab_b[zb]
    t48H = {}
    l1t = {}
    for nm in "ri":
        ps = sel_mm(sel_sb["SELL1_H"], masterH[nm][:], 112, F)
        sb1 = pk.tile([48, F], FP32, tag=f"l1hH{nm}")
        if nm == "r":
            nc.scalar.copy(sb1[:], ps[64:112, :])
        else:
            nc.vector.tensor_copy(sb1[:], ps[64:112, :])
        lt1 = pk.tile([48, P1T], FP32, tag=f"l1tH{nm}")
        nc.scalar.copy(lt1[:], ps[0:48, FS_P1:F])
        l1t[nm] = lt1
        t48 = pers.tile([48, F], FP32R, tag=f"t48H{nm}", name=f"t48H{nm}")
        t48H[nm] = (ps, sb1, t48)
    cmul6(48, FS_P1,
          (t48H["r"][0][0:48, 0:FS_P1], t48H["i"][0][0:48, 0:FS_P1]),
          (t48H["r"][1][:, 0:FS_P1], t48H["i"][1][:, 0:FS_P1]),
          t48H["r"][2][:, 0:FS_P1], t48H["i"][2][:, 0:FS_P1])
    cmul6p(48, P1T,
           (l1t["r"][:], l1t["i"][:]),
           (t48H["r"][1][:, FS_P1:F], t48H["i"][1][:, FS_P1:F]),
           t48H["r"][2][:, FS_P1:F], t48H["i"][2][:, FS_P1:F])

    psL1G = sel_mm(sel_sb["SELL1_G"], masterG[:], 112, 2 * P)
    sbL1Ga = pk.tile([48, 2 * P], FP32, tag="l1Ga")
    nc.scalar.copy(sbL1Ga[:], psL1G[0:48, :])
    sbL1Gb = pk.tile([48, 2 * P], FP32, tag="l1Gb")
    nc.scalar.copy(sbL1Gb[:], psL1G[64:112, :])
    t48G = pers.tile([48, 2 * P], FP32R, tag="t48G", name="t48G")
    cmul6p(48, P,
           (sbL1Ga[:, 0:P], sbL1Ga[:, P:2 * P]),
           (sbL1Gb[:, 0:P], sbL1Gb[:, P:2 * P]),
           t48G[:, 0:P], t48G[:, P:2 * P])

    # ---- stage 3 (L2): e_g = tmp * tab_c[zc], column-split DVE / GPSIMD
    # H g0+g1 fused as one [128, F] set -> eRH = [H0r; H1r], eIH = [H0i; H1i].
    # The T matmuls compensate with K=64 accumulating pairs.
    TL = F - FS_L2
    c01H = {}
    repH = {}
    reptH = {}
    for nm in "ri":
        c01 = sel_mm(sel_sb["SELL2A_H"], masterH[nm][:], 128, F)
        c01sb = pk.tile([128, F], FP32, tag=f"c01H{nm}")
        if nm == "r":
            nc.scalar.copy(c01sb[:], c01[:])
        else:
            nc.vector.tensor_copy(c01sb[:], c01[:])
        c01H[nm] = c01sb
        rep = psum_pool.tile([128, F], FP32, tag="ps")
        for c0 in range(0, F, 512):
            mmr(rep[:, c0:c0 + 512], crslice("REPA_H"),
                t48H[nm][2][:, c0:c0 + 512])
        repH[nm] = rep
        rt = pk.tile([128, TL], FP32, tag=f"reptH{nm}")
        nc.scalar.copy(rt[:], repH[nm][:, FS_L2:F])
        reptH[nm] = rt
    eRH = pers.tile([128, F], FP32R, tag="eRH", name="eRH")
    eIH = pers.tile([128, F], FP32R, tag="eIH", name="eIH")
    cmul6(128, FS_L2,
          (repH["r"][0:128, 0:FS_L2], repH["i"][0:128, 0:FS_L2]),
          (c01H["r"][:, 0:FS_L2], c01H["i"][:, 0:FS_L2]),
          eRH[:, 0:FS_L2], eIH[:, 0:FS_L2])
    cmul6p(128, TL,
           (reptH["r"][:], reptH["i"][:]),
           (c01H["r"][:, FS_L2:F], c01H["i"][:, FS_L2:F]),
           eRH[:, FS_L2:F], eIH[:, FS_L2:F])

    cp2H = {}
    rbH = {}
    rbtH = {}
    for nm in "ri":
        c2 = sel_mm(sel_sb["SELL2B_H"], masterH[nm][:], 64, F)
        c2sb = pk.tile([64, F], FP32, tag=f"c2H{nm}")
        nc.scalar.copy(c2sb[:], c2[:])
        cp2H[nm] = c2sb
        rb = psum_pool.tile([64, F], FP32, tag="ps")
        for c0 in range(0, F, 512):
            mmr(rb[:, c0:c0 + 512], crslice("REPB_H"),
                t48H[nm][2][:, c0:c0 + 512])
        rbH[nm] = rb
        rbt = pk.tile([64, TL], FP32, tag=f"rbtH{nm}")
        nc.scalar.copy(rbt[:], rb[:, FS_L2:F])
        rbtH[nm] = rbt
    eR2H = pers.tile([64, F], FP32R, tag="eR2H", name="eR2H")
    eI2H = pers.tile([64, F], FP32R, tag="eI2H", name="eI2H")
    cmul6(64, FS_L2,
          (rbH["r"][:, 0:FS_L2], rbH["i"][:, 0:FS_L2]),
          (cp2H["r"][:, 0:FS_L2], cp2H["i"][:, 0:FS_L2]),
          eR2H[:, 0:FS_L2], eI2H[:, 0:FS_L2])
    cmul6p(64, TL,
           (rbtH["r"][:], rbtH["i"][:]),
           (cp2H["r"][:, FS_L2:F], cp2H["i"][:, FS_L2:F]),
           eR2H[:, FS_L2:F], eI2H[:, FS_L2:F])

    # G side (GPSIMD, SBUF operands via single ACT evacuations)
    repG = psum_pool.tile([128, 2 * P], FP32, tag="ps")
    mmr(repG[:], crslice("REPA_G"), t48G[:])
    repGsb = pk.tile([128, 2 * P], FP32, tag="repG")
    nc.scalar.copy(repGsb[:], repG[:])
    c01G = sel_mm(sel_sb["SELL2A_G"], masterG[:], 128, 2 * P)
    c01Gsb = pk.tile([128, 2 * P], FP32, tag="c01G")
    nc.scalar.copy(c01Gsb[:], c01G[:])
    eG01 = pers.tile([128, 2 * P], FP32R, tag="eG01", name="eG01")
    cmul6p(128, P,
           (repGsb[:, 0:P], repGsb[:, P:2 * P]),
           (c01Gsb[:, 0:P], c01Gsb[:, P:2 * P]),
           eG01[:, 0:P], eG01[:, P:2 * P])

    rbG = psum_pool.tile([64, 2 * P], FP32, tag="ps")
    mmr(rbG[:], crslice("REPB_G"), t48G[:])
    rbGsb = pk.tile([64, 2 * P], FP32, tag="rbG")
    nc.scalar.copy(rbGsb[:], rbG[:])
    c2G = sel_mm(sel_sb["SELL2B_G"], masterG[:], 64, 2 * P)
    c2Gsb = pk.tile([64, 2 * P], FP32, tag="c2G")
    nc.scalar.copy(c2Gsb[:], c2G[:])
    eG2 = pers.tile([64, 2 * P], FP32R, tag="eG2", name="eG2")
    cmul6p(64, P,
           (rbGsb[:, 0:P], rbGsb[:, P:2 * P]),
           (c2Gsb[:, 0:P], c2Gsb[:, P:2 * P]),
           eG2[:, 0:P], eG2[:, P:2 * P])

    # negated imag halves (lhsT for the real-part T matmuls)
    negG01 = pers.tile([128, P], FP32R, tag="negG01")
    nc.scalar.mul(negG01[:], eG01[:, P:2 * P], -1.0)
    negG2 = pers.tile([64, P], FP32R, tag="negG2")
    nc.scalar.mul(negG2[:], eG2[:, P:2 * P], -1.0)

    # ---- stage 4: T matmuls -- per (group, comp, chunk) a K=64 pair
    # accumulated in PSUM: Tr = Gr^T Hr + (-Gi)^T Hi ; Ti = Gr^T Hi + Gi^T Hr
    # Order: T0, T1 (combine inputs) first, then T2 (only needed by the
    # final reduction) so p01 overlaps the T2 matmuls.
    def t_mms(g):
        if g < 2:
            Gr = eG01[64 * g:64 * g + 64, 0:P]
            Gi = eG01[64 * g:64 * g + 64, P:2 * P]
            Gin = negG01[64 * g:64 * g + 64, :]
            Hr = eRH[64 * g:64 * g + 64, :]
            Hi = eIH[64 * g:64 * g + 64, :]
        else:
            Gr = eG2[:, 0:P]
            Gi = eG2[:, P:2 * P]
            Gin = negG2[:]
            Hr = eR2H[:]
            Hi = eI2H[:]
        tr = psum_pool.tile([P, F], FP32, tag="ps")
        ti = psum_pool.tile([P, F], FP32, tag="ps")
        for c0 in range(0, F, 512):
            c1 = c0 + 512
            mmr(tr[:, c0:c1], Gr, Hr[:, c0:c1], start=True, stop=False)
            mmr(tr[:, c0:c1], Gin, Hi[:, c0:c1], start=False, stop=True)
            mmr(ti[:, c0:c1], Gr, Hi[:, c0:c1], start=True, stop=False)
            mmr(ti[:, c0:c1], Gi, Hr[:, c0:c1], start=False, stop=True)
        return tr, ti

    t1r_ps, t1i_ps = t_mms(1)
    t0r, t0i = t_mms(0)
    t1r = pers.tile([P, F], FP32, tag="T1r")
    t1i = pers.tile([P, F], FP32, tag="T1i")
    for c0 in range(0, F, 512):
        c1 = c0 + 512
        nc.scalar.copy(t1r[:, c0:c1], t1r_ps[:, c0:c1])
        nc.vector.tensor_copy(t1i[:, c0:c1], t1i_ps[:, c0:c1])

    # p01 = T0*T1: col-split DVE (T0 from PSUM) / GPSIMD (T0 tail via ACT)
    TP = F - FS_PF
    p01r = pers.tile([P, F], FP32, tag="p01r")
    p01i = pers.tile([P, F], FP32, tag="p01i")
    t0tr = pers.tile([P, TP], FP32, tag="t0tr")
    t0ti = pers.tile([P, TP], FP32, tag="t0ti")
    nc.scalar.copy(t0tr[:], t0r[:, FS_PF:F])
    nc.scalar.copy(t0ti[:], t0i[:, FS_PF:F])
    cmul6(P, FS_PF,
          (t0r[:, 0:FS_PF], t0i[:, 0:FS_PF]),
          (t1r[:, 0:FS_PF], t1i[:, 0:FS_PF]),
          p01r[:, 0:FS_PF], p01i[:, 0:FS_PF])
    cmul6p(P, TP,
           (t0tr[:], t0ti[:]),
           (t1r[:, FS_PF:F], t1i[:, FS_PF:F]),
           p01r[:, FS_PF:F], p01i[:, FS_PF:F])

    t2r, t2i = t_mms(2)

    # ---- final reduction: acc[p, k] = sum_f p01 * T2 products (DVE,
    # full width, T2 straight from PSUM).
    # Host combines: perm_r = c0 - c1, perm_i = c2 + c3
    scr2 = pers.tile([P, F], FP32, tag="ttr_scr")
    accD = pers.tile([P, 4], FP32, tag="accD")
    pairs = [(p01r, t2r), (p01i, t2i), (p01r, t2i), (p01i, t2r)]
    for k, (a, b) in enumerate(pairs):
        nc.vector.scalar_tensor_tensor(
            out=scr2[:], in0=b[:], scalar=1.0, in1=a[:],
            op0=OP.mult, op1=OP.mult, accum_out=accD[:, k:k + 1])

    dma(tens["OUT"][:, 0:4], accD[:])

    ctx.close()


# ---------------------------------------------------------------- entry point
def kernel(A_real: np.ndarray, A_imag: np.ndarray) -> np.ndarray:
    B = A_real.shape[0]
    assert B == 8 and A_real.shape == (B, N, N)
    if "nc" not in _CACHE:
        _CACHE["nc"] = build_kernel()
    nc = _CACHE["nc"]
    in_maps = [make_in_map(A_real[b], A_imag[b]) for b in range(B)]
    res = run_bass_kernel_spmd(nc, in_maps, list(range(B)))
    out = np.empty(B, dtype=np.float32)
    for b in range(B):
        acc = res.results[b]["OUT"].reshape(128, 4).astype(np.float64)
        s = acc.sum(axis=0)
        pr = s[0] - s[1]
        pi = s[2] + s[3]
        pa2 = np.float32(pr) ** 2 + np.float32(pi) ** 2
        out[b] = np.float32(EMU * SCALE2 * pa2 + DARK)
    return out


if __name__ == "__main__":
    A_real = np.load("/tmp/A_real.npy")
    A_imag = np.load("/tmp/A_imag.npy")
    print(kernel(A_real, A_imag))


# revision 41
# speedup vs baseline: 3.0090x; 1.0237x over previous
"""Trainium2 Bass kernel: boson-sampler probabilities via Glynn's permanent formula.

Math (per 18x18 complex matrix A):
  perm(A) = 2^(1-n) * sum_{d in {+-1}^n, d_0=+1} (prod_k d_k) * prod_i (sum_j d_j A[i,j])
The 2^17 sign vectors form a [128 x 1024] grid (7 "p" bits drive columns 11..17,
10 "f" bits drive columns 1..10; column 0 fixed +1). Row-sums factor as
rs_i = RP_i(p) + RF_i(f); rows are grouped [6,6,6] and each group's product
expands as T_g[p,f] = sum_{c<64} G_g[c,p] * H_g[c,f] -- fp32r matmuls on the
tensor engine (full rate at free>=256). The 64-row G/H tables (all sub-products
of 6 rows) are built hierarchically (pairs -> quads -> tables): packed operand
sets are assembled from SBUF "master" row tiles by 0/1 selection matmuls, and
each level is a set of elementwise complex-multiply ops column-split across
the DVE and GPSIMD engines. Glynn parity signs are folded into group 2's
tables via sign-scaled mask constants. G-side (p-axis) work packs real|imag
side by side in one [32, 256] master so every G matmul runs at free=256.
The final sum(T0*T1*T2) reduces via 8 scalar_tensor_tensor accumulations
(col-split DVE/Pool); |perm|^2, the (underflowed-to-zero) classical term and
the dark-count offset are applied on the host. One NeuronCore per batch
element.
"""

import sys

sys.path.insert(0, "/opt/trn_rl_repo")

import numpy as np

import concourse.bacc as bacc
import concourse.bass as bass
import concourse.tile as tile
from concourse import mybir
from concourse.bass_utils import run_bass_kernel_spmd

FP32 = mybir.dt.float32
FP32R = mybir.dt.float32r
OP = mybir.AluOpType

N = 18
PBITS, FBITS = 7, 10
P, F = 1 << PBITS, 1 << FBITS          # 128, 1024
EMU = 0.85 * (1 - 0.02) * (1 - 0.02) * (1 - 0.01)
DARK = 1e-6 * N
SCALE2 = float(2.0 ** (2 * (1 - N)))

# master row map (same for both sides; imag comp has zeros at ONES/SIGN)
M_RF = 0          # rows 0..17: row-sums RF_i / RP_i
M_SE = 18         # sign-scaled even row of pair 8 (RFe' / RPe')
M_SO = 19         # sign-scaled odd row (RFo' / RPo')
M_ONE = 20        # ones (real) / zeros (imag)
M_SGN = 21        # sign row sF / sP (real) / zeros (imag)
M_PP = 32         # rows 32..41: pair products PP_q (row 41 = PP' scaled)
M_PPS = 41        # row 41: PP' = sign-scaled pair-8 product (32-aligned for GPSIMD)
M_PAD = 42
CROWS = 48        # const-pack row count (REPA/REPB lhsTs span 48 rows)

# column split: DVE takes [0:x], GPSIMD takes [x:F] of each wide stage
FS_P1 = 704       # pair / L1 stages
FS_L2 = 576       # L2 table-build stages
FS_PF = 576       # p01 / final reduction stages

_CACHE = {}


def _pm_mask(nvals, bits):
    v = np.arange(nvals, dtype=np.uint32)
    m = (v[:, None] >> np.arange(bits, dtype=np.uint32)[None, :]) & 1
    return (1.0 - 2.0 * m).astype(np.float32).T.copy()   # [bits, nvals]


def _parity(nvals, bits):
    v = np.arange(nvals, dtype=np.uint32)
    pc = np.zeros(nvals, dtype=np.uint32)
    for k in range(bits):
        pc += (v >> k) & 1
    return np.where(pc % 2 == 0, 1.0, -1.0).astype(np.float32)


class SideSpec:
    """H: f-side (width 1024, pair-table identity at z=3);
       G: p-side (width 128, identity at z=0)."""

    def __init__(self, name, width, mult_z):
        self.name = name
        self.w = width
        self.mult_z = list(mult_z)
        self.idz = ({0, 1, 2, 3} - set(mult_z)).pop()

    # pair-table entry -> master row (pair q, entry z); identity z -> ones row
    def tab_row(self, q, z):
        if z == self.idz:
            return M_ONE
        if self.name == "H":
            return {0: M_PP + q, 1: 2 * q + 1, 2: 2 * q}[z]
        return {1: 2 * q, 2: 2 * q + 1, 3: M_PP + q}[z]

    # group-2 scaled pair-table entry (pair 8) -> master row
    def tab2_row(self, z):
        if z == self.idz:
            return M_SGN
        if self.name == "H":
            return {0: M_PPS, 1: M_SO, 2: M_SE}[z]
        return {1: M_SE, 2: M_SO, 3: M_PPS}[z]


HSPEC = SideSpec("H", F, (0, 1, 2))
GSPEC = SideSpec("G", P, (1, 2, 3))


def _sel(rows, m_pad=None):
    """Selection matrix [M_PAD, len(rows)] with one 1 per used column."""
    M = len(rows) if m_pad is None else m_pad
    s = np.zeros((M_PAD, M), np.float32)
    for m, k in enumerate(rows):
        if k is not None:
            s[k, m] = 1.0
    return s


# const pack column layout: computed once at import
def _build_pack():
    cols = {}
    blocks = []
    off = 0

    def add(name, arr):
        nonlocal off
        a = np.zeros((CROWS, arr.shape[1]), np.float32)
        a[0:arr.shape[0], :] = arr
        cols[name] = (off, arr.shape[1])
        blocks.append(a)
        off += arr.shape[1]

    # MFX: [ones;pm(10) | (ones;pm)*sF] (row 0 = ones source, row 11 = sF source)
    mF = np.concatenate([np.ones((1, F), np.float32), _pm_mask(F, FBITS)], axis=0)
    sF = _parity(F, FBITS)
    add("MFX", np.concatenate([mF, mF * sF[None, :]], axis=0))           # [22, F]
    # MPX: [pm(7) | pm*sP | ones | sP]
    mP = _pm_mask(P, PBITS)
    sP = _parity(P, PBITS)
    add("MPX", np.concatenate(
        [mP, mP * sP[None, :], np.ones((1, P), np.float32), sP[None, :]], axis=0))
    # unit columns for the widened RS matmul lhsT (cols 20,21), per side
    uh = np.zeros((22, 2), np.float32)
    uh[0, 0] = 1.0      # -> MFX row 0 (ones)
    uh[11, 1] = 1.0     # -> MFX row 11 (sF)
    ug = np.zeros((16, 2), np.float32)
    ug[14, 0] = 1.0     # -> MPX row 14 (ones)
    ug[15, 1] = 1.0     # -> MPX row 15 (sP)
    cols["_U"] = (uh, ug)

    def digits(c):
        return c % 4, (c // 4) % 4, c // 16          # za, zb, zc

    for spec in (HSPEC, GSPEC):
        nm = spec.name
        # L1: 48 distinct products tmp48 indexed by (g, c2), c2 = za + 4zb:
        # in0/in1 packed [48 | pad | 48] in one mm
        in0 = []
        in1 = []
        for g in range(3):
            for c2 in range(16):
                in0.append(spec.tab_row(3 * g, c2 % 4))
                in1.append(spec.tab_row(3 * g + 1, c2 // 4))
        add(f"SELL1_{nm}", _sel(in0 + [None] * 16 + in1, 112))
        # REP: replicate tmp48 rows into L2 src layout (lhsT for rep matmuls)
        # repA: [64g + c] <- tmp48[16g + (c % 16)] for g=0,1 ; repB: g=2
        repA = np.zeros((48, 128), np.float32)
        for g in range(2):
            for c in range(64):
                repA[16 * g + (c % 16), 64 * g + c] = 1.0
        add(f"REPA_{nm}", repA)
        repB = np.zeros((48, 64), np.float32)
        for c in range(64):
            repB[32 + (c % 16), c] = 1.0
        add(f"REPB_{nm}", repB)
        # L2 c-packs: set1 rows [64g+c] = tab_c[zc]; set2 = scaled tab2'[zc]
        rows = []
        for g in range(2):
            for c in range(64):
                rows.append(spec.tab_row(3 * g + 2, digits(c)[2]))
        add(f"SELL2A_{nm}", _sel(rows))
        rows = [spec.tab2_row(digits(c)[2]) for c in range(64)]
        add(f"SELL2B_{nm}", _sel(rows))
    return np.concatenate(blocks, axis=1), cols


CPACK, CPACK_COLS = _build_pack()
U_HG = CPACK_COLS.pop("_U")
CW = CPACK.shape[1]

def _build_packr():
    parts = []
    offs = {}
    off = 0
    for nm in ("REPA_H", "REPB_H", "REPA_G", "REPB_G"):
        o, w = CPACK_COLS[nm]
        parts.append(CPACK[0:CROWS, o:o + w])
        offs[nm] = (off, w)
        off += w
    return np.ascontiguousarray(np.concatenate(parts, axis=1)), offs


CPACKR, CPACKR_COLS = _build_packr()
CRW = CPACKR.shape[1]


def build_lts(Ar, Ai):
    """Host-side lhsT pack: per (side, comp) the widened row-sum lhsT
    (32 cols) and the pair-pack lhsT (42 cols), one [22, 296] tensor."""
    lts = np.zeros((22, 296), np.float32)
    for si, (lo, kb, K) in enumerate(((0, 11, 22), (11, 7, 16))):
        for ci, A in ((0, Ar), (1, Ai)):
            base = 74 * (2 * si + ci)
            AT = np.ascontiguousarray(A.T, dtype=np.float32)
            lts[0:kb, base:base + 18] = AT[lo:lo + kb, 0:18]
            lts[kb:2 * kb, base + 18:base + 20] = AT[lo:lo + kb, 16:18]
            if ci == 0:
                lts[0:K, base + 20:base + 22] = U_HG[si]
            pb = base + 32
            lts[0:kb, pb:pb + 9] = AT[lo:lo + kb, 0:17:2]
            lts[kb:2 * kb, pb + 9] = AT[lo:lo + kb, 16]
            lts[0:kb, pb + 32:pb + 41] = AT[lo:lo + kb, 1:18:2]
            lts[0:kb, pb + 41] = AT[lo:lo + kb, 17]
    return lts


def make_in_map(Ar, Ai):
    return {"CPACK": CPACK, "CPACKR": CPACKR, "LTS": build_lts(Ar, Ai)}


def host_consts():
    return {"CPACK": CPACK}


# ---------------------------------------------------------------- kernel body
def build_kernel(loop_iters=None):
    nc = bacc.Bacc("TRN2", target_bir_lowering=False, debug=False)

    tens = {}
    tens["LTS"] = nc.dram_tensor("LTS", [22, 296], FP32, kind="ExternalInput").ap()
    tens["CPACK"] = nc.dram_tensor("CPACK", [CROWS, CW], FP32,
                                   kind="ExternalInput").ap()
    tens["CPACKR"] = nc.dram_tensor("CPACKR", [CROWS, CRW], FP32R,
                                    kind="ExternalInput").ap()
    tens["OUT"] = nc.dram_tensor("OUT", [128, 8], FP32, kind="ExternalOutput").ap()

    with tile.TileContext(nc) as tc:
        if loop_iters is None:
            _body(nc, tc, tens)
        else:
            with tc.For_i(0, loop_iters, 1):
                _body(nc, tc, tens)
    nc.compile()
    return nc


def _body(nc, tc, tens):
    from contextlib import ExitStack

    ctx = ExitStack()
    pers = ctx.enter_context(tc.tile_pool(name="pers", bufs=1))
    pk = ctx.enter_context(tc.tile_pool(name="pk", bufs=2))
    cm = ctx.enter_context(tc.tile_pool(name="cm", bufs=2))
    psum_pool = ctx.enter_context(tc.tile_pool(name="psum", bufs=4, space="PSUM"))
    dma = nc.sync.dma_start
    dma2 = nc.gpsimd.dma_start          # SWDGE queues for small input loads

    def mmr(out_ap, lhsT_ap, rhs_ap, start=True, stop=True):
        """fp32r matmul: full rate (1 cyc/row) on trn2 when free >= 256."""
        nc.tensor.matmul(out_ap, lhsT_ap.bitcast(FP32R), rhs_ap.bitcast(FP32R),
                         start=start, stop=stop)

    def cmul6(rows, w, i0, i1, outr, outi):
        """DVE complex multiply: (i0r,i0i)*(i1r,i1i) -> (outr,outi)."""
        e = nc.vector
        i0r, i0i = i0
        i1r, i1i = i1
        t1 = cm.tile([rows, w], FP32, tag="cm_t1")
        t2 = cm.tile([rows, w], FP32, tag="cm_t2")
        e.tensor_mul(t1[:], i0r, i1r)
        e.tensor_mul(t2[:], i0i, i1i)
        e.tensor_sub(outr, t1[:], t2[:])
        e.tensor_mul(t1[:], i0r, i1i)
        e.tensor_mul(t2[:], i0i, i1r)
        e.tensor_add(outi, t1[:], t2[:])

    def cmul6p(rows, w, i0, i1, outr, outi):
        """GPSIMD complex multiply (plain tensor-tensor ops)."""
        e = nc.gpsimd
        i0r, i0i = i0
        i1r, i1i = i1
        t1 = cm.tile([rows, w], FP32, tag="gp_t1")
        t2 = cm.tile([rows, w], FP32, tag="gp_t2")
        e.tensor_mul(t1[:], i0r, i1r)
        e.tensor_mul(t2[:], i0i, i1i)
        e.tensor_sub(outr, t1[:], t2[:])
        e.tensor_mul(t1[:], i0r, i1i)
        e.tensor_mul(t2[:], i0i, i1r)
        e.tensor_add(outi, t1[:], t2[:])

    def sel_mm(sel_sb, msrc, m, w):
        """Pack = SEL.T @ master -> PSUM [m, w]."""
        ps = psum_pool.tile([m, w], FP32, tag="ps")
        for c0 in range(0, w, 512):
            c1 = min(c0 + 512, w)
            nc.tensor.matmul(ps[:, c0:c1], sel_sb[:], msrc[:, c0:c1],
                             start=True, stop=True)
        return ps

    # ---- stage 0: A loads, widened row-sum matmuls -> master rows 0..21
    cpk = pers.tile([CROWS, CW], FP32, tag="cpack")
    dma(cpk[:, 0:128], tens["CPACK"][:, 0:128])
    cpkr = pers.tile([CROWS, CRW], FP32R, tag="cpackr")
    nc.scalar.dma_start(cpkr[:], tens["CPACKR"][:, :])

    def crslice(name):
        off, width = CPACKR_COLS[name]
        return cpkr[0:48, off:off + width]
    lts = pers.tile([22, 296], FP32, tag="lts")
    dma(lts[:], tens["LTS"][:, :])
    dma(cpk[:, 128:512], tens["CPACK"][:, 128:512])
    nc.scalar.dma_start(cpk[:, 512:1024], tens["CPACK"][:, 512:1024])
    dma2(cpk[:, 1024:CW], tens["CPACK"][:, 1024:CW])

    warm = psum_pool.tile([128, 64], FP32, tag="ps")
    for _ in range(8):
        nc.tensor.matmul(warm[:], cpk[0:48, 0:128], cpk[0:48, 0:64],
                         start=True, stop=True)

    def cslice(name, nrows=None):
        off, width = CPACK_COLS[name]
        nr = M_PAD if nrows is None else nrows
        return cpk[0:nr, off:off + width]

    lhsT_rs = {}
    lhsT_pp = {}
    KRS = {"H": 22, "G": 16}
    for si, side in enumerate("HG"):
        for ci, nm in enumerate("ri"):
            base = 74 * (2 * si + ci)
            K_rs = KRS[side]
            lhsT_rs[(side, nm)] = lts[0:K_rs, base:base + 32]
            lhsT_pp[(side, nm)] = lts[0:K_rs, base + 32:base + 74]

    mask_sb = {"H": cslice("MFX", 22), "G": cslice("MPX", 16)}
    sel_sb = {}
    for spec in (HSPEC, GSPEC):
        for s in ("SELL1", "SELL2A", "SELL2B"):
            key = f"{s}_{spec.name}"
            sel_sb[key] = cslice(key)
        for s in ("REPA", "REPB"):
            key = f"{s}_{spec.name}"
            sel_sb[key] = cslice(key, 48)

    # H masters: one [M_PAD, F] tile per component; G master: [M_PAD, 2P]
    # with real in cols 0:P, imag in P:2P. All 32 rows get written (22 by
    # the RS copy, 10 by the pair stage) -- no memset needed.
    masterH = {}
    for nm in "ri":
        t = pers.tile([M_PAD, F], FP32, tag=f"mstH{nm}", name=f"mstH{nm}")
        masterH[nm] = t
    masterG = pers.tile([M_PAD, 2 * P], FP32, tag="mstG")

    # ---- stage 1: pair products -> master rows 32..41 (packs come
    # straight from the rearranged ltp lhsTs -- no master dependency)
    P1T = F - FS_P1
    psH1 = {}
    for nm in "ri":
        ps = sel_mm(lhsT_pp[("H", nm)], mask_sb["H"], 42, F)
        sb = pk.tile([10, F], FP32, tag=f"halfH{nm}")
        if nm == "r":
            nc.scalar.copy(sb[:], ps[32:42, :])
        else:
            nc.vector.tensor_copy(sb[:], ps[32:42, :])
        p0t = pk.tile([10, P1T], FP32, tag=f"p0tH{nm}")
        nc.scalar.copy(p0t[:], ps[0:10, FS_P1:F])
        psH1[nm] = (ps, sb, p0t)
    cmul6(10, FS_P1,
          (psH1["r"][0][0:10, 0:FS_P1], psH1["i"][0][0:10, 0:FS_P1]),
          (psH1["r"][1][:, 0:FS_P1], psH1["i"][1][:, 0:FS_P1]),
          masterH["r"][M_PP:M_PP + 10, 0:FS_P1],
          masterH["i"][M_PP:M_PP + 10, 0:FS_P1])
    cmul6p(10, P1T,
           (psH1["r"][2][:], psH1["i"][2][:]),
           (psH1["r"][1][:, FS_P1:F], psH1["i"][1][:, FS_P1:F]),
           masterH["r"][M_PP:M_PP + 10, FS_P1:F],
           masterH["i"][M_PP:M_PP + 10, FS_P1:F])

    psG1 = psum_pool.tile([42, 2 * P], FP32, tag="ps")
    nc.tensor.matmul(psG1[:, 0:P], lhsT_pp[("G", "r")], mask_sb["G"][:],
                     start=True, stop=True)
    nc.tensor.matmul(psG1[:, P:2 * P], lhsT_pp[("G", "i")], mask_sb["G"][:],
                     start=True, stop=True)
    sbG1a = pk.tile([10, 2 * P], FP32, tag="selpGa")
    nc.scalar.copy(sbG1a[:], psG1[0:10, :])
    sbG1b = pk.tile([10, 2 * P], FP32, tag="selpGb")
    nc.scalar.copy(sbG1b[:], psG1[32:42, :])
    cmul6p(10, P,
           (sbG1a[:, 0:P], sbG1a[:, P:2 * P]),
           (sbG1b[:, 0:P], sbG1b[:, P:2 * P]),
           masterG[M_PP:M_PP + 10, 0:P], masterG[M_PP:M_PP + 10, P:2 * P])

    # row-sum masters (needed from L1 onward; emitted after the pair
    # stage so its PSUM evacuations win the ACT queue early)
    for nm in "ri":
        lt = lhsT_rs[("H", nm)]
        ps = psum_pool.tile([32, F], FP32, tag="ps")
        for c0 in range(0, F, 512):
            c1 = min(c0 + 512, F)
            nc.tensor.matmul(ps[:, c0:c1], lt, mask_sb["H"][:, c0:c1],
                             start=True, stop=True)
        nc.scalar.copy(masterH[nm][0:32, :], ps[:])
    psG = psum_pool.tile([32, 2 * P], FP32, tag="ps")
    nc.tensor.matmul(psG[:, 0:P], lhsT_rs[("G", "r")], mask_sb["G"][:],
                     start=True, stop=True)
    nc.tensor.matmul(psG[:, P:2 * P], lhsT_rs[("G", "i")], mask_sb["G"][:],
                     start=True, stop=True)
    nc.scalar.copy(masterG[0:32, :], psG[:])


    # ---- stage 2 (L1): tmp48[16g + c2] = tab_a[za] * tab_b[zb]
    t48H = {}
    l1t = {}
    for nm in "ri":
        ps = sel_mm(sel_sb["SELL1_H"], masterH[nm][:], 112, F)
        sb1 = pk.tile([48, F], FP32, tag=f"l1hH{nm}")
        if nm == "r":
            nc.scalar.copy(sb1[:], ps[64:112, :])
        else:
            nc.vector.tensor_copy(sb1[:], ps[64:112, :])
        lt1 = pk.tile([48, P1T], FP32, tag=f"l1tH{nm}")
        nc.scalar.copy(lt1[:], ps[0:48, FS_P1:F])
        l1t[nm] = lt1
        t48 = pers.tile([48, F], FP32R, tag=f"t48H{nm}", name=f"t48H{nm}")
        t48H[nm] = (ps, sb1, t48)
    cmul6(48, FS_P1,
          (t48H["r"][0][0:48, 0:FS_P1], t48H["i"][0][0:48, 0:FS_P1]),
          (t48H["r"][1][:, 0:FS_P1], t48H["i"][1][:, 0:FS_P1]),
          t48H["r"][2][:, 0:FS_P1], t48H["i"][2][:, 0:FS_P1])
    cmul6p(48, P1T,
           (l1t["r"][:], l1t["i"][:]),
           (t48H["r"][1][:, FS_P1:F], t48H["i"][1][:, FS_P1:F]),
           t48H["r"][2][:, FS_P1:F], t48H["i"][2][:, FS_P1:F])

    psL1G = sel_mm(sel_sb["SELL1_G"], masterG[:], 112, 2 * P)
    sbL1Ga = pk.tile([48, 2 * P], FP32, tag="l1Ga")
    nc.scalar.copy(sbL1Ga[:], psL1G[0:48, :])
    sbL1Gb = pk.tile([48, 2 * P], FP32, tag="l1Gb")
    nc.scalar.copy(sbL1Gb[:], psL1G[64:112, :])
    t48G = pers.tile([48, 2 * P], FP32R, tag="t48G", name="t48G")
    cmul6p(48, P,
           (sbL1Ga[:, 0:P], sbL1Ga[:, P:2 * P]),
           (sbL1Gb[:, 0:P], sbL1Gb[:, P:2 * P]),
           t48G[:, 0:P], t48G[:, P:2 * P])

    # ---- stage 3 (L2): e_g = tmp * tab_c[zc], column-split DVE / GPSIMD
    # H g0+g1 fused as one [128, F] set -> eRH = [H0r; H1r], eIH = [H0i; H1i].
    # The T matmuls compensate with K=64 accumulating pairs.
    TL = F - FS_L2
    c01H = {}
    repH = {}
    reptH = {}
    for nm in "ri":
        c01 = sel_mm(sel_sb["SELL2A_H"], masterH[nm][:], 128, F)
        c01sb = pk.tile([128, F], FP32, tag=f"c01H{nm}")
        if nm == "r":
            nc.scalar.copy(c01sb[:], c01[:])
        else:
            nc.vector.tensor_copy(c01sb[:], c01[:])
        c01H[nm] = c01sb
        rep = psum_pool.tile([128, F], FP32, tag="ps")
        for c0 in range(0, F, 512):
            mmr(rep[:, c0:c0 + 512], crslice("REPA_H"),
                t48H[nm][2][:, c0:c0 + 512])
        repH[nm] = rep
        rt = pk.tile([128, TL], FP32, tag=f"reptH{nm}")
        nc.scalar.copy(rt[:], repH[nm][:, FS_L2:F])
        reptH[nm] = rt
    eRH = pers.tile([128, F], FP32R, tag="eRH", name="eRH")
    eIH = pers.tile([128, F], FP32R, tag="eIH", name="eIH")
    cmul6(128, FS_L2,
          (repH["r"][0:128, 0:FS_L2], repH["i"][0:128, 0:FS_L2]),
          (c01H["r"][:, 0:FS_L2], c01H["i"][:, 0:FS_L2]),
          eRH[:, 0:FS_L2], eIH[:, 0:FS_L2])
    cmul6p(128, TL,
           (reptH["r"][:], reptH["i"][:]),
           (c01H["r"][:, FS_L2:F], c01H["i"][:, FS_L2:F]),
           eRH[:, FS_L2:F], eIH[:, FS_L2:F])

    cp2H = {}
    rbH = {}
    rbtH = {}
    for nm in "ri":
        c2 = sel_mm(sel_sb["SELL2B_H"], masterH[nm][:], 64, F)
        c2sb = pk.tile([64, F], FP32, tag=f"c2H{nm}")
        nc.scalar.copy(c2sb[:], c2[:])
        cp2H[nm] = c2sb
        rb = psum_pool.tile([64, F], FP32, tag="ps")
        for c0 in range(0, F, 512):
            mmr(rb[:, c0:c0 + 512], crslice("REPB_H"),
                t48H[nm][2][:, c0:c0 + 512])
        rbH[nm] = rb
        rbt = pk.tile([64, TL], FP32, tag=f"rbtH{nm}")
        nc.scalar.copy(rbt[:], rb[:, FS_L2:F])
        rbtH[nm] = rbt
    eR2H = pers.tile([64, F], FP32R, tag="eR2H", name="eR2H")
    eI2H = pers.tile([64, F], FP32R, tag="eI2H", name="eI2H")
    cmul6(64, FS_L2,
          (rbH["r"][:, 0:FS_L2], rbH["i"][:, 0:FS_L2]),
          (cp2H["r"][:, 0:FS_L2], cp2H["i"][:, 0:FS_L2]),
          eR2H[:, 0:FS_L2], eI2H[:, 0:FS_L2])
    cmul6p(64, TL,
           (rbtH["r"][:], rbtH["i"][:]),
           (cp2H["r"][:, FS_L2:F], cp2H["i"][:, FS_L2:F]),
           eR2H[:, FS_L2:F], eI2H[:, FS_L2:F])

    # G side (GPSIMD, SBUF operands via single ACT evacuations)
    repG = psum_pool.tile([128, 2 * P], FP32, tag="ps")
    mmr(repG[:], crslice("REPA_G"), t48G[:])
    repGsb = pk.tile([128, 2 * P], FP32, tag="repG")
    nc.scalar.copy(repGsb[:], repG[:])
    c01G = sel_mm(sel_sb["SELL2A_G"], masterG[:], 128, 2 * P)
    c01Gsb = pk.tile([128, 2 * P], FP32, tag="c01G")
    nc.scalar.copy(c01Gsb[:], c01G[:])
    eG01 = pers.tile([128, 2 * P], FP32R, tag="eG01", name="eG01")
    cmul6p(128, P,
           (repGsb[:, 0:P], repGsb[:, P:2 * P]),
           (c01Gsb[:, 0:P], c01Gsb[:, P:2 * P]),
           eG01[:, 0:P], eG01[:, P:2 * P])

    rbG = psum_pool.tile([64, 2 * P], FP32, tag="ps")
    mmr(rbG[:], crslice("REPB_G"), t48G[:])
    rbGsb = pk.tile([64, 2 * P], FP32, tag="rbG")
    nc.scalar.copy(rbGsb[:], rbG[:])
    c2G = sel_mm(sel_sb["SELL2B_G"], masterG[:], 64, 2 * P)
    c2Gsb = pk.tile([64, 2 * P], FP32, tag="c2G")
    nc.scalar.copy(c2Gsb[:], c2G[:])
    eG2 = pers.tile([64, 2 * P], FP32R, tag="eG2", name="eG2")
    cmul6p(64, P,
           (rbGsb[:, 0:P], rbGsb[:, P:2 * P]),
           (c2Gsb[:, 0:P], c2Gsb[:, P:2 * P]),
           eG2[:, 0:P], eG2[:, P:2 * P])

    # negated imag halves (lhsT for the real-part T matmuls)
    negG01 = pers.tile([128, P], FP32R, tag="negG01")
    nc.scalar.mul(negG01[:], eG01[:, P:2 * P], -1.0)
    negG2 = pers.tile([64, P], FP32R, tag="negG2")
    nc.scalar.mul(negG2[:], eG2[:, P:2 * P], -1.0)

    # ---- stage 4: T matmuls -- per (group, comp, chunk) a K=64 pair
    # accumulated in PSUM: Tr = Gr^T Hr + (-Gi)^T Hi ; Ti = Gr^T Hi + Gi^T Hr
    # Order: T0, T1 (combine inputs) first, then T2 (only needed by the
    # final reduction) so p01 overlaps the T2 matmuls.
    def t_mms(g):
        if g < 2:
            Gr = eG01[64 * g:64 * g + 64, 0:P]
            Gi = eG01[64 * g:64 * g + 64, P:2 * P]
            Gin = negG01[64 * g:64 * g + 64, :]
            Hr = eRH[64 * g:64 * g + 64, :]
            Hi = eIH[64 * g:64 * g + 64, :]
        else:
            Gr = eG2[:, 0:P]
            Gi = eG2[:, P:2 * P]
            Gin = negG2[:]
            Hr = eR2H[:]
            Hi = eI2H[:]
        tr = psum_pool.tile([P, F], FP32, tag="ps")
        ti = psum_pool.tile([P, F], FP32, tag="ps")
        for c0 in range(0, F, 512):
            c1 = c0 + 512
            mmr(tr[:, c0:c1], Gr, Hr[:, c0:c1], start=True, stop=False)
            mmr(tr[:, c0:c1], Gin, Hi[:, c0:c1], start=False, stop=True)
            mmr(ti[:, c0:c1], Gr, Hi[:, c0:c1], start=True, stop=False)
            mmr(ti[:, c0:c1], Gi, Hr[:, c0:c1], start=False, stop=True)
        return tr, ti

    t1r_ps, t1i_ps = t_mms(1)
    t0r, t0i = t_mms(0)
    t1r = pers.tile([P, F], FP32, tag="T1r")
    t1i = pers.tile([P, F], FP32, tag="T1i")
    for c0 in range(0, F, 512):
        c1 = c0 + 512
        nc.scalar.copy(t1r[:, c0:c1], t1r_ps[:, c0:c1])
        nc.vector.tensor_copy(t1i[:, c0:c1], t1i_ps[:, c0:c1])

    # p01 = T0*T1: col-split DVE (T0 from PSUM) / GPSIMD (T0 tail via ACT)
    TP = F - FS_PF
    p01r = pers.tile([P, F], FP32, tag="p01r")
    p01i = pers.tile([P, F], FP32, tag="p01i")
    t0tr = pers.tile([P, TP], FP32, tag="t0tr")
    t0ti = pers.tile([P, TP], FP32, tag="t0ti")
    nc.scalar.copy(t0tr[:], t0r[:, FS_PF:F])
    nc.scalar.copy(t0ti[:], t0i[:, FS_PF:F])
    cmul6(P, FS_PF,
          (t0r[:, 0:FS_PF], t0i[:, 0:FS_PF]),
          (t1r[:, 0:FS_PF], t1i[:, 0:FS_PF]),
          p01r[:, 0:FS_PF], p01i[:, 0:FS_PF])
    cmul6p(P, TP,
           (t0tr[:], t0ti[:]),
           (t1r[:, FS_PF:F], t1i[:, FS_PF:F]),
           p01r[:, FS_PF:F], p01i[:, FS_PF:F])

    t2r, t2i = t_mms(2)

    # ---- final reduction, engine-split: DVE runs STT-accum over cols
    # 0:XT (T2 straight from PSUM); for the tail GPSIMD forms the products
    # and ACT accumulates them (activation Copy with accum_out).
    # Host combines: perm_r = (c0-c1)+(c4-c5), perm_i = (c2+c3)+(c6+c7)
    XT = 576
    TT2 = F - XT
    t2tr = pers.tile([P, TT2], FP32, tag="t2tr")
    t2ti = pers.tile([P, TT2], FP32, tag="t2ti")
    nc.scalar.copy(t2tr[:], t2r[:, XT:F])
    nc.scalar.copy(t2ti[:], t2i[:, XT:F])
    scr2 = pers.tile([P, XT], FP32, tag="ttr_scr")
    accD = pers.tile([P, 4], FP32, tag="accD")
    accA = pers.tile([P, 4], FP32, tag="accA")
    pairs = [(p01r, t2r, t2tr), (p01i, t2i, t2ti), (p01r, t2i, t2ti),
             (p01i, t2r, t2tr)]
    wo = pers.tile([P, TT2], FP32, tag="two")
    for k, (a, b, bt) in enumerate(pairs):
        nc.vector.scalar_tensor_tensor(
            out=scr2[:], in0=b[:, 0:XT], scalar=1.0, in1=a[:, 0:XT],
            op0=OP.mult, op1=OP.mult, accum_out=accD[:, k:k + 1])
        wk = pk.tile([P, TT2], FP32, tag="tw")
        nc.gpsimd.tensor_mul(wk[:], bt[:], a[:, XT:F])
        nc.scalar.activation(wo[:], wk[:], mybir.ActivationFunctionType.Copy,
                             accum_out=accA[:, k:k + 1])

    dma(tens["OUT"][:, 0:4], accD[:])
    dma(tens["OUT"][:, 4:8], accA[:])

    ctx.close()


# ---------------------------------------------------------------- entry point
def kernel(A_real: np.ndarray, A_imag: np.ndarray) -> np.ndarray:
    B = A_real.shape[0]
    assert B == 8 and A_real.shape == (B, N, N)
    if "nc" not in _CACHE:
        _CACHE["nc"] = build_kernel()
    nc = _CACHE["nc"]
    in_maps = [make_in_map(A_real[b], A_imag[b]) for b in range(B)]
    res = run_bass_kernel_spmd(nc, in_maps, list(range(B)))
    out = np.empty(B, dtype=np.float32)
    for b in range(B):
        acc = res.results[b]["OUT"].reshape(128, 8).astype(np.float64)
        s = acc.sum(axis=0)
        pr = (s[0] - s[1]) + (s[4] - s[5])
        pi = (s[2] + s[3]) + (s[6] + s[7])
        pa2 = np.float32(pr) ** 2 + np.float32(pi) ** 2
        out[b] = np.float32(EMU * SCALE2 * pa2 + DARK)
    return out


if __name__ == "__main__":
    A_real = np.load("/tmp/A_real.npy")
    A_imag = np.load("/tmp/A_imag.npy")
    print(kernel(A_real, A_imag))


# revision 44
# speedup vs baseline: 3.0647x; 1.0185x over previous
"""Trainium2 Bass kernel: boson-sampler probabilities via Glynn's permanent formula.

Math (per 18x18 complex matrix A):
  perm(A) = 2^(1-n) * sum_{d in {+-1}^n, d_0=+1} (prod_k d_k) * prod_i (sum_j d_j A[i,j])
The 2^17 sign vectors form a [128 x 1024] grid (7 "p" bits drive columns 11..17,
10 "f" bits drive columns 1..10; column 0 fixed +1). Row-sums factor as
rs_i = RP_i(p) + RF_i(f); rows are grouped [6,6,6] and each group's product
expands as T_g[p,f] = sum_{c<64} G_g[c,p] * H_g[c,f] -- fp32r matmuls on the
tensor engine (full rate at free>=256). The 64-row G/H tables (all sub-products
of 6 rows) are built hierarchically (pairs -> quads -> tables): packed operand
sets are assembled from SBUF "master" row tiles by 0/1 selection matmuls, and
each level is a set of elementwise complex-multiply ops column-split across
the DVE and GPSIMD engines. Glynn parity signs are folded into group 2's
tables via sign-scaled mask constants. G-side (p-axis) work packs real|imag
side by side in one [32, 256] master so every G matmul runs at free=256.
The final sum(T0*T1*T2) reduces via 8 scalar_tensor_tensor accumulations
(col-split DVE/Pool); |perm|^2, the (underflowed-to-zero) classical term and
the dark-count offset are applied on the host. One NeuronCore per batch
element.
"""

import sys

sys.path.insert(0, "/opt/trn_rl_repo")

import numpy as np

import concourse.bacc as bacc
import concourse.bass as bass
import concourse.tile as tile
from concourse import mybir
from concourse.bass_utils import run_bass_kernel_spmd

FP32 = mybir.dt.float32
FP32R = mybir.dt.float32r
OP = mybir.AluOpType

N = 18
PBITS, FBITS = 7, 10
P, F = 1 << PBITS, 1 << FBITS          # 128, 1024
EMU = 0.85 * (1 - 0.02) * (1 - 0.02) * (1 - 0.01)
DARK = 1e-6 * N
SCALE2 = float(2.0 ** (2 * (1 - N)))

# master row map (same for both sides; imag comp has zeros at ONES/SIGN)
M_RF = 0          # rows 0..17: row-sums RF_i / RP_i
M_SE = 18         # sign-scaled even row of pair 8 (RFe' / RPe')
M_SO = 19         # sign-scaled odd row (RFo' / RPo')
M_ONE = 20        # ones (real) / zeros (imag)
M_SGN = 21        # sign row sF / sP (real) / zeros (imag)
M_PP = 32         # rows 32..41: pair products PP_q (row 41 = PP' scaled)
M_PPS = 41        # row 41: PP' = sign-scaled pair-8 product (32-aligned for GPSIMD)
M_PAD = 42
CROWS = 48        # const-pack row count (REPA/REPB lhsTs span 48 rows)

# column split: DVE takes [0:x], GPSIMD takes [x:F] of each wide stage
FS_P1 = 704       # pair / L1 stages
FS_L2 = 576       # L2 table-build stages
FS_PF = 576       # p01 / final reduction stages

_CACHE = {}


def _pm_mask(nvals, bits):
    v = np.arange(nvals, dtype=np.uint32)
    m = (v[:, None] >> np.arange(bits, dtype=np.uint32)[None, :]) & 1
    return (1.0 - 2.0 * m).astype(np.float32).T.copy()   # [bits, nvals]


def _parity(nvals, bits):
    v = np.arange(nvals, dtype=np.uint32)
    pc = np.zeros(nvals, dtype=np.uint32)
    for k in range(bits):
        pc += (v >> k) & 1
    return np.where(pc % 2 == 0, 1.0, -1.0).astype(np.float32)


class SideSpec:
    """H: f-side (width 1024, pair-table identity at z=3);
       G: p-side (width 128, identity at z=0)."""

    def __init__(self, name, width, mult_z):
        self.name = name
        self.w = width
        self.mult_z = list(mult_z)
        self.idz = ({0, 1, 2, 3} - set(mult_z)).pop()

    # pair-table entry -> master row (pair q, entry z); identity z -> ones row
    def tab_row(self, q, z):
        if z == self.idz:
            return M_ONE
        if self.name == "H":
            return {0: M_PP + q, 1: 2 * q + 1, 2: 2 * q}[z]
        return {1: 2 * q, 2: 2 * q + 1, 3: M_PP + q}[z]

    # group-2 scaled pair-table entry (pair 8) -> master row
    def tab2_row(self, z):
        if z == self.idz:
            return M_SGN
        if self.name == "H":
            return {0: M_PPS, 1: M_SO, 2: M_SE}[z]
        return {1: M_SE, 2: M_SO, 3: M_PPS}[z]


HSPEC = SideSpec("H", F, (0, 1, 2))
GSPEC = SideSpec("G", P, (1, 2, 3))


def _sel(rows, m_pad=None):
    """Selection matrix [M_PAD, len(rows)] with one 1 per used column."""
    M = len(rows) if m_pad is None else m_pad
    s = np.zeros((M_PAD, M), np.float32)
    for m, k in enumerate(rows):
        if k is not None:
            s[k, m] = 1.0
    return s


# const pack column layout: computed once at import
def _build_pack():
    cols = {}
    blocks = []
    off = 0

    def add(name, arr):
        nonlocal off
        a = np.zeros((CROWS, arr.shape[1]), np.float32)
        a[0:arr.shape[0], :] = arr
        cols[name] = (off, arr.shape[1])
        blocks.append(a)
        off += arr.shape[1]

    # MFX: [ones;pm(10) | (ones;pm)*sF] (row 0 = ones source, row 11 = sF source)
    mF = np.concatenate([np.ones((1, F), np.float32), _pm_mask(F, FBITS)], axis=0)
    sF = _parity(F, FBITS)
    add("MFX", np.concatenate([mF, mF * sF[None, :]], axis=0))           # [22, F]
    # MPX: [pm(7) | pm*sP | ones | sP]
    mP = _pm_mask(P, PBITS)
    sP = _parity(P, PBITS)
    add("MPX", np.concatenate(
        [mP, mP * sP[None, :], np.ones((1, P), np.float32), sP[None, :]], axis=0))
    # unit columns for the widened RS matmul lhsT (cols 20,21), per side
    uh = np.zeros((22, 2), np.float32)
    uh[0, 0] = 1.0      # -> MFX row 0 (ones)
    uh[11, 1] = 1.0     # -> MFX row 11 (sF)
    ug = np.zeros((16, 2), np.float32)
    ug[14, 0] = 1.0     # -> MPX row 14 (ones)
    ug[15, 1] = 1.0     # -> MPX row 15 (sP)
    cols["_U"] = (uh, ug)

    def digits(c):
        return c % 4, (c // 4) % 4, c // 16          # za, zb, zc

    for spec in (HSPEC, GSPEC):
        nm = spec.name
        # L1: 48 distinct products tmp48 indexed by (g, c2), c2 = za + 4zb:
        # in0/in1 packed [48 | pad | 48] in one mm
        in0 = []
        in1 = []
        for g in range(3):
            for c2 in range(16):
                in0.append(spec.tab_row(3 * g, c2 % 4))
                in1.append(spec.tab_row(3 * g + 1, c2 // 4))
        add(f"SELL1_{nm}", _sel(in0 + [None] * 16 + in1, 112))
        # REP: replicate tmp48 rows into L2 src layout (lhsT for rep matmuls)
        # repA: [64g + c] <- tmp48[16g + (c % 16)] for g=0,1 ; repB: g=2
        repA = np.zeros((48, 128), np.float32)
        for g in range(2):
            for c in range(64):
                repA[16 * g + (c % 16), 64 * g + c] = 1.0
        add(f"REPA_{nm}", repA)
        repB = np.zeros((48, 64), np.float32)
        for c in range(64):
            repB[32 + (c % 16), c] = 1.0
        add(f"REPB_{nm}", repB)
        # L2 c-packs: set1 rows [64g+c] = tab_c[zc]; set2 = scaled tab2'[zc]
        rows = []
        for g in range(2):
            for c in range(64):
                rows.append(spec.tab_row(3 * g + 2, digits(c)[2]))
        add(f"SELL2A_{nm}", _sel(rows))
        rows = [spec.tab2_row(digits(c)[2]) for c in range(64)]
        add(f"SELL2B_{nm}", _sel(rows))
    return np.concatenate(blocks, axis=1), cols


CPACK, CPACK_COLS = _build_pack()
U_HG = CPACK_COLS.pop("_U")
CW = CPACK.shape[1]

def _build_packr():
    parts = []
    offs = {}
    off = 0
    for nm in ("REPA_H", "REPB_H", "REPA_G", "REPB_G"):
        o, w = CPACK_COLS[nm]
        parts.append(CPACK[0:CROWS, o:o + w])
        offs[nm] = (off, w)
        off += w
    return np.ascontiguousarray(np.concatenate(parts, axis=1)), offs


CPACKR, CPACKR_COLS = _build_packr()
CRW = CPACKR.shape[1]


def build_lts(Ar, Ai):
    """Host-side lhsT pack: per (side, comp) the widened row-sum lhsT
    (32 cols) and the pair-pack lhsT (42 cols), one [22, 296] tensor."""
    lts = np.zeros((22, 296), np.float32)
    for si, (lo, kb, K) in enumerate(((0, 11, 22), (11, 7, 16))):
        for ci, A in ((0, Ar), (1, Ai)):
            base = 74 * (2 * si + ci)
            AT = np.ascontiguousarray(A.T, dtype=np.float32)
            lts[0:kb, base:base + 18] = AT[lo:lo + kb, 0:18]
            lts[kb:2 * kb, base + 18:base + 20] = AT[lo:lo + kb, 16:18]
            if ci == 0:
                lts[0:K, base + 20:base + 22] = U_HG[si]
            pb = base + 32
            lts[0:kb, pb:pb + 9] = AT[lo:lo + kb, 0:17:2]
            lts[kb:2 * kb, pb + 9] = AT[lo:lo + kb, 16]
            lts[0:kb, pb + 32:pb + 41] = AT[lo:lo + kb, 1:18:2]
            lts[0:kb, pb + 41] = AT[lo:lo + kb, 17]
    return lts


def make_in_map(Ar, Ai):
    return {"CPACK": CPACK, "CPACKR": CPACKR, "LTS": build_lts(Ar, Ai)}


def host_consts():
    return {"CPACK": CPACK}


# ---------------------------------------------------------------- kernel body
def build_kernel(loop_iters=None):
    nc = bacc.Bacc("TRN2", target_bir_lowering=False, debug=False)

    tens = {}
    tens["LTS"] = nc.dram_tensor("LTS", [22, 296], FP32, kind="ExternalInput").ap()
    tens["CPACK"] = nc.dram_tensor("CPACK", [CROWS, CW], FP32,
                                   kind="ExternalInput").ap()
    tens["CPACKR"] = nc.dram_tensor("CPACKR", [CROWS, CRW], FP32R,
                                    kind="ExternalInput").ap()
    tens["OUT"] = nc.dram_tensor("OUT", [128, 8], FP32, kind="ExternalOutput").ap()

    with tile.TileContext(nc) as tc:
        if loop_iters is None:
            _body(nc, tc, tens)
        else:
            with tc.For_i(0, loop_iters, 1):
                _body(nc, tc, tens)
    nc.compile()
    return nc


def _body(nc, tc, tens):
    from contextlib import ExitStack

    ctx = ExitStack()
    pers = ctx.enter_context(tc.tile_pool(name="pers", bufs=1))
    pk = ctx.enter_context(tc.tile_pool(name="pk", bufs=2))
    cm = ctx.enter_context(tc.tile_pool(name="cm", bufs=2))
    psum_pool = ctx.enter_context(tc.tile_pool(name="psum", bufs=4, space="PSUM"))
    dma = nc.sync.dma_start
    dma2 = nc.gpsimd.dma_start          # SWDGE queues for small input loads

    def mmr(out_ap, lhsT_ap, rhs_ap, start=True, stop=True):
        """fp32r matmul: full rate (1 cyc/row) on trn2 when free >= 256."""
        nc.tensor.matmul(out_ap, lhsT_ap.bitcast(FP32R), rhs_ap.bitcast(FP32R),
                         start=start, stop=stop)

    def cmul6(rows, w, i0, i1, outr, outi):
        """DVE complex multiply: (i0r,i0i)*(i1r,i1i) -> (outr,outi)."""
        e = nc.vector
        i0r, i0i = i0
        i1r, i1i = i1
        t1 = cm.tile([rows, w], FP32, tag="cm_t1")
        t2 = cm.tile([rows, w], FP32, tag="cm_t2")
        e.tensor_mul(t1[:], i0r, i1r)
        e.tensor_mul(t2[:], i0i, i1i)
        e.tensor_sub(outr, t1[:], t2[:])
        e.tensor_mul(t1[:], i0r, i1i)
        e.tensor_mul(t2[:], i0i, i1r)
        e.tensor_add(outi, t1[:], t2[:])

    def cmul6p(rows, w, i0, i1, outr, outi):
        """GPSIMD complex multiply (plain tensor-tensor ops)."""
        e = nc.gpsimd
        i0r, i0i = i0
        i1r, i1i = i1
        t1 = cm.tile([rows, w], FP32, tag="gp_t1")
        t2 = cm.tile([rows, w], FP32, tag="gp_t2")
        e.tensor_mul(t1[:], i0r, i1r)
        e.tensor_mul(t2[:], i0i, i1i)
        e.tensor_sub(outr, t1[:], t2[:])
        e.tensor_mul(t1[:], i0r, i1i)
        e.tensor_mul(t2[:], i0i, i1r)
        e.tensor_add(outi, t1[:], t2[:])

    def sel_mm(sel_sb, msrc, m, w):
        """Pack = SEL.T @ master -> PSUM [m, w]."""
        ps = psum_pool.tile([m, w], FP32, tag="ps")
        for c0 in range(0, w, 512):
            c1 = min(c0 + 512, w)
            nc.tensor.matmul(ps[:, c0:c1], sel_sb[:], msrc[:, c0:c1],
                             start=True, stop=True)
        return ps

    # ---- stage 0: A loads, widened row-sum matmuls -> master rows 0..21
    cpk = pers.tile([CROWS, CW], FP32, tag="cpack")
    dma(cpk[:, 0:128], tens["CPACK"][:, 0:128])
    cpkr = pers.tile([CROWS, CRW], FP32R, tag="cpackr")
    nc.scalar.dma_start(cpkr[:], tens["CPACKR"][:, :])

    def crslice(name):
        off, width = CPACKR_COLS[name]
        return cpkr[0:48, off:off + width]
    lts = pers.tile([22, 296], FP32, tag="lts")
    dma(lts[:], tens["LTS"][:, :])
    dma(cpk[:, 128:512], tens["CPACK"][:, 128:512])
    nc.scalar.dma_start(cpk[:, 512:1024], tens["CPACK"][:, 512:1024])
    dma2(cpk[:, 1024:CW], tens["CPACK"][:, 1024:CW])

    warm = psum_pool.tile([128, 64], FP32, tag="ps")
    for _ in range(8):
        nc.tensor.matmul(warm[:], cpk[0:48, 0:128], cpk[0:48, 0:64],
                         start=True, stop=True)

    def cslice(name, nrows=None):
        off, width = CPACK_COLS[name]
        nr = M_PAD if nrows is None else nrows
        return cpk[0:nr, off:off + width]

    lhsT_rs = {}
    lhsT_pp = {}
    KRS = {"H": 22, "G": 16}
    for si, side in enumerate("HG"):
        for ci, nm in enumerate("ri"):
            base = 74 * (2 * si + ci)
            K_rs = KRS[side]
            lhsT_rs[(side, nm)] = lts[0:K_rs, base:base + 32]
            lhsT_pp[(side, nm)] = lts[0:K_rs, base + 32:base + 74]

    mask_sb = {"H": cslice("MFX", 22), "G": cslice("MPX", 16)}
    sel_sb = {}
    for spec in (HSPEC, GSPEC):
        for s in ("SELL1", "SELL2A", "SELL2B"):
            key = f"{s}_{spec.name}"
            sel_sb[key] = cslice(key)
        for s in ("REPA", "REPB"):
            key = f"{s}_{spec.name}"
            sel_sb[key] = cslice(key, 48)

    # H masters: one [M_PAD, F] tile per component; G master: [M_PAD, 2P]
    # with real in cols 0:P, imag in P:2P. All 32 rows get written (22 by
    # the RS copy, 10 by the pair stage) -- no memset needed.
    masterH = {}
    for nm in "ri":
        t = pers.tile([M_PAD, F], FP32, tag=f"mstH{nm}", name=f"mstH{nm}")
        masterH[nm] = t
    masterG = pers.tile([M_PAD, 2 * P], FP32, tag="mstG")

    # ---- stage 1: pair products -> master rows 32..41 (packs come
    # straight from the rearranged ltp lhsTs -- no master dependency)
    P1T = F - FS_P1
    psH1 = {}
    for nm in "ri":
        ps = sel_mm(lhsT_pp[("H", nm)], mask_sb["H"], 42, F)
        sb = pk.tile([10, F], FP32, tag=f"halfH{nm}")
        if nm == "r":
            nc.scalar.copy(sb[:], ps[32:42, :])
        else:
            nc.vector.tensor_copy(sb[:], ps[32:42, :])
        p0t = pk.tile([10, P1T], FP32, tag=f"p0tH{nm}")
        nc.scalar.copy(p0t[:], ps[0:10, FS_P1:F])
        psH1[nm] = (ps, sb, p0t)
    cmul6(10, FS_P1,
          (psH1["r"][0][0:10, 0:FS_P1], psH1["i"][0][0:10, 0:FS_P1]),
          (psH1["r"][1][:, 0:FS_P1], psH1["i"][1][:, 0:FS_P1]),
          masterH["r"][M_PP:M_PP + 10, 0:FS_P1],
          masterH["i"][M_PP:M_PP + 10, 0:FS_P1])
    cmul6p(10, P1T,
           (psH1["r"][2][:], psH1["i"][2][:]),
           (psH1["r"][1][:, FS_P1:F], psH1["i"][1][:, FS_P1:F]),
           masterH["r"][M_PP:M_PP + 10, FS_P1:F],
           masterH["i"][M_PP:M_PP + 10, FS_P1:F])

    psG1 = psum_pool.tile([42, 2 * P], FP32, tag="ps")
    nc.tensor.matmul(psG1[:, 0:P], lhsT_pp[("G", "r")], mask_sb["G"][:],
                     start=True, stop=True)
    nc.tensor.matmul(psG1[:, P:2 * P], lhsT_pp[("G", "i")], mask_sb["G"][:],
                     start=True, stop=True)
    sbG1a = pk.tile([10, 2 * P], FP32, tag="selpGa")
    nc.scalar.copy(sbG1a[:], psG1[0:10, :])
    sbG1b = pk.tile([10, 2 * P], FP32, tag="selpGb")
    nc.scalar.copy(sbG1b[:], psG1[32:42, :])
    cmul6p(10, P,
           (sbG1a[:, 0:P], sbG1a[:, P:2 * P]),
           (sbG1b[:, 0:P], sbG1b[:, P:2 * P]),
           masterG[M_PP:M_PP + 10, 0:P], masterG[M_PP:M_PP + 10, P:2 * P])

    # row-sum masters (needed from L1 onward; emitted after the pair
    # stage so its PSUM evacuations win the ACT queue early)
    for nm in "ri":
        lt = lhsT_rs[("H", nm)]
        ps = psum_pool.tile([32, F], FP32, tag="ps")
        for c0 in range(0, F, 512):
            c1 = min(c0 + 512, F)
            nc.tensor.matmul(ps[:, c0:c1], lt, mask_sb["H"][:, c0:c1],
                             start=True, stop=True)
        nc.scalar.copy(masterH[nm][0:32, :], ps[:])
    psG = psum_pool.tile([32, 2 * P], FP32, tag="ps")
    nc.tensor.matmul(psG[:, 0:P], lhsT_rs[("G", "r")], mask_sb["G"][:],
                     start=True, stop=True)
    nc.tensor.matmul(psG[:, P:2 * P], lhsT_rs[("G", "i")], mask_sb["G"][:],
                     start=True, stop=True)
    nc.scalar.copy(masterG[0:32, :], psG[:])


    # ---- stage 2 (L1): tmp48[16g + c2] = tab_a[za] * tab_b[zb]
    t48H = {}
    l1t = {}
    for nm in "ri":
        ps = sel_mm(sel_sb["SELL1_H"], masterH[nm][:], 112, F)
        sb1 = pk.tile([48, F], FP32, tag=f"l1hH{nm}")
        if nm == "r":
            nc.scalar.copy(sb1[:], ps[64:112, :])
        else:
            nc.vector.tensor_copy(sb1[:], ps[64:112, :])
        lt1 = pk.tile([48, P1T], FP32, tag=f"l1tH{nm}")
        nc.scalar.copy(lt1[:], ps[0:48, FS_P1:F])
        l1t[nm] = lt1
        t48 = pers.tile([48, F], FP32R, tag=f"t48H{nm}", name=f"t48H{nm}")
        t48H[nm] = (ps, sb1, t48)
    cmul6(48, FS_P1,
          (t48H["r"][0][0:48, 0:FS_P1], t48H["i"][0][0:48, 0:FS_P1]),
          (t48H["r"][1][:, 0:FS_P1], t48H["i"][1][:, 0:FS_P1]),
          t48H["r"][2][:, 0:FS_P1], t48H["i"][2][:, 0:FS_P1])
    cmul6p(48, P1T,
           (l1t["r"][:], l1t["i"][:]),
           (t48H["r"][1][:, FS_P1:F], t48H["i"][1][:, FS_P1:F]),
           t48H["r"][2][:, FS_P1:F], t48H["i"][2][:, FS_P1:F])

    psL1G = sel_mm(sel_sb["SELL1_G"], masterG[:], 112, 2 * P)
    sbL1Ga = pk.tile([48, 2 * P], FP32, tag="l1Ga")
    nc.scalar.copy(sbL1Ga[:], psL1G[0:48, :])
    sbL1Gb = pk.tile([48, 2 * P], FP32, tag="l1Gb")
    nc.scalar.copy(sbL1Gb[:], psL1G[64:112, :])
    t48G = pers.tile([48, 2 * P], FP32R, tag="t48G", name="t48G")
    cmul6p(48, P,
           (sbL1Ga[:, 0:P], sbL1Ga[:, P:2 * P]),
           (sbL1Gb[:, 0:P], sbL1Gb[:, P:2 * P]),
           t48G[:, 0:P], t48G[:, P:2 * P])

    # ---- stage 3 (L2): e_g = tmp * tab_c[zc], column-split DVE / GPSIMD
    # H g0+g1 fused as one [128, F] set -> eRH = [H0r; H1r], eIH = [H0i; H1i].
    # The T matmuls compensate with K=64 accumulating pairs.
    TL = F - FS_L2
    c01H = {}
    repH = {}
    reptH = {}
    for nm in "ri":
        c01 = sel_mm(sel_sb["SELL2A_H"], masterH[nm][:], 128, F)
        c01sb = pk.tile([128, F], FP32, tag=f"c01H{nm}")
        if nm == "r":
            nc.scalar.copy(c01sb[:], c01[:])
        else:
            nc.vector.tensor_copy(c01sb[:], c01[:])
        c01H[nm] = c01sb
        rep = psum_pool.tile([128, F], FP32, tag="ps")
        for c0 in range(0, F, 512):
            mmr(rep[:, c0:c0 + 512], crslice("REPA_H"),
                t48H[nm][2][:, c0:c0 + 512])
        repH[nm] = rep
        rt = pk.tile([128, TL], FP32, tag=f"reptH{nm}")
        nc.scalar.copy(rt[:], repH[nm][:, FS_L2:F])
        reptH[nm] = rt
    eRH = pers.tile([128, F], FP32R, tag="eRH", name="eRH")
    eIH = pers.tile([128, F], FP32R, tag="eIH", name="eIH")
    cmul6(128, FS_L2,
          (repH["r"][0:128, 0:FS_L2], repH["i"][0:128, 0:FS_L2]),
          (c01H["r"][:, 0:FS_L2], c01H["i"][:, 0:FS_L2]),
          eRH[:, 0:FS_L2], eIH[:, 0:FS_L2])
    cmul6p(128, TL,
           (reptH["r"][:], reptH["i"][:]),
           (c01H["r"][:, FS_L2:F], c01H["i"][:, FS_L2:F]),
           eRH[:, FS_L2:F], eIH[:, FS_L2:F])

    cp2H = {}
    rbH = {}
    rbtH = {}
    for nm in "ri":
        c2 = sel_mm(sel_sb["SELL2B_H"], masterH[nm][:], 64, F)
        c2sb = pk.tile([64, F], FP32, tag=f"c2H{nm}")
        nc.scalar.copy(c2sb[:], c2[:])
        cp2H[nm] = c2sb
        rb = psum_pool.tile([64, F], FP32, tag="ps")
        for c0 in range(0, F, 512):
            mmr(rb[:, c0:c0 + 512], crslice("REPB_H"),
                t48H[nm][2][:, c0:c0 + 512])
        rbH[nm] = rb
        rbt = pk.tile([64, TL], FP32, tag=f"rbtH{nm}")
        nc.scalar.copy(rbt[:], rb[:, FS_L2:F])
        rbtH[nm] = rbt
    eR2H = pers.tile([64, F], FP32R, tag="eR2H", name="eR2H")
    eI2H = pers.tile([64, F], FP32R, tag="eI2H", name="eI2H")
    cmul6(64, FS_L2,
          (rbH["r"][:, 0:FS_L2], rbH["i"][:, 0:FS_L2]),
          (cp2H["r"][:, 0:FS_L2], cp2H["i"][:, 0:FS_L2]),
          eR2H[:, 0:FS_L2], eI2H[:, 0:FS_L2])
    cmul6p(64, TL,
           (rbtH["r"][:], rbtH["i"][:]),
           (cp2H["r"][:, FS_L2:F], cp2H["i"][:, FS_L2:F]),
           eR2H[:, FS_L2:F], eI2H[:, FS_L2:F])

    # G side (GPSIMD, SBUF operands via single ACT evacuations)
    repG = psum_pool.tile([128, 2 * P], FP32, tag="ps")
    mmr(repG[:], crslice("REPA_G"), t48G[:])
    repGsb = pk.tile([128, 2 * P], FP32, tag="repG")
    nc.scalar.copy(repGsb[:], repG[:])
    c01G = sel_mm(sel_sb["SELL2A_G"], masterG[:], 128, 2 * P)
    c01Gsb = pk.tile([128, 2 * P], FP32, tag="c01G")
    nc.scalar.copy(c01Gsb[:], c01G[:])
    eG01 = pers.tile([128, 2 * P], FP32R, tag="eG01", name="eG01")
    cmul6p(128, P,
           (repGsb[:, 0:P], repGsb[:, P:2 * P]),
           (c01Gsb[:, 0:P], c01Gsb[:, P:2 * P]),
           eG01[:, 0:P], eG01[:, P:2 * P])

    rbG = psum_pool.tile([64, 2 * P], FP32, tag="ps")
    mmr(rbG[:], crslice("REPB_G"), t48G[:])
    rbGsb = pk.tile([64, 2 * P], FP32, tag="rbG")
    nc.scalar.copy(rbGsb[:], rbG[:])
    c2G = sel_mm(sel_sb["SELL2B_G"], masterG[:], 64, 2 * P)
    c2Gsb = pk.tile([64, 2 * P], FP32, tag="c2G")
    nc.scalar.copy(c2Gsb[:], c2G[:])
    eG2 = pers.tile([64, 2 * P], FP32R, tag="eG2", name="eG2")
    cmul6p(64, P,
           (rbGsb[:, 0:P], rbGsb[:, P:2 * P]),
           (c2Gsb[:, 0:P], c2Gsb[:, P:2 * P]),
           eG2[:, 0:P], eG2[:, P:2 * P])

    # negated imag halves (lhsT for the real-part T matmuls)
    negG01 = pers.tile([128, P], FP32R, tag="negG01")
    nc.scalar.mul(negG01[:], eG01[:, P:2 * P], -1.0)
    negG2 = pers.tile([64, P], FP32R, tag="negG2")
    nc.scalar.mul(negG2[:], eG2[:, P:2 * P], -1.0)

    # ---- stage 4: T matmuls -- per (group, comp, chunk) a K=64 pair
    # accumulated in PSUM: Tr = Gr^T Hr + (-Gi)^T Hi ; Ti = Gr^T Hi + Gi^T Hr
    # Order: T0, T1 (combine inputs) first, then T2 (only needed by the
    # final reduction) so p01 overlaps the T2 matmuls.
    def t_mms(g):
        if g < 2:
            Gr = eG01[64 * g:64 * g + 64, 0:P]
            Gi = eG01[64 * g:64 * g + 64, P:2 * P]
            Gin = negG01[64 * g:64 * g + 64, :]
            Hr = eRH[64 * g:64 * g + 64, :]
            Hi = eIH[64 * g:64 * g + 64, :]
        else:
            Gr = eG2[:, 0:P]
            Gi = eG2[:, P:2 * P]
            Gin = negG2[:]
            Hr = eR2H[:]
            Hi = eI2H[:]
        tr = psum_pool.tile([P, F], FP32, tag="ps")
        ti = psum_pool.tile([P, F], FP32, tag="ps")
        for c0 in range(0, F, 512):
            c1 = c0 + 512
            mmr(tr[:, c0:c1], Gr, Hr[:, c0:c1], start=True, stop=False)
            mmr(tr[:, c0:c1], Gin, Hi[:, c0:c1], start=False, stop=True)
            mmr(ti[:, c0:c1], Gr, Hi[:, c0:c1], start=True, stop=False)
            mmr(ti[:, c0:c1], Gi, Hr[:, c0:c1], start=False, stop=True)
        return tr, ti

    t1r_ps, t1i_ps = t_mms(1)
    t0r, t0i = t_mms(0)
    t1r = pers.tile([P, F], FP32, tag="T1r")
    t1i = pers.tile([P, F], FP32, tag="T1i")
    for c0 in range(0, F, 512):
        c1 = c0 + 512
        nc.scalar.copy(t1r[:, c0:c1], t1r_ps[:, c0:c1])
        nc.vector.tensor_copy(t1i[:, c0:c1], t1i_ps[:, c0:c1])

    # p01 = T0*T1: col-split DVE (T0 from PSUM) / GPSIMD (T0 tail via ACT)
    TP = F - FS_PF
    p01r = pers.tile([P, F], FP32, tag="p01r")
    p01i = pers.tile([P, F], FP32, tag="p01i")
    t0tr = pers.tile([P, TP], FP32, tag="t0tr")
    t0ti = pers.tile([P, TP], FP32, tag="t0ti")
    nc.scalar.copy(t0tr[:], t0r[:, FS_PF:F])
    nc.scalar.copy(t0ti[:], t0i[:, FS_PF:F])
    cmul6(P, FS_PF,
          (t0r[:, 0:FS_PF], t0i[:, 0:FS_PF]),
          (t1r[:, 0:FS_PF], t1i[:, 0:FS_PF]),
          p01r[:, 0:FS_PF], p01i[:, 0:FS_PF])
    cmul6p(P, TP,
           (t0tr[:], t0ti[:]),
           (t1r[:, FS_PF:F], t1i[:, FS_PF:F]),
           p01r[:, FS_PF:F], p01i[:, FS_PF:F])

    t2r, t2i = t_mms(2)

    # ---- final reduction, engine-split: DVE runs STT-accum over cols
    # 0:XT (T2 straight from PSUM); for the tail GPSIMD forms the products
    # and ACT accumulates them (activation Copy with accum_out).
    # Host combines: perm_r = (c0-c1)+(c4-c5), perm_i = (c2+c3)+(c6+c7)
    XT = 576
    TT2 = F - XT
    t2tr = pers.tile([P, TT2], FP32, tag="t2tr")
    t2ti = pers.tile([P, TT2], FP32, tag="t2ti")
    nc.scalar.copy(t2tr[:], t2r[:, XT:F])
    nc.scalar.copy(t2ti[:], t2i[:, XT:F])
    scr2 = pers.tile([P, XT], FP32, tag="ttr_scr")
    accD = pers.tile([P, 4], FP32, tag="accD")
    accA = pers.tile([P, 4], FP32, tag="accA")
    pairs = [(p01r, t2r, t2tr), (p01i, t2i, t2ti), (p01r, t2i, t2ti),
             (p01i, t2r, t2tr)]
    wo = pers.tile([P, TT2], FP32, tag="two")
    for k, (a, b, bt) in enumerate(pairs):
        nc.vector.scalar_tensor_tensor(
            out=scr2[:], in0=b[:, 0:XT], scalar=1.0, in1=a[:, 0:XT],
            op0=OP.mult, op1=OP.mult, accum_out=accD[:, k:k + 1])
        wk = pk.tile([P, TT2], FP32, tag="tw")
        nc.gpsimd.tensor_mul(wk[:], bt[:], a[:, XT:F])
        nc.scalar.activation(wo[:], wk[:], mybir.ActivationFunctionType.Copy,
                             accum_out=accA[:, k:k + 1])

    dma(tens["OUT"][:, 0:4], accD[:])
    dma(tens["OUT"][:, 4:8], accA[:])

    ctx.close()


# ---------------------------------------------------------------- entry point
def kernel(A_real: np.ndarray, A_imag: np.ndarray) -> np.ndarray:
    B = A_real.shape[0]
    assert B == 8 and A_real.shape == (B, N, N)
    if "nc" not in _CACHE:
        _CACHE["nc"] = build_kernel()
    nc = _CACHE["nc"]
    in_maps = [make_in_map(A_real[b], A_imag[b]) for b in range(B)]
    res = run_bass_kernel_spmd(nc, in_maps, list(range(B)))
    out = np.empty(B, dtype=np.float32)
    for b in range(B):
        acc = res.results[b]["OUT"].reshape(128, 8).astype(np.float64)
        s = acc.sum(axis=0)
        pr = (s[0] - s[1]) + (s[4] - s[5])
        pi = (s[2] + s[3]) + (s[6] + s[7])
        pa2 = np.float32(pr) ** 2 + np.float32(pi) ** 2
        out[b] = np.float32(EMU * SCALE2 * pa2 + DARK)
    return out


if __name__ == "__main__":
    A_real = np.load("/tmp/A_real.npy")
    A_imag = np.load("/tmp/A_imag.npy")
    print(kernel(A_real, A_imag))
